# revision 59
# baseline (speedup 1.0000x reference)
"""DeltaNet-style block (nn_DeltaNet_31877247271438) on 8 trn2 NeuronCores.

Sharding: core c -> (batch b = c//2, pair-rank r = c%2).  Within a batch pair:
  - head-parallel: rank r owns heads {2r, 2r+1} (feature cols [512r, 512r+512))
  - cross-head mixes are K-split with pairwise collectives:
      * channel_mixer (folded with kernel_mix into one matrix Q): partial sums
        ReduceScatter'ed (fp16) so each core receives its own heads' ms_out
      * fusion-MLP hidden is column-split; logits partials ReduceScatter'ed
      * bn features AllGather'ed (tiny)
  - the final Wo matmul partials are summed on the host.

v2: inter-phase tensors stay in SBUF (no DRAM staging round-trips); l-major
forms of k/v come from DMA-xbar transposes; the delta rule's 16 chunk
inversions are batched (only the S recurrence is serial); beta is folded
into the mask multiply so the kb row form is never materialized; a manually
aliased SBUF arena lets phase-scoped tensors (hsT/xc, y, u/wT/aT, hdn_pre)
share the same bytes.
"""
import sys
sys.path.insert(0, '/opt/trn_rl_repo')

import numpy as np
import ml_dtypes

import concourse.bass as bass
import concourse.tile as tile
from concourse import bacc, mybir
from concourse.bass_utils import run_bass_kernel_spmd

F32 = mybir.dt.float32
F16 = mybir.dt.float16
F8 = mybir.dt.float8e4
DR = mybir.MatmulPerfMode.DoubleRow
FP8S = 64.0
AF = mybir.ActivationFunctionType
ALU = mybir.AluOpType

B, L, D, H = 4, 2048, 1024, 4
d = 256          # per-head dim
C = 512          # channels owned per core (2 heads)
NLT = 16         # l-tiles of 128
NLW = 4          # l-windows of 512
NCH = 16         # delta chunks of 128
KQKV = 4         # qkv conv taps
MSK = (3, 15, 31)
NTAPS = sum(MSK)  # 49
PADV = 32
RG = [[0, 1], [2, 3], [4, 5], [6, 7]]

ARENA = 36864    # f16 elems per partition in the aliased arena


def bc_mid(ap2, n):
    """[P, F] AP -> [P, n, F] with a 0-stride middle dim (free-dim bcast)."""
    assert len(ap2.ap) == 2
    return bass.AP(tensor=ap2.tensor, offset=ap2.offset,
                   ap=[ap2.ap[0], [0, n], ap2.ap[1]])


def build_program(debug=False):
    nc = bacc.Bacc("TRN2", target_bir_lowering=False, debug=False,
                   num_devices=8)

    io = {}
    io["hsT"] = nc.declare_dram_parameter("hsT", [D, L], F16, False)
    io["wq"] = nc.declare_dram_parameter("wq", [D, C], F16, False)
    io["wk"] = nc.declare_dram_parameter("wk", [D, C], F16, False)
    io["wv"] = nc.declare_dram_parameter("wv", [D, C], F16, False)
    io["wb"] = nc.declare_dram_parameter("wb", [D, 2], F16, False)
    io["cdiag"] = nc.declare_dram_parameter("cdiag", [3, 4, KQKV, 128, 128],
                                            F16, False)
    io["mspair"] = nc.declare_dram_parameter("mspair", [4, 23, 2, 128, 128],
                                             F8, False)
    io["mssing"] = nc.declare_dram_parameter("mssing", [4, 3, 128, 128],
                                             F8, False)
    io["qmix8"] = nc.declare_dram_parameter("qmix8", [12 * 128, D], F8, False)
    io["fw1h"] = nc.declare_dram_parameter("fw1h", [D, 1024], F16, False)
    io["fw1b"] = nc.declare_dram_parameter("fw1b", [16, 1024], F16, False)
    io["fb1"] = nc.declare_dram_parameter("fb1", [1024], F32, False)
    io["fw2"] = nc.declare_dram_parameter("fw2", [1024, 12], F16, False)
    io["b2o"] = nc.declare_dram_parameter("b2o", [128, 6], F32, False)
    io["wo"] = nc.declare_dram_parameter("wo", [C, D], F16, False)
    io["masks"] = nc.declare_dram_parameter("masks", [5, 128, 128], F32, False)
    io["onesrow"] = nc.declare_dram_parameter("onesrow", [1, 128], F32, False)
    io["onescol"] = nc.declare_dram_parameter("onescol", [128, 1], F32, False)
    io["ident16"] = nc.declare_dram_parameter("ident16", [128, 128], F16, False)
    io["out_part"] = nc.declare_dram_parameter("out_part", [L, D], F32, True)

    sc = {}
    sc["dout_s"] = nc.dram_tensor("dout_s", [L, C], F16)
    sc["cm_in"] = nc.dram_tensor("cm_in", [2, L, C], F16)
    sc["cm_out"] = nc.dram_tensor("cm_out", [L, C], F16)
    sc["bn_in"] = nc.dram_tensor("bn_in", [L, 8], F32)
    sc["bn_out"] = nc.dram_tensor("bn_out", [2, L, 8], F32)
    sc["lg_in"] = nc.dram_tensor("lg_in", [2, L, 6], F32)
    sc["lg_out"] = nc.dram_tensor("lg_out", [L, 6], F32)

    with tile.TileContext(nc) as tc:
        _body(nc, tc, io, sc)
    nc.compile()
    return nc


def _body(nc, tc, io, sc):
    from contextlib import ExitStack
    ctx = ExitStack()
    with ctx:
        consts = ctx.enter_context(tc.tile_pool(name="consts", bufs=1))
        outer = ctx.enter_context(tc.tile_pool(name="outer", bufs=1))

        masks = consts.tile([128, 5, 128], F32)
        nc.sync.dma_start(out=masks,
                          in_=io["masks"][:].rearrange("m p f -> p m f"))
        ident = masks[:, 4, :]
        ident16 = consts.tile([128, 128], F16)
        nc.sync.dma_start(out=ident16, in_=io["ident16"][:])
        onescol16 = consts.tile([128, 1], F16)
        nc.vector.memset(onescol16, 1.0)
        onesrow16 = consts.tile([1, 128], F16)
        nc.vector.memset(onesrow16, 1.0)
        eps6 = consts.tile([128, 1], F32)
        nc.vector.memset(eps6, 1e-6)
        eps5 = consts.tile([128, 1], F32)
        nc.vector.memset(eps5, 1e-5)

        beta_lp = outer.tile([128, NLT, 2], F32)
        S16 = outer.tile([128, 2, 2, d], F16)
        nc.vector.memset(S16, 0.0)
        qT = outer.tile([128, 2, 2, L], F16)       # (h, dtile, l) row form
        kT = outer.tile([128, 2, 2, L], F16)
        klc = outer.tile([128, 2, 2, NLT, 128], F16)  # (h, dtile, lt, dk)
        vlc = outer.tile([128, 2, 2, NLT, 128], F16)
        bn_sb = outer.tile([128, NLT, 8], F32)
        bnv4 = outer.tile([128, NLT, 4], F32)
        arena = outer.tile([128, ARENA], F16)

        # arena regions (manually aliased, lifetimes disjoint)
        hsT = arena[:, 0:16384].rearrange("p (kt l) -> p kt l", l=L)
        xc = arena[:, 16384:24592].rearrange("p (ct l) -> p ct l", l=L + 4)
        vt_bf = arena[:, 24592:32912].rearrange("p (ct l) -> p ct l",
                                                l=PADV + L)
        y_bf = arena[:, 0:24576].rearrange("p (j l) -> p j l", l=L)
        u_all = arena[:, 0:8192].rearrange("p (n h e) -> p n h e", h=2, e=d)
        wT_all = arena[:, 8192:16384].rearrange("p (n kt h e) -> p n kt h e",
                                                kt=2, h=2, e=128)
        aT_all = arena[:, 16384:20480].rearrange("p (n h e) -> p n h e",
                                                 h=2, e=128)
        hdn_pre = arena[:, 20480:36864].rearrange("p (mt l) -> p mt l", l=L)

        hsT_r = io["hsT"][:].rearrange("(kt p) l -> p kt l", p=128)

        # =================== PHASE A ======================================
        with tc.tile_pool(name="pa1", bufs=1) as pa1, \
             tc.tile_pool(name="pa2", bufs=2) as pa2, \
             tc.tile_pool(name="pas", bufs=3) as pas, \
             tc.tile_pool(name="psA", bufs=1, space="PSUM") as psA:
            nc.sync.dma_start(out=hsT, in_=hsT_r)

            # ---- beta (l-partition form) ---------------------------------
            wb_sb = pa1.tile([128, 8, 2], F16)
            nc.sync.dma_start(
                out=wb_sb, in_=io["wb"][:].rearrange("(kt p) c -> p kt c",
                                                     p=128))
            for lt in range(NLT):
                pb = psA.tile([128, 2], F32, tag="pb", bufs=1, name="psb")
                for kt in range(8):
                    nc.tensor.matmul(pb, hsT[:, kt, lt*128:(lt+1)*128],
                                     wb_sb[:, kt, :],
                                     start=(kt == 0), stop=(kt == 7))
                nc.scalar.activation(out=beta_lp[:, lt, :], in_=pb,
                                     func=AF.Sigmoid)

            # ---- q, k, v: proj -> conv -> silu -> (norm) -----------------
            for tnm, ti in (("v", 2), ("q", 0), ("k", 1)):
                PAD = 4
                w_sb = pa2.tile([128, 8, C], F16, tag="w_sb", bufs=2,
                                name=f"w_{tnm}")
                nc.sync.dma_start(
                    out=w_sb,
                    in_=io["w" + tnm][:].rearrange("(kt p) c -> p kt c",
                                                   p=128))
                cdg = pa2.tile([128, 4, KQKV, 128], F16, tag="cdg", bufs=2,
                               name=f"cdg_{tnm}")
                nc.sync.dma_start(
                    out=cdg,
                    in_=io["cdiag"][ti].rearrange("ct tap p f -> p ct tap f"))
                if tnm != "v":
                    nc.vector.memset(xc[:, :, 0:4], 0.0)
                else:
                    nc.vector.memset(vt_bf[:, :, 0:PADV], 0.0)

                for ct in range(4):
                    xp = pa2.tile([128, 4 + L], F16, tag="xp", bufs=2,
                                  name=f"xp_{tnm}{ct}")
                    nc.vector.memset(xp[:, 0:4], 0.0)
                    for lw in range(NLW):
                        pp = psA.tile([128, 512], F32, tag="pp", bufs=2,
                                      name="psp")
                        for kt in range(8):
                            nc.tensor.matmul(
                                pp, w_sb[:, kt, ct*128:(ct+1)*128],
                                hsT[:, kt, lw*512:(lw+1)*512],
                                start=(kt == 0), stop=(kt == 7))
                        nc.scalar.copy(out=xp[:, 4+lw*512:4+(lw+1)*512],
                                       in_=pp)
                    for lw in range(NLW):
                        pc = psA.tile([128, 512], F32, tag="pc", bufs=2,
                                      name="psc")
                        for dd in range(KQKV):
                            off = 4 + lw*512 - dd
                            nc.tensor.matmul(
                                pc, cdg[:, ct, dd, :], xp[:, off:off+512],
                                start=(dd == 0), stop=(dd == KQKV-1))
                        if tnm == "v":
                            nc.scalar.activation(
                                out=vt_bf[:, ct, PADV+lw*512:PADV+(lw+1)*512],
                                in_=pc, func=AF.Silu)
                        else:
                            nc.scalar.activation(
                                out=xc[:, ct, PAD+lw*512:PAD+(lw+1)*512],
                                in_=pc, func=AF.Silu)
                if tnm == "v":
                    for ct in range(4):
                        nc.sync.dma_start_transpose(
                            out=vlc[:, ct // 2, ct % 2],
                            in_=vt_bf[:, ct, PADV:PADV+L])
                    continue

                # ---- l2norm -> qT / kT row forms -------------------------
                dst = qT if tnm == "q" else kT
                for h in range(2):
                    for lw in range(NLW):
                        lsl = slice(PAD+lw*512, PAD+(lw+1)*512)
                        osl = slice(lw*512, (lw+1)*512)
                        sqs = pas.tile([128, 2, 512], F16, tag="sq", bufs=3,
                                       name="sq")
                        for i, ct in enumerate((2*h, 2*h+1)):
                            nc.vector.tensor_tensor(
                                out=sqs[:, i, :], in0=xc[:, ct, lsl],
                                in1=xc[:, ct, lsl], op=ALU.mult)
                        ssr = pas.tile([1, 2, 512], F32, tag="ssr", bufs=2,
                                       name="ssr")
                        nc.gpsimd.tensor_reduce(
                            out=ssr, in_=sqs, axis=mybir.AxisListType.C,
                            op=ALU.add)
                        sr = pas.tile([1, 512], F32, tag="sr", bufs=2,
                                      name="sr")
                        nc.vector.tensor_tensor(out=sr, in0=ssr[:, 0, :],
                                                in1=ssr[:, 1, :], op=ALU.add)
                        nc.scalar.activation(out=sr, in_=sr, func=AF.Sqrt,
                                             bias=eps6[0:1, :])
                        srt = pas.tile([1, 512], F16, tag="srt", bufs=2,
                                       name="srt")
                        with nc.allow_low_precision("l2norm scale fp16"):
                            nc.vector.reciprocal(out=srt, in_=sr)
                        pbc = psA.tile([128, 512], F32, tag="pn2", bufs=2,
                                       name="psbc")
                        nc.tensor.matmul(pbc, onesrow16, srt,
                                         start=True, stop=True)
                        for dt in range(2):
                            ct = 2*h + dt
                            nc.vector.tensor_tensor(
                                out=dst[:, h, dt, osl], in0=xc[:, ct, lsl],
                                in1=pbc, op=ALU.mult)
                if tnm == "k":
                    for h in range(2):
                        for dt in range(2):
                            nc.sync.dma_start_transpose(
                                out=klc[:, h, dt], in_=kT[:, h, dt, :])

            # bn(v): |v| partial sums (finalized later)
            for lt in range(NLT):
                nc.vector.tensor_reduce(
                    out=bnv4[:, lt, :], in_=vlc[:, :, :, lt, :],
                    axis=mybir.AxisListType.X, op=ALU.add,
                    apply_absolute_value=True)

        # =================== PHASE B: multiscale conv + qmix ==============
        with tc.tile_pool(name="pb1", bufs=1) as pb1, \
             tc.tile_pool(name="pbm", bufs=2) as pbm, \
             tc.tile_pool(name="pbs", bufs=3) as pbs, \
             tc.tile_pool(name="psB", bufs=1, space="PSUM") as psB:
            y8 = arena[:, 0:12288].bitcast(F8).rearrange(
                "p (j l) -> p j l", l=L)
            vt8 = arena[:, 12288:16448].bitcast(F8).rearrange(
                "p (ct l) -> p ct l", l=PADV + L)
            with nc.allow_low_precision("fp8 conv input"):
                for ct in range(4):
                    nc.vector.tensor_copy(out=vt8[:, ct, :],
                                          in_=vt_bf[:, ct, :])

            def pair_ap(off, n, ct):
                a = vt8[:, ct, off:off+n]
                return bass.AP(tensor=a.tensor, offset=a.offset,
                               ap=[a.ap[0], [1, 2], a.ap[1]])

            for ct in range(4):
                msd8 = pbm.tile([128, 23, 2, 128], F8, tag=f"msd{ct % 2}",
                                bufs=1, name=f"msd8_{ct}")
                nc.gpsimd.dma_start(
                    out=msd8,
                    in_=io["mspair"][ct].rearrange("j t p f -> p j t f"))
                mss8 = pbm.tile([128, 3, 128], F8, tag=f"mss{ct % 2}",
                                bufs=1, name=f"mss8_{ct}")
                nc.gpsimd.dma_start(
                    out=mss8,
                    in_=io["mssing"][ct].rearrange("j p f -> p j f"))
                for lw in range(NLW):
                    base_pi = 0
                    for si, ks in enumerate(MSK):
                        npair = (ks - 1) // 2
                        py = psB.tile([128, 512], F32, tag="py", bufs=4,
                                      name="psy")
                        for p_ in range(npair):
                            off = PADV + lw*512 - (2*p_ + 1)
                            nc.tensor.matmul(
                                py, msd8[:, base_pi + p_, :, :],
                                pair_ap(off, 512, ct),
                                start=(p_ == 0), stop=False, perf_mode=DR)
                        off = PADV + lw*512 - (ks - 1)
                        nc.tensor.matmul(py, mss8[:, si, :],
                                         vt8[:, ct, off:off+512],
                                         start=False, stop=True)
                        nc.scalar.mul(
                            out=y8[:, si*4+ct, lw*512:(lw+1)*512],
                            in_=py, mul=1.0/FP8S)
                        base_pi += npair

            qmix8_sb = pb1.tile([128, 6, 2, D], F8)
            nc.gpsimd.dma_start(
                out=qmix8_sb,
                in_=io["qmix8"][:].rearrange("(pp j p) o -> p pp j o",
                                             j=2, p=128))
            cms_v = [arena[:, 33040+i*1024:33040+(i+1)*1024].rearrange(
                "p (o c) -> p o c", o=2) for i in range(3)]
            for lt in range(NLT):
                cms = cms_v[lt % 3]
                for oh in range(2):
                    pq = psB.tile([128, 512], F32, tag="pq", bufs=4,
                                  name="psq")
                    for p_ in range(6):
                        nc.tensor.matmul(
                            pq, y8[:, 2*p_:2*p_+2, lt*128:(lt+1)*128],
                            qmix8_sb[:, p_, :, oh*512:(oh+1)*512],
                            start=(p_ == 0), stop=(p_ == 5), perf_mode=DR)
                    if oh == 0:
                        nc.vector.tensor_scalar_mul(cms[:, oh, :], pq,
                                                    1.0/FP8S)
                    else:
                        nc.scalar.mul(out=cms[:, oh, :], in_=pq,
                                      mul=1.0/FP8S)
                nc.gpsimd.dma_start(
                    out=sc["cm_in"][:, lt*128:(lt+1)*128, :].rearrange(
                        "o l c -> l o c"),
                    in_=cms)
            nc.gpsimd.collective_compute(
                "ReduceScatter", ALU.add, replica_groups=RG,
                ins=[sc["cm_in"][:]], outs=[sc["cm_out"][:]])

        # =================== PHASE B3: delta rule =========================
        with tc.tile_pool(name="pd1", bufs=1) as pd1, \
             tc.tile_pool(name="pdc", bufs=1) as pdc, \
             tc.tile_pool(name="pdw", bufs=1) as pdw, \
             tc.tile_pool(name="psD", bufs=1, space="PSUM") as psD:

            def blk(name, bufs=2):
                return pdc.tile([128, 2, 128], F16, tag=name, name=name,
                                bufs=bufs)

            def pd_(name):
                return psD.tile([128, 2, 128], F32, tag="pd", bufs=3,
                                name=name)

            def mm2(pt, lhs_fn, rhs_fn, n_k=1):
                for h in range(2):
                    for kt in range(n_k):
                        nc.tensor.matmul(pt[:, h, :], lhs_fn(h, kt),
                                         rhs_fn(h, kt),
                                         start=(kt == 0), stop=(kt == n_k-1))

            GRP = 3
            specs = [("T2", "TdT", "Td"), ("T2T", "Td", "TdT"),
                     ("T4", "T2T", "T2"), ("T4T", "T2", "T2T"),
                     ("T8", "T4T", "T4"), ("T8T", "T4", "T4T"),
                     ("T16", "T8T", "T8")]

            def blkg(name):
                return pdc.tile([128, 2, 128], F16, tag=name, name=name,
                                bufs=3)

            fw1h_r = io["fw1h"][:].rearrange("(kt p) m -> p kt m", p=128)
            _fus_state = {"hst": None}

            def _emit_fusion(step):
                lw, mt = step // 8, step % 8
                if mt == 0:
                    hst = pd1.tile([128, 8, 512], F16, tag="hst", bufs=1,
                                   name="hst")
                    nc.sync.dma_start(out=hst,
                                      in_=hsT_r[:, :, lw*512:(lw+1)*512])
                    _fus_state["hst"] = hst
                hst = _fus_state["hst"]
                fwt = pd1.tile([128, 8, 128], F16, tag="fwt", bufs=3,
                               name=f"fwt{mt}")
                nc.sync.dma_start(out=fwt,
                                  in_=fw1h_r[:, :, mt*128:(mt+1)*128])
                ph = psD.tile([128, 512], F32, tag="pX5", bufs=2,
                              name="psh")
                for kt in range(8):
                    nc.tensor.matmul(ph, fwt[:, kt, :], hst[:, kt, :],
                                     start=(kt == 0), stop=(kt == 7))
                if mt % 2 == 0:
                    nc.scalar.copy(
                        out=hdn_pre[:, mt, lw*512:(lw+1)*512], in_=ph)
                else:
                    nc.vector.tensor_copy(
                        out=hdn_pre[:, mt, lw*512:(lw+1)*512], in_=ph)

            for g0 in range(0, NCH, GRP):
                cis = list(range(g0, min(g0 + GRP, NCH)))
                ls = {ci: slice(ci*128, (ci+1)*128) for ci in cis}
                t = {ci: {} for ci in cis}

                # step: G = K K^T, mask+beta -> Td, To
                for ci in cis:
                    pG = pd_("pG")
                    mm2(pG, lambda h, kt: kT[:, h, kt, ls[ci]],
                        lambda h, kt: kT[:, h, kt, ls[ci]], n_k=2)
                    t[ci]["Td"], t[ci]["To"] = blkg("Td"), blkg("To")
                    for h in range(2):
                        nc.vector.scalar_tensor_tensor(
                            out=t[ci]["Td"][:, h, :], in0=pG[:, h, :],
                            scalar=beta_lp[:, ci, h:h+1], in1=masks[:, 0, :],
                            op0=ALU.mult, op1=ALU.mult)
                        nc.vector.scalar_tensor_tensor(
                            out=t[ci]["To"][:, h, :], in0=pG[:, h, :],
                            scalar=beta_lp[:, ci, h:h+1], in1=masks[:, 1, :],
                            op0=ALU.mult, op1=ALU.mult)
                # step: TdT = transpose(Td)
                for ci in cis:
                    pT = psD.tile([128, 2, 128], F16, tag="pdT", bufs=2,
                                  name="pTdT")
                    for h in range(2):
                        nc.tensor.transpose(pT[:, h, :], t[ci]["Td"][:, h, :],
                                            ident16)
                    t[ci]["TdT"] = blkg("TdT")
                    nc.scalar.copy(out=t[ci]["TdT"], in_=pT)
                # steps: squaring chain
                for si_, (nm, ln, rn) in enumerate(specs):
                    for ci in cis:
                        pq2 = pd_("pq2")
                        mm2(pq2, lambda h, kt, a=t[ci][ln]: a[:, h, :],
                            lambda h, kt, b_=t[ci][rn]: b_[:, h, :])
                        t[ci][nm] = blkg(nm)
                        if (si_ + ci) % 2 == 0:
                            nc.scalar.copy(out=t[ci][nm], in_=pq2)
                        else:
                            nc.vector.tensor_copy(out=t[ci][nm], in_=pq2)
                # steps: MT product chain -> DT
                for ci in cis:
                    MT = pdc.tile([128, 2, 128], F16, tag="MT", name="MT",
                                  bufs=2 * GRP)
                    nc.vector.tensor_tensor(out=MT, in0=t[ci]["TdT"],
                                            in1=bc_mid(ident16, 2),
                                            op=ALU.add)
                    t[ci]["MT"] = MT
                for nm in ("T2", "T4", "T8", "T16"):
                    for ci in cis:
                        pm = pd_("pm")
                        for h in range(2):
                            nc.tensor.matmul(pm[:, h, :], t[ci][nm][:, h, :],
                                             t[ci]["MT"][:, h, :],
                                             start=True, stop=False)
                            nc.tensor.matmul(pm[:, h, :], ident16,
                                             t[ci]["MT"][:, h, :],
                                             start=False, stop=True)
                        MTn = pdc.tile([128, 2, 128], F16, tag="MT",
                                       name="MT", bufs=2 * GRP)
                        if ci % 2 == 0:
                            nc.scalar.copy(out=MTn, in_=pm)
                        else:
                            nc.vector.tensor_copy(out=MTn, in_=pm)
                        t[ci]["MT"] = MTn
                # steps: B, BT, B2T
                for ci in cis:
                    pB = pd_("pB")
                    mm2(pB, lambda h, kt: t[ci]["MT"][:, h, :],
                        lambda h, kt: t[ci]["To"][:, h, :])
                    t[ci]["Bm"] = blkg("Bm")
                    nc.scalar.copy(out=t[ci]["Bm"], in_=pB)
                for ci in cis:
                    pBT = pd_("pBT")
                    mm2(pBT, lambda h, kt: t[ci]["To"][:, h, :],
                        lambda h, kt: t[ci]["MT"][:, h, :])
                    t[ci]["BT"] = blkg("BT")
                    nc.vector.tensor_copy(out=t[ci]["BT"], in_=pBT)
                for ci in cis:
                    pB2 = pd_("pB2")
                    mm2(pB2, lambda h, kt: t[ci]["Bm"][:, h, :],
                        lambda h, kt: t[ci]["BT"][:, h, :])
                    t[ci]["B2T"] = blkg("B2T")
                    nc.scalar.copy(out=t[ci]["B2T"], in_=pB2)
                # step: aT
                for ci in cis:
                    pA4 = pd_("pA4")
                    mm2(pA4, lambda h, kt: kT[:, h, kt, ls[ci]],
                        lambda h, kt: qT[:, h, kt, ls[ci]], n_k=2)
                    nc.vector.tensor_tensor(out=aT_all[:, ci], in0=pA4,
                                            in1=bc_mid(masks[:, 3, :], 2),
                                            op=ALU.mult)
                # steps: X = [beta*v | beta*k], 3-stage apply
                for ci in cis:
                    X = pdw.tile([128, 2, 512], F16, tag="X", bufs=4,
                                 name="X")
                    for h in range(2):
                        nc.vector.tensor_scalar_mul(
                            X[:, h, 0:256].rearrange("p (a b) -> p a b", a=2),
                            vlc[:, h, :, ci, :], beta_lp[:, ci, h:h+1])
                        nc.vector.tensor_scalar_mul(
                            X[:, h, 256:512].rearrange(
                                "p (a b) -> p a b", a=2),
                            klc[:, h, :, ci, :], beta_lp[:, ci, h:h+1])
                    t[ci]["X"] = X
                for ci in cis:
                    x1t = pdw.tile([128, 2, 512], F16, tag="x1t", bufs=3,
                                   name="x1t")
                    for h in range(2):
                        px = psD.tile([128, 512], F32, tag="pX5", bufs=2,
                                      name="pX1")
                        nc.tensor.matmul(px, t[ci]["MT"][:, h, :],
                                         t[ci]["X"][:, h, :],
                                         start=True, stop=True)
                        if h == 0:
                            nc.scalar.copy(out=x1t[:, h, :], in_=px)
                        else:
                            nc.vector.tensor_copy(out=x1t[:, h, :], in_=px)
                    t[ci]["x1t"] = x1t
                for ci in cis:
                    y1t = pdw.tile([128, 2, 512], F16, tag="y1t", bufs=3,
                                   name="y1t")
                    for h in range(2):
                        px = psD.tile([128, 512], F32, tag="pX5", bufs=2,
                                      name="pX2")
                        nc.tensor.matmul(px, t[ci]["B2T"][:, h, :],
                                         t[ci]["x1t"][:, h, :],
                                         start=True, stop=False)
                        nc.tensor.matmul(px, ident16, t[ci]["x1t"][:, h, :],
                                         start=False, stop=True)
                        if h == 0:
                            nc.vector.tensor_copy(out=y1t[:, h, :], in_=px)
                        else:
                            nc.scalar.copy(out=y1t[:, h, :], in_=px)
                    t[ci]["y1t"] = y1t
                for ci in cis:
                    wtmp = pdw.tile([128, 2, 256], F16, tag="wtmp", bufs=3,
                                    name="wtmp")
                    for h in range(2):
                        px = psD.tile([128, 512], F32, tag="pX5", bufs=2,
                                      name="pX3")
                        nc.tensor.matmul(px, t[ci]["BT"][:, h, :],
                                         t[ci]["y1t"][:, h, :],
                                         start=True, stop=False)
                        nc.tensor.matmul(px, ident16, t[ci]["y1t"][:, h, :],
                                         start=False, stop=True)
                        nc.vector.tensor_copy(out=u_all[:, ci, h, :],
                                              in_=px[:, 0:256])
                        nc.scalar.copy(out=wtmp[:, h, :], in_=px[:, 256:512])
                    t[ci]["wtmp"] = wtmp
                for ci in cis:
                    for kt in range(2):
                        ptw = psD.tile([128, 2, 128], F16, tag="pdT", bufs=2,
                                       name="ptw")
                        for h in range(2):
                            nc.tensor.transpose(
                                ptw[:, h, :],
                                t[ci]["wtmp"][:, h, kt*128:(kt+1)*128],
                                ident16)
                        nc.scalar.mul(out=wT_all[:, ci, kt], in_=ptw,
                                      mul=-1.0)

                ngrp = (NCH + GRP - 1) // GRP
                gi = g0 // GRP
                for fstep in range(32 * gi // ngrp, 32 * (gi + 1) // ngrp):
                    _emit_fusion(fstep)

                # --- serial S part (per chunk) ---------------------------
                for ci in cis:
                    pup = psD.tile([128, 2, d], F32, tag="pS", bufs=1,
                                   name="pup")
                    for h in range(2):
                        for kt in range(2):
                            nc.tensor.matmul(pup[:, h, :],
                                             wT_all[:, ci, kt, h, :],
                                             S16[:, h, kt, :],
                                             start=(kt == 0), stop=False)
                        nc.tensor.matmul(pup[:, h, :], ident16,
                                         u_all[:, ci, h, :],
                                         start=False, stop=True)
                    # pup now holds u - w^T S = upr directly (wT is negated)
                    uprt = pdw.tile([128, 2, d], F16, tag="uprt", bufs=2,
                                    name="uprt")
                    nc.scalar.copy(out=uprt, in_=pup)

                    po = psD.tile([128, 2, d], F32, tag="pS", bufs=1,
                                  name="po")
                    for h in range(2):
                        for kt in range(2):
                            nc.tensor.matmul(po[:, h, :],
                                             qT[:, h, kt, ls[ci]],
                                             S16[:, h, kt, :],
                                             start=(kt == 0), stop=False)
                        nc.tensor.matmul(po[:, h, :], aT_all[:, ci, h, :],
                                         uprt[:, h, :],
                                         start=False, stop=True)
                    dsb = pdw.tile([128, 2, d], F16, tag="dsb", bufs=1,
                                   name="dsb")
                    nc.scalar.copy(out=dsb, in_=po)
                    nc.gpsimd.dma_start(
                        out=sc["dout_s"][ls[ci], :],
                        in_=dsb.rearrange("p h e -> p (h e)"))
                    nc.vector.tensor_reduce(
                        out=bn_sb[:, ci, 2:4], in_=dsb,
                        axis=mybir.AxisListType.X, op=ALU.add,
                        apply_absolute_value=True)

                    for h in range(2):
                        pdS = psD.tile([128, 2, d], F32, tag="pS", bufs=1,
                                       name=f"pdS{h}")
                        for kt in range(2):
                            nc.tensor.matmul(pdS[:, kt, :],
                                             klc[:, h, kt, ci, :],
                                             uprt[:, h, :],
                                             start=True, stop=True)
                        nc.vector.scalar_tensor_tensor(
                            out=S16[:, h], in0=pdS, scalar=1.0,
                            in1=S16[:, h], op0=ALU.mult, op1=ALU.add)

            # ---- bn features finalize + AllGather ------------------------
            ctx_bn = tc.tile_wait_until(0.40)
            ctx_bn.__enter__()
            for lt in range(NLT):
                cmt_b = pd1.tile([128, C], F16, tag="cmt_b", bufs=3,
                                 name="cmt_b")
                nc.gpsimd.dma_start(
                    out=cmt_b, in_=sc["cm_out"][lt*128:(lt+1)*128, :])
                nc.vector.tensor_reduce(
                    out=bn_sb[:, lt, 0:2],
                    in_=cmt_b.rearrange("p (h e) -> p h e", e=d),
                    axis=mybir.AxisListType.X, op=ALU.add,
                    apply_absolute_value=True)
            nc.vector.tensor_reduce(
                out=bn_sb[:, :, 4:6],
                in_=bnv4[:].rearrange("p lt (h t) -> p lt h t", t=2),
                axis=mybir.AxisListType.X, op=ALU.add)
            nc.gpsimd.dma_start(
                out=sc["bn_in"][:].rearrange("(lt p) c -> p lt c", p=128),
                in_=bn_sb)
            nc.gpsimd.collective_compute(
                "AllGather", ALU.bypass, replica_groups=RG,
                ins=[sc["bn_in"][:]], outs=[sc["bn_out"][:]])
            ctx_bn.__exit__(None, None, None)

        # =================== PHASE C ======================================
        with tc.tile_pool(name="pc1", bufs=1) as pc1, \
             tc.tile_pool(name="pc2", bufs=2) as pc2, \
             tc.tile_pool(name="pcs", bufs=4) as pcs, \
             tc.tile_pool(name="psC", bufs=1, space="PSUM") as psC:

            def psc(name, tag="pg"):
                return psC.tile([128, 512], F32, tag=tag, bufs=2, name=name)

            bnT = [pc1.tile([8, L], F16, name=f"bnT{m}") for m in range(2)]
            for m in range(2):
                bng = pc2.tile([128, NLT, 8], F32, tag="bng", name=f"bng{m}")
                nc.sync.dma_start(
                    out=bng,
                    in_=sc["bn_out"][m].rearrange("(lt p) c -> p lt c",
                                                  p=128))
                for lt in range(NLT):
                    ptb = psc("ptb")
                    ptbv = ptb[0:8, 0:128]
                    nc.tensor.transpose(ptbv, bng[:, lt, :], ident)
                    nc.scalar.mul(out=bnT[m][:, lt*128:(lt+1)*128],
                                  in_=ptbv, mul=1.0/d)

            # ---- fusion tail: bn part + gelu -----------------------------
            fb1_sb = pc1.tile([128, 8], F32)
            nc.sync.dma_start(out=fb1_sb,
                              in_=io["fb1"][:].rearrange("(m p) -> p m",
                                                         p=128))
            fw1b_sb = pc1.tile([8, 2, 1024], F16)
            nc.sync.dma_start(
                out=fw1b_sb,
                in_=io["fw1b"][:].rearrange("(m p) c -> p m c", p=8))
            for lw in range(NLW):
                for mt in range(8):
                    ph2 = psC.tile([128, 512], F32, tag="pf2", bufs=2, name="psh2")
                    for m in range(2):
                        nc.tensor.matmul(ph2,
                                         fw1b_sb[:, m, mt*128:(mt+1)*128],
                                         bnT[m][:, lw*512:(lw+1)*512],
                                         start=(m == 0), stop=(m == 1))
                    hdf = pcs.tile([128, 512], F16, tag="hdf", bufs=3,
                                   name="hdf")
                    nc.vector.scalar_tensor_tensor(
                        out=hdf, in0=ph2, scalar=1.0,
                        in1=hdn_pre[:, mt, lw*512:(lw+1)*512],
                        op0=ALU.mult, op1=ALU.add)
                    nc.scalar.activation(
                        out=hdn_pre[:, mt, lw*512:(lw+1)*512], in_=hdf,
                        func=AF.Gelu, bias=fb1_sb[:, mt:mt+1])

            fw2_sb = pc1.tile([128, 8, 12], F16)
            nc.sync.dma_start(
                out=fw2_sb,
                in_=io["fw2"][:].rearrange("(kt p) c -> p kt c", p=128))
            lg_sb = pc1.tile([128, NLT, 12], F32)
            for lt in range(NLT):
                pl = psc("psl")
                plv = pl[:, 0:12]
                for kt in range(8):
                    nc.tensor.matmul(plv, hdn_pre[:, kt, lt*128:(lt+1)*128],
                                     fw2_sb[:, kt, :],
                                     start=(kt == 0), stop=(kt == 7))
                nc.scalar.copy(out=lg_sb[:, lt, :], in_=plv)
            for m in range(2):
                nc.gpsimd.dma_start(
                    out=sc["lg_in"][m].rearrange("(lt p) c -> p lt c", p=128),
                    in_=lg_sb[:, :, m*6:(m+1)*6])
            nc.gpsimd.collective_compute(
                "ReduceScatter", ALU.add, replica_groups=RG,
                ins=[sc["lg_in"][:]], outs=[sc["lg_out"][:]])

            # ---- softmax gates -------------------------------------------
            b2_sb = pc1.tile([128, 6], F32)
            nc.sync.dma_start(out=b2_sb, in_=io["b2o"][:])
            lgo = pc1.tile([128, NLT, 2, 3], F32)
            nc.sync.dma_start(
                out=lgo,
                in_=sc["lg_out"][:].rearrange("(lt p) (h e) -> p lt h e",
                                              p=128, e=3))
            nc.vector.tensor_tensor(
                out=lgo, in0=lgo,
                in1=bass.AP(tensor=b2_sb.tensor, offset=b2_sb.offset,
                            ap=[b2_sb.ap[0], [0, NLT], [3, 2], [1, 3]]),
                op=ALU.add)
            rmax = pc1.tile([128, NLT, 2], F32)
            nc.vector.tensor_reduce(out=rmax, in_=lgo,
                                    axis=mybir.AxisListType.X, op=ALU.max)
            nc.vector.tensor_tensor(
                out=lgo, in0=lgo,
                in1=rmax[:, :, :, None].to_broadcast([128, NLT, 2, 3]),
                op=ALU.subtract)
            nc.scalar.activation(out=lgo, in_=lgo, func=AF.Exp)
            rsum = pc1.tile([128, NLT, 2], F32)
            nc.vector.tensor_reduce(out=rsum, in_=lgo,
                                    axis=mybir.AxisListType.X, op=ALU.add)
            nc.vector.reciprocal(out=rsum, in_=rsum)
            nc.vector.tensor_tensor(
                out=lgo, in0=lgo,
                in1=rsum[:, :, :, None].to_broadcast([128, NLT, 2, 3]),
                op=ALU.mult)

            # ---- gate mix + RMSNorm + Wo ---------------------------------
            wo_sb = pc1.tile([128, 4, D], F16)
            nc.sync.dma_start(
                out=wo_sb,
                in_=io["wo"][:].rearrange("(kt p) n -> p kt n", p=128))
            dout_sb = arena[:, 0:8192].rearrange("p (lt h e) -> p lt h e",
                                                 h=2, e=d)
            cm_sb = arena[:, 8192:16384].rearrange("p (lt c) -> p lt c",
                                                   c=C)
            nc.gpsimd.dma_start(
                out=cm_sb,
                in_=sc["cm_out"][:].rearrange("(lt p) c -> p lt c", p=128))
            nc.gpsimd.dma_start(
                out=dout_sb,
                in_=sc["dout_s"][:].rearrange("(lt p) (h e) -> p lt h e",
                                              p=128, e=d))
            for lt in range(NLT):
                dov = dout_sb[:, lt]
                o_t = pcs.tile([128, 2, d], F16, tag="o_t", name="o_t")
                ssq = pcs.tile([128, 2], F32, tag="ssq", name="ssq")
                scr = pcs.tile([128, d], F32, tag="scr", name="scr")
                for h in range(2):
                    nc.vector.tensor_scalar_mul(
                        o_t[:, h, :],
                        cm_sb[:, lt, h*256:(h+1)*256], lgo[:, lt, h, 0:1])
                    nc.vector.scalar_tensor_tensor(
                        out=o_t[:, h, :], in0=dov[:, h, :],
                        scalar=lgo[:, lt, h, 1:2], in1=o_t[:, h, :],
                        op0=ALU.mult, op1=ALU.add)
                    nc.vector.scalar_tensor_tensor(
                        out=o_t[:, h, :].rearrange("p (a b) -> p a b", a=2),
                        in0=vlc[:, h, :, lt, :],
                        scalar=lgo[:, lt, h, 2:3],
                        in1=o_t[:, h, :].rearrange("p (a b) -> p a b", a=2),
                        op0=ALU.mult, op1=ALU.add)
                    nc.scalar.activation(out=scr, in_=o_t[:, h, :],
                                         func=AF.Square,
                                         accum_out=ssq[:, h:h+1])
                nc.scalar.activation(out=ssq, in_=ssq, func=AF.Sqrt,
                                     scale=1.0/d, bias=eps5)
                nc.vector.reciprocal(out=ssq, in_=ssq)
                for h in range(2):
                    nc.vector.tensor_scalar_mul(o_t[:, h, :], o_t[:, h, :],
                                                ssq[:, h:h+1])
                pto = psC.tile([128, 4, 128], F16, tag="pto", bufs=2,
                               name="psto")
                for ct in range(4):
                    h, dt = ct // 2, ct % 2
                    nc.tensor.transpose(pto[:, ct, :],
                                        o_t[:, h, dt*128:(dt+1)*128],
                                        ident16)
                oT = pcs.tile([128, 4, 128], F16, tag="oT", name="oT")
                nc.vector.tensor_copy(out=oT, in_=pto)
                orow = pcs.tile([128, D], F32, tag="orow", name="orow")
                for nh in range(2):
                    pw = psC.tile([128, 512], F32, tag="pw", bufs=2, name="psw")
                    for ct in range(4):
                        nc.tensor.matmul(pw, oT[:, ct, :],
                                         wo_sb[:, ct, nh*512:(nh+1)*512],
                                         start=(ct == 0), stop=(ct == 3))
                    nc.scalar.copy(out=orow[:, nh*512:(nh+1)*512], in_=pw)
                nc.sync.dma_start(out=io["out_part"][lt*128:(lt+1)*128, :],
                                  in_=orow)


# ======================= host side =======================================

def _diag_tiles(w_own, taps, out_dtype):
    """w_own: (C, k) conv weights for this core's channels.
    Returns (4, k, 128, 128) diag tiles; tap dd uses column k-1-dd."""
    k = w_own.shape[1]
    out = np.zeros((4, k, 128, 128), dtype=out_dtype)
    for ct in range(4):
        for dd in range(k):
            np.fill_diagonal(out[ct, dd], w_own[ct*128:(ct+1)*128, k-1-dd])
    return out


def _host_inputs(inputs):
    hs = np.asarray(inputs["hidden_states"], np.float32)
    Wq = np.asarray(inputs["Wq"], np.float32)
    Wk = np.asarray(inputs["Wk"], np.float32)
    Wv = np.asarray(inputs["Wv"], np.float32)
    Wb = np.asarray(inputs["Wb"], np.float32)
    cq = np.asarray(inputs["conv_q_w"], np.float32)
    ck = np.asarray(inputs["conv_k_w"], np.float32)
    cv = np.asarray(inputs["conv_v_w"], np.float32)
    w3 = np.asarray(inputs["ms_w3"], np.float32)
    w15 = np.asarray(inputs["ms_w15"], np.float32)
    w31 = np.asarray(inputs["ms_w31"], np.float32)
    kmix = np.asarray(inputs["kernel_mix_w"], np.float32)
    cmix = np.asarray(inputs["channel_mixer_w"], np.float32)
    fw1 = np.asarray(inputs["fusion_w1"], np.float32)
    fb1 = np.asarray(inputs["fusion_b1"], np.float32)
    fw2 = np.asarray(inputs["fusion_w2"], np.float32)
    fb2 = np.asarray(inputs["fusion_b2"], np.float32)
    onw = np.asarray(inputs["o_norm_w"], np.float32)
    Wo = np.asarray(inputs["Wo"], np.float32)

    # combined kernel_mix -> channel_mixer matrix Q: (3D, D)
    Q = np.zeros((3 * D, D), np.float32)
    for h in range(H):
        Q[h*3*d:(h+1)*3*d] = kmix @ cmix[h*d:(h+1)*d]

    masks = np.zeros((5, 128, 128), np.float32)
    i_, j_ = np.mgrid[0:128, 0:128]
    blk = (i_ // 32) == (j_ // 32)
    masks[0] = -((i_ > j_) & blk).astype(np.float32)
    masks[1] = -((i_ > j_) & ~blk).astype(np.float32)
    masks[2] = -((j_ > i_) & blk).astype(np.float32)
    masks[3] = (j_ >= i_).astype(np.float32)
    masks[4] = np.eye(128, dtype=np.float32)

    Wo_s = Wo * np.tile(onw, H)[:, None]

    in_maps = []
    for c in range(8):
        b, r = divmod(c, 2)
        cs = slice(C*r, C*(r+1))
        qmix = np.concatenate(
            [Q[1024*s + C*r: 1024*s + C*r + C] for s in range(3)], 0)
        f8 = ml_dtypes.float8_e4m3
        mspair = np.zeros((4, 23, 2, 128, 128), np.float32)
        mssing = np.zeros((4, 3, 128, 128), np.float32)
        for ct in range(4):
            pi = 0
            for si, (w, ks) in enumerate(((w3, 3), (w15, 15), (w31, 31))):
                dg = _diag_tiles(w[cs], ks, np.float32)[ct] * 64.0
                for p_ in range((ks - 1) // 2):
                    mspair[ct, pi, 0] = dg[2*p_ + 1]
                    mspair[ct, pi, 1] = dg[2*p_]
                    pi += 1
                mssing[ct, si] = dg[ks - 1]
        cdiag = np.stack([_diag_tiles(w[cs], KQKV, np.float16)
                          for w in (cq, ck, cv)], 0)
        fw1b = np.zeros((16, 1024), np.float32)
        for m in range(2):
            for src in range(3):
                for h_ in range(2):
                    fw1b[m*8 + src*2 + h_] = \
                        fw1[D + src*4 + 2*m + h_, 1024*r:1024*(r+1)]
        fw2p = np.zeros((1024, 12), np.float32)
        b2o = np.zeros((6,), np.float32)
        for jm in range(2):
            for h_ in range(2):
                for br in range(3):
                    gcol = (2*jm + h_)*3 + br
                    fw2p[:, jm*6 + h_*3 + br] = fw2[1024*r:1024*(r+1), gcol]
        for h_ in range(2):
            for br in range(3):
                b2o[h_*3 + br] = fb2[(2*r + h_)*3 + br]
        m = {
            "hsT": np.ascontiguousarray(hs[b].T).astype(np.float16),
            "wq": np.ascontiguousarray(Wq[:, cs]).astype(np.float16),
            "wk": np.ascontiguousarray(Wk[:, cs]).astype(np.float16),
            "wv": np.ascontiguousarray(Wv[:, cs]).astype(np.float16),
            "wb": np.ascontiguousarray(Wb[:, 2*r:2*r+2]).astype(np.float16),
            "cdiag": cdiag,
            "mspair": mspair.astype(f8),
            "mssing": mssing.astype(f8),
            "qmix8": (qmix * 64.0).astype(f8),
            "fw1h": np.ascontiguousarray(
                fw1[:D, 1024*r:1024*(r+1)]).astype(np.float16),
            "fw1b": fw1b.astype(np.float16),
            "fb1": np.ascontiguousarray(fb1[1024*r:1024*(r+1)]),
            "fw2": fw2p.astype(np.float16),
            "b2o": np.tile(b2o, (128, 1)),
            "wo": np.ascontiguousarray(Wo_s[cs, :]).astype(np.float16),
            "masks": masks,
            "onesrow": np.ones((1, 128), np.float32),
            "onescol": np.ones((128, 1), np.float32),
            "ident16": np.eye(128, dtype=np.float16),
        }
        in_maps.append(m)
    return in_maps


_PROG = {}


def _get_program(debug=False):
    key = bool(debug)
    if key not in _PROG:
        _PROG[key] = build_program(debug=debug)
    return _PROG[key]


def run(inputs, debug=False, **kw):
    nc = _get_program(debug=debug)
    in_maps = _host_inputs(inputs)
    res = run_bass_kernel_spmd(nc, in_maps, list(range(8)), **kw)
    return res


def kernel(**inputs):
    res = run(inputs)
    out = np.zeros((B, L, D), np.float32)
    for b in range(B):
        out[b] = res.results[2*b]["out_part"] + res.results[2*b+1]["out_part"]
    return out


if __name__ == "__main__":
    nc = build_program()
    print("program built ok")


# revision 72
# speedup vs baseline: 1.0263x; 1.0263x over previous
"""DeltaNet-style block (nn_DeltaNet_31877247271438) on 8 trn2 NeuronCores.

Sharding: core c -> (batch b = c//2, pair-rank r = c%2).  Within a batch pair:
  - head-parallel: rank r owns heads {2r, 2r+1} (feature cols [512r, 512r+512))
  - cross-head mixes are K-split with pairwise collectives:
      * channel_mixer (folded with kernel_mix into one matrix Q): partial sums
        ReduceScatter'ed (fp16) so each core receives its own heads' ms_out
      * fusion-MLP hidden is column-split; logits partials ReduceScatter'ed
      * bn features AllGather'ed (tiny)
  - the final Wo matmul partials are summed on the host.

v2: inter-phase tensors stay in SBUF (no DRAM staging round-trips); l-major
forms of k/v come from DMA-xbar transposes; the delta rule's 16 chunk
inversions are batched (only the S recurrence is serial); beta is folded
into the mask multiply so the kb row form is never materialized; a manually
aliased SBUF arena lets phase-scoped tensors (hsT/xc, y, u/wT/aT, hdn_pre)
share the same bytes.
"""
import sys
sys.path.insert(0, '/opt/trn_rl_repo')

import numpy as np
import ml_dtypes

import concourse.bass as bass
import concourse.tile as tile
from concourse import bacc, mybir
from concourse.bass_utils import run_bass_kernel_spmd

F32 = mybir.dt.float32
F16 = mybir.dt.float16
F8 = mybir.dt.float8e4
DR = mybir.MatmulPerfMode.DoubleRow
FP8S = 64.0
AF = mybir.ActivationFunctionType
ALU = mybir.AluOpType

B, L, D, H = 4, 2048, 1024, 4
d = 256          # per-head dim
C = 512          # channels owned per core (2 heads)
NLT = 16         # l-tiles of 128
NLW = 4          # l-windows of 512
NCH = 16         # delta chunks of 128
KQKV = 4         # qkv conv taps
MSK = (3, 15, 31)
NTAPS = sum(MSK)  # 49
PADV = 32
RG = [[0, 1], [2, 3], [4, 5], [6, 7]]

ARENA = 36864    # f16 elems per partition in the aliased arena


def bc_mid(ap2, n):
    """[P, F] AP -> [P, n, F] with a 0-stride middle dim (free-dim bcast)."""
    assert len(ap2.ap) == 2
    return bass.AP(tensor=ap2.tensor, offset=ap2.offset,
                   ap=[ap2.ap[0], [0, n], ap2.ap[1]])


def build_program(debug=False):
    nc = bacc.Bacc("TRN2", target_bir_lowering=False, debug=False,
                   num_devices=8)

    io = {}
    io["hsT"] = nc.declare_dram_parameter("hsT", [D, L], F16, False)
    io["wq"] = nc.declare_dram_parameter("wq", [D, C], F16, False)
    io["wk"] = nc.declare_dram_parameter("wk", [D, C], F16, False)
    io["wv"] = nc.declare_dram_parameter("wv", [D, C], F16, False)
    io["wb"] = nc.declare_dram_parameter("wb", [D, 2], F16, False)
    io["cdiag"] = nc.declare_dram_parameter("cdiag", [3, 4, KQKV, 128, 128],
                                            F16, False)
    io["mspair"] = nc.declare_dram_parameter("mspair", [4, 23, 2, 128, 128],
                                             F8, False)
    io["mssing"] = nc.declare_dram_parameter("mssing", [4, 3, 128, 128],
                                             F8, False)
    io["qmix8"] = nc.declare_dram_parameter("qmix8", [12 * 128, D], F8, False)
    io["fw1h"] = nc.declare_dram_parameter("fw1h", [D, 1024], F16, False)
    io["fw1b"] = nc.declare_dram_parameter("fw1b", [16, 1024], F16, False)
    io["fb1"] = nc.declare_dram_parameter("fb1", [1024], F32, False)
    io["fw2"] = nc.declare_dram_parameter("fw2", [1024, 12], F16, False)
    io["b2o"] = nc.declare_dram_parameter("b2o", [128, 6], F32, False)
    io["wo"] = nc.declare_dram_parameter("wo", [C, D], F16, False)
    io["masks"] = nc.declare_dram_parameter("masks", [5, 128, 128], F32, False)
    io["onesrow"] = nc.declare_dram_parameter("onesrow", [1, 128], F32, False)
    io["onescol"] = nc.declare_dram_parameter("onescol", [128, 1], F32, False)
    io["ident16"] = nc.declare_dram_parameter("ident16", [128, 128], F16, False)
    io["out_part"] = nc.declare_dram_parameter("out_part", [L, D], F32, True)

    sc = {}
    sc["dout_s"] = nc.dram_tensor("dout_s", [L, C], F16)
    sc["cm_in"] = nc.dram_tensor("cm_in", [2, L, C], F16)
    sc["cm_out"] = nc.dram_tensor("cm_out", [L, C], F16)
    sc["bn_in_a"] = nc.dram_tensor("bn_in_a", [1024, 8], F32)
    sc["bn_out_a"] = nc.dram_tensor("bn_out_a", [2, 1024, 8], F32)
    sc["bn_in_b"] = nc.dram_tensor("bn_in_b", [1024, 8], F32)
    sc["bn_out_b"] = nc.dram_tensor("bn_out_b", [2, 1024, 8], F32)
    sc["lg_in"] = nc.dram_tensor("lg_in", [2, L, 6], F32)
    sc["lg_out"] = nc.dram_tensor("lg_out", [L, 6], F32)

    with tile.TileContext(nc) as tc:
        _body(nc, tc, io, sc)
    nc.compile()
    return nc


def _body(nc, tc, io, sc):
    from contextlib import ExitStack
    ctx = ExitStack()
    with ctx:
        consts = ctx.enter_context(tc.tile_pool(name="consts", bufs=1))
        outer = ctx.enter_context(tc.tile_pool(name="outer", bufs=1))

        masks = consts.tile([128, 5, 128], F32)
        nc.sync.dma_start(out=masks,
                          in_=io["masks"][:].rearrange("m p f -> p m f"))
        ident = masks[:, 4, :]
        ident16 = consts.tile([128, 128], F16)
        nc.sync.dma_start(out=ident16, in_=io["ident16"][:])
        onescol16 = consts.tile([128, 1], F16)
        nc.vector.memset(onescol16, 1.0)
        onesrow16 = consts.tile([1, 128], F16)
        nc.vector.memset(onesrow16, 1.0)
        eps6 = consts.tile([128, 1], F32)
        nc.vector.memset(eps6, 1e-6)
        eps5 = consts.tile([128, 1], F32)
        nc.vector.memset(eps5, 1e-5)

        beta_lp = outer.tile([128, NLT, 2], F32)
        S16 = outer.tile([128, 2, 2, d], F16)
        nc.vector.memset(S16, 0.0)
        qT = outer.tile([128, 2, 2, L], F16)       # (h, dtile, l) row form
        kT = outer.tile([128, 2, 2, L], F16)
        klc = outer.tile([128, 2, 2, NLT, 128], F16)  # (h, dtile, lt, dk)
        vlc = outer.tile([128, 2, 2, NLT, 128], F16)
        bn_sb = outer.tile([128, NLT, 8], F32)
        bnv4 = outer.tile([128, NLT, 4], F32)
        arena = outer.tile([128, ARENA], F16)

        # arena regions (manually aliased, lifetimes disjoint)
        hsT = arena[:, 0:16384].rearrange("p (kt l) -> p kt l", l=L)
        xc = arena[:, 16384:24592].rearrange("p (ct l) -> p ct l", l=L + 4)
        vt_bf = arena[:, 24592:32912].rearrange("p (ct l) -> p ct l",
                                                l=PADV + L)
        y_bf = arena[:, 0:24576].rearrange("p (j l) -> p j l", l=L)
        u_all = arena[:, 0:8192].rearrange("p (n h e) -> p n h e", h=2, e=d)
        wT_all = arena[:, 8192:16384].rearrange("p (n kt h e) -> p n kt h e",
                                                kt=2, h=2, e=128)
        aT_all = arena[:, 16384:20480].rearrange("p (n h e) -> p n h e",
                                                 h=2, e=128)
        hdn_pre = arena[:, 20480:36864].rearrange("p (mt l) -> p mt l", l=L)

        hsT_r = io["hsT"][:].rearrange("(kt p) l -> p kt l", p=128)

        # =================== PHASE A ======================================
        with tc.tile_pool(name="pa1", bufs=1) as pa1, \
             tc.tile_pool(name="pa2", bufs=2) as pa2, \
             tc.tile_pool(name="pas", bufs=3) as pas, \
             tc.tile_pool(name="psA", bufs=1, space="PSUM") as psA:
            nc.sync.dma_start(out=hsT, in_=hsT_r)

            # ---- beta (l-partition form) ---------------------------------
            wb_sb = pa1.tile([128, 8, 2], F16)
            nc.sync.dma_start(
                out=wb_sb, in_=io["wb"][:].rearrange("(kt p) c -> p kt c",
                                                     p=128))
            for lt in range(NLT):
                pb = psA.tile([128, 2], F32, tag="pb", bufs=1, name="psb")
                for kt in range(8):
                    nc.tensor.matmul(pb, hsT[:, kt, lt*128:(lt+1)*128],
                                     wb_sb[:, kt, :],
                                     start=(kt == 0), stop=(kt == 7))
                nc.scalar.activation(out=beta_lp[:, lt, :], in_=pb,
                                     func=AF.Sigmoid)

            # ---- q, k, v: proj -> conv -> silu -> (norm) -----------------
            for tnm, ti in (("v", 2), ("k", 1), ("q", 0)):
                PAD = 4
                w_sb = pa2.tile([128, 8, C], F16, tag="w_sb", bufs=2,
                                name=f"w_{tnm}")
                nc.sync.dma_start(
                    out=w_sb,
                    in_=io["w" + tnm][:].rearrange("(kt p) c -> p kt c",
                                                   p=128))
                cdg = pa2.tile([128, 4, KQKV, 128], F16, tag="cdg", bufs=2,
                               name=f"cdg_{tnm}")
                nc.sync.dma_start(
                    out=cdg,
                    in_=io["cdiag"][ti].rearrange("ct tap p f -> p ct tap f"))
                if tnm != "v":
                    nc.vector.memset(xc[:, :, 0:4], 0.0)
                else:
                    nc.vector.memset(vt_bf[:, :, 0:PADV], 0.0)

                for ct in range(4):
                    xp = pa2.tile([128, 4 + L], F16, tag="xp", bufs=2,
                                  name=f"xp_{tnm}{ct}")
                    nc.vector.memset(xp[:, 0:4], 0.0)
                    for lw in range(NLW):
                        pp = psA.tile([128, 512], F32, tag="pp", bufs=2,
                                      name="psp")
                        for kt in range(8):
                            nc.tensor.matmul(
                                pp, w_sb[:, kt, ct*128:(ct+1)*128],
                                hsT[:, kt, lw*512:(lw+1)*512],
                                start=(kt == 0), stop=(kt == 7))
                        nc.scalar.copy(out=xp[:, 4+lw*512:4+(lw+1)*512],
                                       in_=pp)
                    for lw in range(NLW):
                        pc = psA.tile([128, 512], F32, tag="pc", bufs=2,
                                      name="psc")
                        for dd in range(KQKV):
                            off = 4 + lw*512 - dd
                            nc.tensor.matmul(
                                pc, cdg[:, ct, dd, :], xp[:, off:off+512],
                                start=(dd == 0), stop=(dd == KQKV-1))
                        if tnm == "v":
                            nc.scalar.activation(
                                out=vt_bf[:, ct, PADV+lw*512:PADV+(lw+1)*512],
                                in_=pc, func=AF.Silu)
                        else:
                            nc.scalar.activation(
                                out=xc[:, ct, PAD+lw*512:PAD+(lw+1)*512],
                                in_=pc, func=AF.Silu)
                if tnm == "v":
                    for ct in range(4):
                        nc.sync.dma_start_transpose(
                            out=vlc[:, ct // 2, ct % 2],
                            in_=vt_bf[:, ct, PADV:PADV+L])
                    continue

                # ---- l2norm -> qT / kT row forms -------------------------
                dst = qT if tnm == "q" else kT
                for h in range(2):
                    for lw in range(NLW):
                        lsl = slice(PAD+lw*512, PAD+(lw+1)*512)
                        osl = slice(lw*512, (lw+1)*512)
                        sqs = pas.tile([128, 2, 512], F16, tag="sq", bufs=3,
                                       name="sq")
                        for i, ct in enumerate((2*h, 2*h+1)):
                            nc.vector.tensor_tensor(
                                out=sqs[:, i, :], in0=xc[:, ct, lsl],
                                in1=xc[:, ct, lsl], op=ALU.mult)
                        ssr = pas.tile([1, 2, 512], F32, tag="ssr", bufs=2,
                                       name="ssr")
                        nc.gpsimd.tensor_reduce(
                            out=ssr, in_=sqs, axis=mybir.AxisListType.C,
                            op=ALU.add)
                        sr = pas.tile([1, 512], F32, tag="sr", bufs=2,
                                      name="sr")
                        nc.vector.tensor_tensor(out=sr, in0=ssr[:, 0, :],
                                                in1=ssr[:, 1, :], op=ALU.add)
                        nc.scalar.activation(out=sr, in_=sr, func=AF.Sqrt,
                                             bias=eps6[0:1, :])
                        srt = pas.tile([1, 512], F16, tag="srt", bufs=2,
                                       name="srt")
                        with nc.allow_low_precision("l2norm scale fp16"):
                            nc.vector.reciprocal(out=srt, in_=sr)
                        pbc = psA.tile([128, 512], F32, tag="pn2", bufs=2,
                                       name="psbc")
                        nc.tensor.matmul(pbc, onesrow16, srt,
                                         start=True, stop=True)
                        for dt in range(2):
                            ct = 2*h + dt
                            nc.vector.tensor_tensor(
                                out=dst[:, h, dt, osl], in0=xc[:, ct, lsl],
                                in1=pbc, op=ALU.mult)
                if tnm == "k":
                    for h in range(2):
                        for dt in range(2):
                            nc.sync.dma_start_transpose(
                                out=klc[:, h, dt], in_=kT[:, h, dt, :])

            # bn(v): |v| partial sums (finalized later)
            for lt in range(NLT):
                nc.vector.tensor_reduce(
                    out=bnv4[:, lt, :], in_=vlc[:, :, :, lt, :],
                    axis=mybir.AxisListType.X, op=ALU.add,
                    apply_absolute_value=True)

        # =================== PHASE B: multiscale conv + qmix ==============
        with tc.tile_pool(name="pb1", bufs=1) as pb1, \
             tc.tile_pool(name="pbm", bufs=2) as pbm, \
             tc.tile_pool(name="pbs", bufs=3) as pbs, \
             tc.tile_pool(name="psB", bufs=1, space="PSUM") as psB:
            y8 = arena[:, 0:12288].bitcast(F8).rearrange(
                "p (j l) -> p j l", l=L)
            vt8 = arena[:, 12288:16448].bitcast(F8).rearrange(
                "p (ct l) -> p ct l", l=PADV + L)
            with nc.allow_low_precision("fp8 conv input"):
                for ct in range(4):
                    nc.vector.tensor_copy(out=vt8[:, ct, :],
                                          in_=vt_bf[:, ct, :])

            def pair_ap(off, n, ct):
                a = vt8[:, ct, off:off+n]
                return bass.AP(tensor=a.tensor, offset=a.offset,
                               ap=[a.ap[0], [1, 2], a.ap[1]])

            for ct in range(4):
                msd8 = pbm.tile([128, 23, 2, 128], F8, tag=f"msd{ct % 2}",
                                bufs=1, name=f"msd8_{ct}")
                nc.gpsimd.dma_start(
                    out=msd8,
                    in_=io["mspair"][ct].rearrange("j t p f -> p j t f"))
                mss8 = pbm.tile([128, 3, 128], F8, tag=f"mss{ct % 2}",
                                bufs=1, name=f"mss8_{ct}")
                nc.gpsimd.dma_start(
                    out=mss8,
                    in_=io["mssing"][ct].rearrange("j p f -> p j f"))
                for lw in range(NLW):
                    base_pi = 0
                    for si, ks in enumerate(MSK):
                        npair = (ks - 1) // 2
                        py = psB.tile([128, 512], F32, tag="py", bufs=4,
                                      name="psy")
                        for p_ in range(npair):
                            off = PADV + lw*512 - (2*p_ + 1)
                            nc.tensor.matmul(
                                py, msd8[:, base_pi + p_, :, :],
                                pair_ap(off, 512, ct),
                                start=(p_ == 0), stop=False, perf_mode=DR)
                        off = PADV + lw*512 - (ks - 1)
                        nc.tensor.matmul(py, mss8[:, si, :],
                                         vt8[:, ct, off:off+512],
                                         start=False, stop=True)
                        nc.scalar.mul(
                            out=y8[:, si*4+ct, lw*512:(lw+1)*512],
                            in_=py, mul=1.0/FP8S)
                        base_pi += npair

            qmix8_sb = pb1.tile([128, 6, 2, D], F8)
            nc.gpsimd.dma_start(
                out=qmix8_sb,
                in_=io["qmix8"][:].rearrange("(pp j p) o -> p pp j o",
                                             j=2, p=128))
            cms_v = [arena[:, 33040+i*1024:33040+(i+1)*1024].rearrange(
                "p (o c) -> p o c", o=2) for i in range(3)]
            for lt in range(NLT):
                cms = cms_v[lt % 3]
                for oh in range(2):
                    pq = psB.tile([128, 512], F32, tag="pq", bufs=4,
                                  name="psq")
                    for p_ in range(6):
                        nc.tensor.matmul(
                            pq, y8[:, 2*p_:2*p_+2, lt*128:(lt+1)*128],
                            qmix8_sb[:, p_, :, oh*512:(oh+1)*512],
                            start=(p_ == 0), stop=(p_ == 5), perf_mode=DR)
                    if oh == 0:
                        nc.vector.tensor_scalar_mul(cms[:, oh, :], pq,
                                                    1.0/FP8S)
                    else:
                        nc.scalar.mul(out=cms[:, oh, :], in_=pq,
                                      mul=1.0/FP8S)
                nc.gpsimd.dma_start(
                    out=sc["cm_in"][:, lt*128:(lt+1)*128, :].rearrange(
                        "o l c -> l o c"),
                    in_=cms)
            nc.gpsimd.collective_compute(
                "ReduceScatter", ALU.add, replica_groups=RG,
                ins=[sc["cm_in"][:]], outs=[sc["cm_out"][:]])

        # =================== PHASE B3: delta rule =========================
        with tc.tile_pool(name="pd1", bufs=1) as pd1, \
             tc.tile_pool(name="pdc", bufs=1) as pdc, \
             tc.tile_pool(name="pdw", bufs=1) as pdw, \
             tc.tile_pool(name="psD", bufs=1, space="PSUM") as psD:

            def blk(name, bufs=2):
                return pdc.tile([128, 2, 128], F16, tag=name, name=name,
                                bufs=bufs)

            def pd_(name):
                return psD.tile([128, 2, 128], F32, tag="pd", bufs=3,
                                name=name)

            def mm2(pt, lhs_fn, rhs_fn, n_k=1):
                for h in range(2):
                    for kt in range(n_k):
                        nc.tensor.matmul(pt[:, h, :], lhs_fn(h, kt),
                                         rhs_fn(h, kt),
                                         start=(kt == 0), stop=(kt == n_k-1))

            GRP = 3
            specs = [("T2", "TdT", "Td"), ("T2T", "Td", "TdT"),
                     ("T4", "T2T", "T2"), ("T4T", "T2", "T2T"),
                     ("T8", "T4T", "T4"), ("T8T", "T4", "T4T"),
                     ("T16", "T8T", "T8")]

            def blkg(name):
                return pdc.tile([128, 2, 128], F16, tag=name, name=name,
                                bufs=3)

            fw1h_r = io["fw1h"][:].rearrange("(kt p) m -> p kt m", p=128)
            _fus_state = {"hst": None}

            def _emit_fusion(step):
                lw, mt = step // 8, step % 8
                if mt == 0:
                    hst = pd1.tile([128, 8, 512], F16, tag="hst", bufs=1,
                                   name="hst")
                    nc.sync.dma_start(out=hst,
                                      in_=hsT_r[:, :, lw*512:(lw+1)*512])
                    _fus_state["hst"] = hst
                hst = _fus_state["hst"]
                fwt = pd1.tile([128, 8, 128], F16, tag="fwt", bufs=3,
                               name=f"fwt{mt}")
                nc.sync.dma_start(out=fwt,
                                  in_=fw1h_r[:, :, mt*128:(mt+1)*128])
                ph = psD.tile([128, 512], F32, tag="pX5", bufs=2,
                              name="psh")
                for kt in range(8):
                    nc.tensor.matmul(ph, fwt[:, kt, :], hst[:, kt, :],
                                     start=(kt == 0), stop=(kt == 7))
                if mt % 2 == 0:
                    nc.scalar.copy(
                        out=hdn_pre[:, mt, lw*512:(lw+1)*512], in_=ph)
                else:
                    nc.vector.tensor_copy(
                        out=hdn_pre[:, mt, lw*512:(lw+1)*512], in_=ph)

            for g0 in range(0, NCH, GRP):
                cis = list(range(g0, min(g0 + GRP, NCH)))
                ls = {ci: slice(ci*128, (ci+1)*128) for ci in cis}
                t = {ci: {} for ci in cis}

                # step: G = K K^T, mask+beta -> Td, To
                for ci in cis:
                    pG = pd_("pG")
                    mm2(pG, lambda h, kt: kT[:, h, kt, ls[ci]],
                        lambda h, kt: kT[:, h, kt, ls[ci]], n_k=2)
                    t[ci]["Td"], t[ci]["To"] = blkg("Td"), blkg("To")
                    for h in range(2):
                        nc.vector.scalar_tensor_tensor(
                            out=t[ci]["Td"][:, h, :], in0=pG[:, h, :],
                            scalar=beta_lp[:, ci, h:h+1], in1=masks[:, 0, :],
                            op0=ALU.mult, op1=ALU.mult)
                        nc.vector.scalar_tensor_tensor(
                            out=t[ci]["To"][:, h, :], in0=pG[:, h, :],
                            scalar=beta_lp[:, ci, h:h+1], in1=masks[:, 1, :],
                            op0=ALU.mult, op1=ALU.mult)
                # step: TdT = transpose(Td)
                for ci in cis:
                    pT = psD.tile([128, 2, 128], F16, tag="pdT", bufs=2,
                                  name="pTdT")
                    for h in range(2):
                        nc.tensor.transpose(pT[:, h, :], t[ci]["Td"][:, h, :],
                                            ident16)
                    t[ci]["TdT"] = blkg("TdT")
                    nc.scalar.copy(out=t[ci]["TdT"], in_=pT)
                # steps: squaring chain
                for si_, (nm, ln, rn) in enumerate(specs):
                    for ci in cis:
                        pq2 = pd_("pq2")
                        mm2(pq2, lambda h, kt, a=t[ci][ln]: a[:, h, :],
                            lambda h, kt, b_=t[ci][rn]: b_[:, h, :])
                        t[ci][nm] = blkg(nm)
                        if (si_ + ci) % 2 == 0:
                            nc.scalar.copy(out=t[ci][nm], in_=pq2)
                        else:
                            nc.vector.tensor_copy(out=t[ci][nm], in_=pq2)
                # steps: MT product chain -> DT
                for ci in cis:
                    MT = pdc.tile([128, 2, 128], F16, tag="MT", name="MT",
                                  bufs=2 * GRP)
                    nc.vector.tensor_tensor(out=MT, in0=t[ci]["TdT"],
                                            in1=bc_mid(ident16, 2),
                                            op=ALU.add)
                    t[ci]["MT"] = MT
                for nm in ("T2", "T4", "T8", "T16"):
                    for ci in cis:
                        pm = pd_("pm")
                        for h in range(2):
                            nc.tensor.matmul(pm[:, h, :], t[ci][nm][:, h, :],
                                             t[ci]["MT"][:, h, :],
                                             start=True, stop=False)
                            nc.tensor.matmul(pm[:, h, :], ident16,
                                             t[ci]["MT"][:, h, :],
                                             start=False, stop=True)
                        MTn = pdc.tile([128, 2, 128], F16, tag="MT",
                                       name="MT", bufs=2 * GRP)
                        if ci % 2 == 0:
                            nc.scalar.copy(out=MTn, in_=pm)
                        else:
                            nc.vector.tensor_copy(out=MTn, in_=pm)
                        t[ci]["MT"] = MTn
                # steps: B, BT, B2T
                for ci in cis:
                    pB = pd_("pB")
                    mm2(pB, lambda h, kt: t[ci]["MT"][:, h, :],
                        lambda h, kt: t[ci]["To"][:, h, :])
                    t[ci]["Bm"] = blkg("Bm")
                    nc.scalar.copy(out=t[ci]["Bm"], in_=pB)
                for ci in cis:
                    pBT = pd_("pBT")
                    mm2(pBT, lambda h, kt: t[ci]["To"][:, h, :],
                        lambda h, kt: t[ci]["MT"][:, h, :])
                    t[ci]["BT"] = blkg("BT")
                    nc.vector.tensor_copy(out=t[ci]["BT"], in_=pBT)
                for ci in cis:
                    pB2 = pd_("pB2")
                    mm2(pB2, lambda h, kt: t[ci]["Bm"][:, h, :],
                        lambda h, kt: t[ci]["BT"][:, h, :])
                    t[ci]["B2T"] = blkg("B2T")
                    nc.scalar.copy(out=t[ci]["B2T"], in_=pB2)
                # step: aT
                for ci in cis:
                    pA4 = pd_("pA4")
                    mm2(pA4, lambda h, kt: kT[:, h, kt, ls[ci]],
                        lambda h, kt: qT[:, h, kt, ls[ci]], n_k=2)
                    nc.vector.tensor_tensor(out=aT_all[:, ci], in0=pA4,
                                            in1=bc_mid(masks[:, 3, :], 2),
                                            op=ALU.mult)
                # steps: X = [beta*v | beta*k], 3-stage apply
                for ci in cis:
                    X = pdw.tile([128, 2, 512], F16, tag="X", bufs=4,
                                 name="X")
                    for h in range(2):
                        nc.vector.tensor_scalar_mul(
                            X[:, h, 0:256].rearrange("p (a b) -> p a b", a=2),
                            vlc[:, h, :, ci, :], beta_lp[:, ci, h:h+1])
                        nc.vector.tensor_scalar_mul(
                            X[:, h, 256:512].rearrange(
                                "p (a b) -> p a b", a=2),
                            klc[:, h, :, ci, :], beta_lp[:, ci, h:h+1])
                    t[ci]["X"] = X
                for ci in cis:
                    x1t = pdw.tile([128, 2, 512], F16, tag="x1t", bufs=3,
                                   name="x1t")
                    for h in range(2):
                        px = psD.tile([128, 512], F32, tag="pX5", bufs=2,
                                      name="pX1")
                        nc.tensor.matmul(px, t[ci]["MT"][:, h, :],
                                         t[ci]["X"][:, h, :],
                                         start=True, stop=True)
                        if h == 0:
                            nc.scalar.copy(out=x1t[:, h, :], in_=px)
                        else:
                            nc.vector.tensor_copy(out=x1t[:, h, :], in_=px)
                    t[ci]["x1t"] = x1t
                for ci in cis:
                    y1t = pdw.tile([128, 2, 512], F16, tag="y1t", bufs=3,
                                   name="y1t")
                    for h in range(2):
                        px = psD.tile([128, 512], F32, tag="pX5", bufs=2,
                                      name="pX2")
                        nc.tensor.matmul(px, t[ci]["B2T"][:, h, :],
                                         t[ci]["x1t"][:, h, :],
                                         start=True, stop=False)
                        nc.tensor.matmul(px, ident16, t[ci]["x1t"][:, h, :],
                                         start=False, stop=True)
                        if h == 0:
                            nc.vector.tensor_copy(out=y1t[:, h, :], in_=px)
                        else:
                            nc.scalar.copy(out=y1t[:, h, :], in_=px)
                    t[ci]["y1t"] = y1t
                for ci in cis:
                    wtmp = pdw.tile([128, 2, 256], F16, tag="wtmp", bufs=3,
                                    name="wtmp")
                    for h in range(2):
                        px = psD.tile([128, 512], F32, tag="pX5", bufs=2,
                                      name="pX3")
                        nc.tensor.matmul(px, t[ci]["BT"][:, h, :],
                                         t[ci]["y1t"][:, h, :],
                                         start=True, stop=False)
                        nc.tensor.matmul(px, ident16, t[ci]["y1t"][:, h, :],
                                         start=False, stop=True)
                        nc.vector.tensor_copy(out=u_all[:, ci, h, :],
                                              in_=px[:, 0:256])
                        nc.scalar.copy(out=wtmp[:, h, :], in_=px[:, 256:512])
                    t[ci]["wtmp"] = wtmp
                for ci in cis:
                    for kt in range(2):
                        ptw = psD.tile([128, 2, 128], F16, tag="pdT", bufs=2,
                                       name="ptw")
                        for h in range(2):
                            nc.tensor.transpose(
                                ptw[:, h, :],
                                t[ci]["wtmp"][:, h, kt*128:(kt+1)*128],
                                ident16)
                        nc.scalar.mul(out=wT_all[:, ci, kt], in_=ptw,
                                      mul=-1.0)

                ngrp = (NCH + GRP - 1) // GRP
                gi = g0 // GRP
                for fstep in range(32 * gi // ngrp, 32 * (gi + 1) // ngrp):
                    _emit_fusion(fstep)

                # --- serial S part (per chunk) ---------------------------
                for ci in cis:
                    pup = psD.tile([128, 2, d], F32, tag="pS", bufs=1,
                                   name="pup")
                    for h in range(2):
                        for kt in range(2):
                            nc.tensor.matmul(pup[:, h, :],
                                             wT_all[:, ci, kt, h, :],
                                             S16[:, h, kt, :],
                                             start=(kt == 0), stop=False)
                        nc.tensor.matmul(pup[:, h, :], ident16,
                                         u_all[:, ci, h, :],
                                         start=False, stop=True)
                    # pup now holds u - w^T S = upr directly (wT is negated)
                    uprt = pdw.tile([128, 2, d], F16, tag="uprt", bufs=2,
                                    name="uprt")
                    nc.scalar.copy(out=uprt, in_=pup)

                    po = psD.tile([128, 2, d], F32, tag="pS", bufs=1,
                                  name="po")
                    for h in range(2):
                        for kt in range(2):
                            nc.tensor.matmul(po[:, h, :],
                                             qT[:, h, kt, ls[ci]],
                                             S16[:, h, kt, :],
                                             start=(kt == 0), stop=False)
                        nc.tensor.matmul(po[:, h, :], aT_all[:, ci, h, :],
                                         uprt[:, h, :],
                                         start=False, stop=True)
                    dsb = pdw.tile([128, 2, d], F16, tag="dsb", bufs=1,
                                   name="dsb")
                    nc.scalar.copy(out=dsb, in_=po)
                    nc.gpsimd.dma_start(
                        out=sc["dout_s"][ls[ci], :],
                        in_=dsb.rearrange("p h e -> p (h e)"))
                    nc.vector.tensor_reduce(
                        out=bn_sb[:, ci, 2:4], in_=dsb,
                        axis=mybir.AxisListType.X, op=ALU.add,
                        apply_absolute_value=True)

                    for h in range(2):
                        pdS = psD.tile([128, 2, d], F32, tag="pS", bufs=1,
                                       name=f"pdS{h}")
                        for kt in range(2):
                            nc.tensor.matmul(pdS[:, kt, :],
                                             klc[:, h, kt, ci, :],
                                             uprt[:, h, :],
                                             start=True, stop=True)
                        nc.vector.scalar_tensor_tensor(
                            out=S16[:, h], in0=pdS, scalar=1.0,
                            in1=S16[:, h], op0=ALU.mult, op1=ALU.add)

            # ---- bn features finalize + AllGather (two L-halves) ---------
            for half, wait_ms in ((0, 0.34), (1, 0.43)):
                ctx_bn = tc.tile_wait_until(wait_ms)
                ctx_bn.__enter__()
                nm_i = "bn_in_a" if half == 0 else "bn_in_b"
                nm_o = "bn_out_a" if half == 0 else "bn_out_b"
                for lt in range(half * 8, half * 8 + 8):
                    cmt_b = pd1.tile([128, C], F16, tag="cmt_b", bufs=3,
                                     name="cmt_b")
                    nc.gpsimd.dma_start(
                        out=cmt_b, in_=sc["cm_out"][lt*128:(lt+1)*128, :])
                    nc.vector.tensor_reduce(
                        out=bn_sb[:, lt, 0:2],
                        in_=cmt_b.rearrange("p (h e) -> p h e", e=d),
                        axis=mybir.AxisListType.X, op=ALU.add,
                        apply_absolute_value=True)
                nc.vector.tensor_reduce(
                    out=bn_sb[:, half*8:half*8+8, 4:6],
                    in_=bnv4[:, half*8:half*8+8].rearrange(
                        "p lt (h t) -> p lt h t", t=2),
                    axis=mybir.AxisListType.X, op=ALU.add)
                nc.gpsimd.dma_start(
                    out=sc[nm_i][:].rearrange("(lt p) c -> p lt c", p=128),
                    in_=bn_sb[:, half*8:half*8+8, :])
                nc.gpsimd.collective_compute(
                    "AllGather", ALU.bypass, replica_groups=RG,
                    ins=[sc[nm_i][:]], outs=[sc[nm_o][:]])
                ctx_bn.__exit__(None, None, None)

        # =================== PHASE C ======================================
        with tc.tile_pool(name="pc1", bufs=1) as pc1, \
             tc.tile_pool(name="pc2", bufs=2) as pc2, \
             tc.tile_pool(name="pcs", bufs=4) as pcs, \
             tc.tile_pool(name="psC", bufs=1, space="PSUM") as psC:

            def psc(name, tag="pg"):
                return psC.tile([128, 512], F32, tag=tag, bufs=2, name=name)

            bnT = [pc1.tile([8, L], F16, name=f"bnT{m}") for m in range(2)]
            for m in range(2):
                for half in range(2):
                    nm_o = "bn_out_a" if half == 0 else "bn_out_b"
                    bng = pc2.tile([128, 8, 8], F32, tag="bng",
                                   name=f"bng{m}{half}")
                    nc.sync.dma_start(
                        out=bng,
                        in_=sc[nm_o][m].rearrange("(lt p) c -> p lt c",
                                                  p=128))
                    for lt8 in range(8):
                        lt = half * 8 + lt8
                        ptb = psc("ptb")
                        ptbv = ptb[0:8, 0:128]
                        nc.tensor.transpose(ptbv, bng[:, lt8, :], ident)
                        nc.scalar.mul(out=bnT[m][:, lt*128:(lt+1)*128],
                                      in_=ptbv, mul=1.0/d)

            # ---- fusion tail: bn part + gelu -----------------------------
            fb1_sb = pc1.tile([128, 8], F32)
            nc.sync.dma_start(out=fb1_sb,
                              in_=io["fb1"][:].rearrange("(m p) -> p m",
                                                         p=128))
            fw1b_sb = pc1.tile([8, 2, 1024], F16)
            nc.sync.dma_start(
                out=fw1b_sb,
                in_=io["fw1b"][:].rearrange("(m p) c -> p m c", p=8))
            for lw in range(NLW):
                for mt in range(8):
                    ph2 = psC.tile([128, 512], F32, tag="pf2", bufs=2, name="psh2")
                    for m in range(2):
                        nc.tensor.matmul(ph2,
                                         fw1b_sb[:, m, mt*128:(mt+1)*128],
                                         bnT[m][:, lw*512:(lw+1)*512],
                                         start=(m == 0), stop=(m == 1))
                    hdf = pcs.tile([128, 512], F16, tag="hdf", bufs=3,
                                   name="hdf")
                    nc.vector.scalar_tensor_tensor(
                        out=hdf, in0=ph2, scalar=1.0,
                        in1=hdn_pre[:, mt, lw*512:(lw+1)*512],
                        op0=ALU.mult, op1=ALU.add)
                    nc.scalar.activation(
                        out=hdn_pre[:, mt, lw*512:(lw+1)*512], in_=hdf,
                        func=AF.Gelu, bias=fb1_sb[:, mt:mt+1])

            fw2_sb = pc1.tile([128, 8, 12], F16)
            nc.sync.dma_start(
                out=fw2_sb,
                in_=io["fw2"][:].rearrange("(kt p) c -> p kt c", p=128))
            lg_sb = pc1.tile([128, NLT, 12], F32)
            for lt in range(NLT):
                pl = psc("psl")
                plv = pl[:, 0:12]
                for kt in range(8):
                    nc.tensor.matmul(plv, hdn_pre[:, kt, lt*128:(lt+1)*128],
                                     fw2_sb[:, kt, :],
                                     start=(kt == 0), stop=(kt == 7))
                nc.scalar.copy(out=lg_sb[:, lt, :], in_=plv)
            for m in range(2):
                nc.gpsimd.dma_start(
                    out=sc["lg_in"][m].rearrange("(lt p) c -> p lt c", p=128),
                    in_=lg_sb[:, :, m*6:(m+1)*6])
            nc.gpsimd.collective_compute(
                "ReduceScatter", ALU.add, replica_groups=RG,
                ins=[sc["lg_in"][:]], outs=[sc["lg_out"][:]])

            # ---- softmax gates -------------------------------------------
            b2_sb = pc1.tile([128, 6], F32)
            nc.sync.dma_start(out=b2_sb, in_=io["b2o"][:])
            lgo = pc1.tile([128, NLT, 2, 3], F32)
            nc.sync.dma_start(
                out=lgo,
                in_=sc["lg_out"][:].rearrange("(lt p) (h e) -> p lt h e",
                                              p=128, e=3))
            nc.vector.tensor_tensor(
                out=lgo, in0=lgo,
                in1=bass.AP(tensor=b2_sb.tensor, offset=b2_sb.offset,
                            ap=[b2_sb.ap[0], [0, NLT], [3, 2], [1, 3]]),
                op=ALU.add)
            rmax = pc1.tile([128, NLT, 2], F32)
            nc.vector.tensor_reduce(out=rmax, in_=lgo,
                                    axis=mybir.AxisListType.X, op=ALU.max)
            nc.vector.tensor_tensor(
                out=lgo, in0=lgo,
                in1=rmax[:, :, :, None].to_broadcast([128, NLT, 2, 3]),
                op=ALU.subtract)
            nc.scalar.activation(out=lgo, in_=lgo, func=AF.Exp)
            rsum = pc1.tile([128, NLT, 2], F32)
            nc.vector.tensor_reduce(out=rsum, in_=lgo,
                                    axis=mybir.AxisListType.X, op=ALU.add)
            nc.vector.reciprocal(out=rsum, in_=rsum)
            nc.vector.tensor_tensor(
                out=lgo, in0=lgo,
                in1=rsum[:, :, :, None].to_broadcast([128, NLT, 2, 3]),
                op=ALU.mult)

            # ---- gate mix + RMSNorm + Wo ---------------------------------
            wo_sb = pc1.tile([128, 4, D], F16)
            nc.sync.dma_start(
                out=wo_sb,
                in_=io["wo"][:].rearrange("(kt p) n -> p kt n", p=128))
            dout_sb = arena[:, 0:8192].rearrange("p (lt h e) -> p lt h e",
                                                 h=2, e=d)
            cm_sb = arena[:, 8192:16384].rearrange("p (lt c) -> p lt c",
                                                   c=C)
            nc.gpsimd.dma_start(
                out=cm_sb,
                in_=sc["cm_out"][:].rearrange("(lt p) c -> p lt c", p=128))
            nc.gpsimd.dma_start(
                out=dout_sb,
                in_=sc["dout_s"][:].rearrange("(lt p) (h e) -> p lt h e",
                                              p=128, e=d))
            for lt in range(NLT):
                dov = dout_sb[:, lt]
                o_t = pcs.tile([128, 2, d], F16, tag="o_t", name="o_t")
                ssq = pcs.tile([128, 2], F32, tag="ssq", name="ssq")
                scr = pcs.tile([128, d], F32, tag="scr", name="scr")
                for h in range(2):
                    nc.vector.tensor_scalar_mul(
                        o_t[:, h, :],
                        cm_sb[:, lt, h*256:(h+1)*256], lgo[:, lt, h, 0:1])
                    nc.vector.scalar_tensor_tensor(
                        out=o_t[:, h, :], in0=dov[:, h, :],
                        scalar=lgo[:, lt, h, 1:2], in1=o_t[:, h, :],
                        op0=ALU.mult, op1=ALU.add)
                    nc.vector.scalar_tensor_tensor(
                        out=o_t[:, h, :].rearrange("p (a b) -> p a b", a=2),
                        in0=vlc[:, h, :, lt, :],
                        scalar=lgo[:, lt, h, 2:3],
                        in1=o_t[:, h, :].rearrange("p (a b) -> p a b", a=2),
                        op0=ALU.mult, op1=ALU.add)
                    nc.scalar.activation(out=scr, in_=o_t[:, h, :],
                                         func=AF.Square,
                                         accum_out=ssq[:, h:h+1])
                nc.scalar.activation(out=ssq, in_=ssq, func=AF.Sqrt,
                                     scale=1.0/d, bias=eps5)
                nc.vector.reciprocal(out=ssq, in_=ssq)
                for h in range(2):
                    nc.vector.tensor_scalar_mul(o_t[:, h, :], o_t[:, h, :],
                                                ssq[:, h:h+1])
                pto = psC.tile([128, 4, 128], F16, tag="pto", bufs=2,
                               name="psto")
                for ct in range(4):
                    h, dt = ct // 2, ct % 2
                    nc.tensor.transpose(pto[:, ct, :],
                                        o_t[:, h, dt*128:(dt+1)*128],
                                        ident16)
                oT = pcs.tile([128, 4, 128], F16, tag="oT", name="oT")
                nc.vector.tensor_copy(out=oT, in_=pto)
                orow = pcs.tile([128, D], F32, tag="orow", name="orow")
                for nh in range(2):
                    pw = psC.tile([128, 512], F32, tag="pw", bufs=2, name="psw")
                    for ct in range(4):
                        nc.tensor.matmul(pw, oT[:, ct, :],
                                         wo_sb[:, ct, nh*512:(nh+1)*512],
                                         start=(ct == 0), stop=(ct == 3))
                    nc.scalar.copy(out=orow[:, nh*512:(nh+1)*512], in_=pw)
                nc.sync.dma_start(out=io["out_part"][lt*128:(lt+1)*128, :],
                                  in_=orow)


# ======================= host side =======================================

def _diag_tiles(w_own, taps, out_dtype):
    """w_own: (C, k) conv weights for this core's channels.
    Returns (4, k, 128, 128) diag tiles; tap dd uses column k-1-dd."""
    k = w_own.shape[1]
    out = np.zeros((4, k, 128, 128), dtype=out_dtype)
    for ct in range(4):
        for dd in range(k):
            np.fill_diagonal(out[ct, dd], w_own[ct*128:(ct+1)*128, k-1-dd])
    return out


def _host_inputs(inputs):
    hs = np.asarray(inputs["hidden_states"], np.float32)
    Wq = np.asarray(inputs["Wq"], np.float32)
    Wk = np.asarray(inputs["Wk"], np.float32)
    Wv = np.asarray(inputs["Wv"], np.float32)
    Wb = np.asarray(inputs["Wb"], np.float32)
    cq = np.asarray(inputs["conv_q_w"], np.float32)
    ck = np.asarray(inputs["conv_k_w"], np.float32)
    cv = np.asarray(inputs["conv_v_w"], np.float32)
    w3 = np.asarray(inputs["ms_w3"], np.float32)
    w15 = np.asarray(inputs["ms_w15"], np.float32)
    w31 = np.asarray(inputs["ms_w31"], np.float32)
    kmix = np.asarray(inputs["kernel_mix_w"], np.float32)
    cmix = np.asarray(inputs["channel_mixer_w"], np.float32)
    fw1 = np.asarray(inputs["fusion_w1"], np.float32)
    fb1 = np.asarray(inputs["fusion_b1"], np.float32)
    fw2 = np.asarray(inputs["fusion_w2"], np.float32)
    fb2 = np.asarray(inputs["fusion_b2"], np.float32)
    onw = np.asarray(inputs["o_norm_w"], np.float32)
    Wo = np.asarray(inputs["Wo"], np.float32)

    # combined kernel_mix -> channel_mixer matrix Q: (3D, D)
    Q = np.zeros((3 * D, D), np.float32)
    for h in range(H):
        Q[h*3*d:(h+1)*3*d] = kmix @ cmix[h*d:(h+1)*d]

    masks = np.zeros((5, 128, 128), np.float32)
    i_, j_ = np.mgrid[0:128, 0:128]
    blk = (i_ // 32) == (j_ // 32)
    masks[0] = -((i_ > j_) & blk).astype(np.float32)
    masks[1] = -((i_ > j_) & ~blk).astype(np.float32)
    masks[2] = -((j_ > i_) & blk).astype(np.float32)
    masks[3] = (j_ >= i_).astype(np.float32)
    masks[4] = np.eye(128, dtype=np.float32)

    Wo_s = Wo * np.tile(onw, H)[:, None]

    in_maps = []
    for c in range(8):
        b, r = divmod(c, 2)
        cs = slice(C*r, C*(r+1))
        qmix = np.concatenate(
            [Q[1024*s + C*r: 1024*s + C*r + C] for s in range(3)], 0)
        f8 = ml_dtypes.float8_e4m3
        mspair = np.zeros((4, 23, 2, 128, 128), np.float32)
        mssing = np.zeros((4, 3, 128, 128), np.float32)
        for ct in range(4):
            pi = 0
            for si, (w, ks) in enumerate(((w3, 3), (w15, 15), (w31, 31))):
                dg = _diag_tiles(w[cs], ks, np.float32)[ct] * 64.0
                for p_ in range((ks - 1) // 2):
                    mspair[ct, pi, 0] = dg[2*p_ + 1]
                    mspair[ct, pi, 1] = dg[2*p_]
                    pi += 1
                mssing[ct, si] = dg[ks - 1]
        cdiag = np.stack([_diag_tiles(w[cs], KQKV, np.float16)
                          for w in (cq, ck, cv)], 0)
        fw1b = np.zeros((16, 1024), np.float32)
        for m in range(2):
            for src in range(3):
                for h_ in range(2):
                    fw1b[m*8 + src*2 + h_] = \
                        fw1[D + src*4 + 2*m + h_, 1024*r:1024*(r+1)]
        fw2p = np.zeros((1024, 12), np.float32)
        b2o = np.zeros((6,), np.float32)
        for jm in range(2):
            for h_ in range(2):
                for br in range(3):
                    gcol = (2*jm + h_)*3 + br
                    fw2p[:, jm*6 + h_*3 + br] = fw2[1024*r:1024*(r+1), gcol]
        for h_ in range(2):
            for br in range(3):
                b2o[h_*3 + br] = fb2[(2*r + h_)*3 + br]
        m = {
            "hsT": np.ascontiguousarray(hs[b].T).astype(np.float16),
            "wq": np.ascontiguousarray(Wq[:, cs]).astype(np.float16),
            "wk": np.ascontiguousarray(Wk[:, cs]).astype(np.float16),
            "wv": np.ascontiguousarray(Wv[:, cs]).astype(np.float16),
            "wb": np.ascontiguousarray(Wb[:, 2*r:2*r+2]).astype(np.float16),
            "cdiag": cdiag,
            "mspair": mspair.astype(f8),
            "mssing": mssing.astype(f8),
            "qmix8": (qmix * 64.0).astype(f8),
            "fw1h": np.ascontiguousarray(
                fw1[:D, 1024*r:1024*(r+1)]).astype(np.float16),
            "fw1b": fw1b.astype(np.float16),
            "fb1": np.ascontiguousarray(fb1[1024*r:1024*(r+1)]),
            "fw2": fw2p.astype(np.float16),
            "b2o": np.tile(b2o, (128, 1)),
            "wo": np.ascontiguousarray(Wo_s[cs, :]).astype(np.float16),
            "masks": masks,
            "onesrow": np.ones((1, 128), np.float32),
            "onescol": np.ones((128, 1), np.float32),
            "ident16": np.eye(128, dtype=np.float16),
        }
        in_maps.append(m)
    return in_maps


_PROG = {}


def _get_program(debug=False):
    key = bool(debug)
    if key not in _PROG:
        _PROG[key] = build_program(debug=debug)
    return _PROG[key]


def run(inputs, debug=False, **kw):
    nc = _get_program(debug=debug)
    in_maps = _host_inputs(inputs)
    res = run_bass_kernel_spmd(nc, in_maps, list(range(8)), **kw)
    return res


def kernel(**inputs):
    res = run(inputs)
    out = np.zeros((B, L, D), np.float32)
    for b in range(B):
        out[b] = res.results[2*b]["out_part"] + res.results[2*b+1]["out_part"]
    return out


if __name__ == "__main__":
    nc = build_program()
    print("program built ok")


# revision 80
# speedup vs baseline: 1.0388x; 1.0122x over previous
"""DeltaNet-style block (nn_DeltaNet_31877247271438) on 8 trn2 NeuronCores.

Sharding: core c -> (batch b = c//2, pair-rank r = c%2).  Within a batch pair:
  - head-parallel: rank r owns heads {2r, 2r+1} (feature cols [512r, 512r+512))
  - cross-head mixes are K-split with pairwise collectives:
      * channel_mixer (folded with kernel_mix into one matrix Q): partial sums
        ReduceScatter'ed (fp16) so each core receives its own heads' ms_out
      * fusion-MLP hidden is column-split; logits partials ReduceScatter'ed
      * bn features AllGather'ed (tiny)
  - the final Wo matmul partials are summed on the host.

v2: inter-phase tensors stay in SBUF (no DRAM staging round-trips); l-major
forms of k/v come from DMA-xbar transposes; the delta rule's 16 chunk
inversions are batched (only the S recurrence is serial); beta is folded
into the mask multiply so the kb row form is never materialized; a manually
aliased SBUF arena lets phase-scoped tensors (hsT/xc, y, u/wT/aT, hdn_pre)
share the same bytes.
"""
import sys
sys.path.insert(0, '/opt/trn_rl_repo')

import numpy as np
import ml_dtypes

import concourse.bass as bass
import concourse.tile as tile
from concourse import bacc, mybir
from concourse.bass_utils import run_bass_kernel_spmd

F32 = mybir.dt.float32
F16 = mybir.dt.float16
F8 = mybir.dt.float8e4
DR = mybir.MatmulPerfMode.DoubleRow
FP8S = 64.0
AF = mybir.ActivationFunctionType
ALU = mybir.AluOpType

B, L, D, H = 4, 2048, 1024, 4
d = 256          # per-head dim
C = 512          # channels owned per core (2 heads)
NLT = 16         # l-tiles of 128
NLW = 4          # l-windows of 512
NCH = 16         # delta chunks of 128
KQKV = 4         # qkv conv taps
MSK = (3, 15, 31)
NTAPS = sum(MSK)  # 49
PADV = 32
RG = [[0, 1], [2, 3], [4, 5], [6, 7]]

ARENA = 36864    # f16 elems per partition in the aliased arena


def bc_mid(ap2, n):
    """[P, F] AP -> [P, n, F] with a 0-stride middle dim (free-dim bcast)."""
    assert len(ap2.ap) == 2
    return bass.AP(tensor=ap2.tensor, offset=ap2.offset,
                   ap=[ap2.ap[0], [0, n], ap2.ap[1]])


def build_program(debug=False):
    nc = bacc.Bacc("TRN2", target_bir_lowering=False, debug=False,
                   num_devices=8)

    io = {}
    io["hsT"] = nc.declare_dram_parameter("hsT", [D, L], F16, False)
    io["wq"] = nc.declare_dram_parameter("wq", [D, C], F16, False)
    io["wk"] = nc.declare_dram_parameter("wk", [D, C], F16, False)
    io["wv"] = nc.declare_dram_parameter("wv", [D, C], F16, False)
    io["wb"] = nc.declare_dram_parameter("wb", [D, 2], F16, False)
    io["cdiag"] = nc.declare_dram_parameter("cdiag", [3, 4, KQKV, 128, 128],
                                            F16, False)
    io["mspair"] = nc.declare_dram_parameter("mspair", [4, 23, 2, 128, 128],
                                             F8, False)
    io["mssing"] = nc.declare_dram_parameter("mssing", [4, 3, 128, 128],
                                             F8, False)
    io["qmix8"] = nc.declare_dram_parameter("qmix8", [12 * 128, D], F8, False)
    io["fw1h"] = nc.declare_dram_parameter("fw1h", [D, 1024], F16, False)
    io["fw1b"] = nc.declare_dram_parameter("fw1b", [16, 1024], F16, False)
    io["fb1"] = nc.declare_dram_parameter("fb1", [1024], F32, False)
    io["fw2"] = nc.declare_dram_parameter("fw2", [1024, 12], F16, False)
    io["b2o"] = nc.declare_dram_parameter("b2o", [128, 6], F32, False)
    io["wo"] = nc.declare_dram_parameter("wo", [C, D], F16, False)
    io["masks"] = nc.declare_dram_parameter("masks", [5, 128, 128], F32, False)
    io["onesrow"] = nc.declare_dram_parameter("onesrow", [1, 128], F32, False)
    io["onescol"] = nc.declare_dram_parameter("onescol", [128, 1], F32, False)
    io["ident16"] = nc.declare_dram_parameter("ident16", [128, 128], F16, False)
    io["out_part"] = nc.declare_dram_parameter("out_part", [L, D], F32, True)

    sc = {}
    sc["dout_s"] = nc.dram_tensor("dout_s", [L, C], F16)
    sc["cm_in"] = nc.dram_tensor("cm_in", [2, L, C], F16)
    sc["cm_out"] = nc.dram_tensor("cm_out", [L, C], F16)
    sc["bn_in_a"] = nc.dram_tensor("bn_in_a", [1024, 8], F32)
    sc["bn_out_a"] = nc.dram_tensor("bn_out_a", [2, 1024, 8], F32)
    sc["bn_in_b"] = nc.dram_tensor("bn_in_b", [1024, 8], F32)
    sc["bn_out_b"] = nc.dram_tensor("bn_out_b", [2, 1024, 8], F32)
    sc["lg_in"] = nc.dram_tensor("lg_in", [2, L, 6], F32)
    sc["lg_out"] = nc.dram_tensor("lg_out", [L, 6], F32)

    with tile.TileContext(nc) as tc:
        _body(nc, tc, io, sc)
    nc.compile()
    return nc


def _body(nc, tc, io, sc):
    from contextlib import ExitStack
    ctx = ExitStack()
    with ctx:
        consts = ctx.enter_context(tc.tile_pool(name="consts", bufs=1))
        outer = ctx.enter_context(tc.tile_pool(name="outer", bufs=1))

        masks = consts.tile([128, 5, 128], F32)
        nc.sync.dma_start(out=masks,
                          in_=io["masks"][:].rearrange("m p f -> p m f"))
        ident = masks[:, 4, :]
        ident16 = consts.tile([128, 128], F16)
        nc.sync.dma_start(out=ident16, in_=io["ident16"][:])
        onescol16 = consts.tile([128, 1], F16)
        nc.vector.memset(onescol16, 1.0)
        onesrow16 = consts.tile([1, 128], F16)
        nc.vector.memset(onesrow16, 1.0)
        eps6 = consts.tile([128, 1], F32)
        nc.vector.memset(eps6, 1e-6)
        eps5 = consts.tile([128, 1], F32)
        nc.vector.memset(eps5, 1e-5)

        beta_lp = outer.tile([128, NLT, 2], F32)
        S16 = outer.tile([128, 2, 2, d], F16)
        nc.vector.memset(S16, 0.0)
        qT = outer.tile([128, 2, 2, L], F16)       # (h, dtile, l) row form
        kT = outer.tile([128, 2, 2, L], F16)
        klc = outer.tile([128, 2, 2, NLT, 128], F16)  # (h, dtile, lt, dk)
        vlc = outer.tile([128, 2, 2, NLT, 128], F16)
        bn_sb = outer.tile([128, NLT, 8], F32)
        bnv4 = outer.tile([128, NLT, 4], F32)
        arena = outer.tile([128, ARENA], F16)

        # arena regions (manually aliased, lifetimes disjoint)
        hsT = arena[:, 0:16384].rearrange("p (kt l) -> p kt l", l=L)
        xc = arena[:, 16384:24592].rearrange("p (ct l) -> p ct l", l=L + 4)
        vt_bf = arena[:, 24592:32912].rearrange("p (ct l) -> p ct l",
                                                l=PADV + L)
        y_bf = arena[:, 0:24576].rearrange("p (j l) -> p j l", l=L)
        u_all = arena[:, 0:2048].rearrange("p (n h e) -> p n h e", h=2, e=d)
        wT_all = arena[:, 2048:4096].rearrange("p (n kt h e) -> p n kt h e",
                                               kt=2, h=2, e=128)
        aT_all = arena[:, 4096:5120].rearrange("p (n h e) -> p n h e",
                                               h=2, e=128)
        hdn_pre = arena[:, 20480:36864].rearrange("p (mt l) -> p mt l", l=L)

        hsT_r = io["hsT"][:].rearrange("(kt p) l -> p kt l", p=128)

        # =================== PHASE A ======================================
        with tc.tile_pool(name="pa1", bufs=1) as pa1, \
             tc.tile_pool(name="pa2", bufs=2) as pa2, \
             tc.tile_pool(name="pas", bufs=3) as pas, \
             tc.tile_pool(name="psA", bufs=1, space="PSUM") as psA:
            nc.sync.dma_start(out=hsT, in_=hsT_r)

            # ---- beta (l-partition form) ---------------------------------
            wb_sb = pa1.tile([128, 8, 2], F16)
            nc.sync.dma_start(
                out=wb_sb, in_=io["wb"][:].rearrange("(kt p) c -> p kt c",
                                                     p=128))
            for lt in range(NLT):
                pb = psA.tile([128, 2], F32, tag="pb", bufs=1, name="psb")
                for kt in range(8):
                    nc.tensor.matmul(pb, hsT[:, kt, lt*128:(lt+1)*128],
                                     wb_sb[:, kt, :],
                                     start=(kt == 0), stop=(kt == 7))
                nc.scalar.activation(out=beta_lp[:, lt, :], in_=pb,
                                     func=AF.Sigmoid)

            # ---- q, k, v: proj -> conv -> silu -> (norm) -----------------
            for tnm, ti in (("v", 2), ("k", 1), ("q", 0)):
                PAD = 4
                w_sb = pa2.tile([128, 8, C], F16, tag="w_sb", bufs=2,
                                name=f"w_{tnm}")
                nc.sync.dma_start(
                    out=w_sb,
                    in_=io["w" + tnm][:].rearrange("(kt p) c -> p kt c",
                                                   p=128))
                cdg = pa2.tile([128, 4, KQKV, 128], F16, tag="cdg", bufs=2,
                               name=f"cdg_{tnm}")
                nc.sync.dma_start(
                    out=cdg,
                    in_=io["cdiag"][ti].rearrange("ct tap p f -> p ct tap f"))
                if tnm != "v":
                    nc.vector.memset(xc[:, :, 0:4], 0.0)
                else:
                    nc.vector.memset(vt_bf[:, :, 0:PADV], 0.0)

                for ct in range(4):
                    xp = pa2.tile([128, 4 + L], F16, tag="xp", bufs=2,
                                  name=f"xp_{tnm}{ct}")
                    nc.vector.memset(xp[:, 0:4], 0.0)
                    for lw in range(NLW):
                        pp = psA.tile([128, 512], F32, tag="pp", bufs=2,
                                      name="psp")
                        for kt in range(8):
                            nc.tensor.matmul(
                                pp, w_sb[:, kt, ct*128:(ct+1)*128],
                                hsT[:, kt, lw*512:(lw+1)*512],
                                start=(kt == 0), stop=(kt == 7))
                        nc.scalar.copy(out=xp[:, 4+lw*512:4+(lw+1)*512],
                                       in_=pp)
                    for lw in range(NLW):
                        pc = psA.tile([128, 512], F32, tag="pc", bufs=2,
                                      name="psc")
                        for dd in range(KQKV):
                            off = 4 + lw*512 - dd
                            nc.tensor.matmul(
                                pc, cdg[:, ct, dd, :], xp[:, off:off+512],
                                start=(dd == 0), stop=(dd == KQKV-1))
                        if tnm == "v":
                            nc.scalar.activation(
                                out=vt_bf[:, ct, PADV+lw*512:PADV+(lw+1)*512],
                                in_=pc, func=AF.Silu)
                        else:
                            nc.scalar.activation(
                                out=xc[:, ct, PAD+lw*512:PAD+(lw+1)*512],
                                in_=pc, func=AF.Silu)
                if tnm == "v":
                    for ct in range(4):
                        nc.sync.dma_start_transpose(
                            out=vlc[:, ct // 2, ct % 2],
                            in_=vt_bf[:, ct, PADV:PADV+L])
                    continue

                # ---- l2norm -> qT / kT row forms -------------------------
                dst = qT if tnm == "q" else kT
                for h in range(2):
                    for lw in range(NLW):
                        lsl = slice(PAD+lw*512, PAD+(lw+1)*512)
                        osl = slice(lw*512, (lw+1)*512)
                        sqs = pas.tile([128, 2, 512], F16, tag="sq", bufs=3,
                                       name="sq")
                        for i, ct in enumerate((2*h, 2*h+1)):
                            nc.vector.tensor_tensor(
                                out=sqs[:, i, :], in0=xc[:, ct, lsl],
                                in1=xc[:, ct, lsl], op=ALU.mult)
                        ssr = pas.tile([1, 2, 512], F32, tag="ssr", bufs=2,
                                       name="ssr")
                        nc.gpsimd.tensor_reduce(
                            out=ssr, in_=sqs, axis=mybir.AxisListType.C,
                            op=ALU.add)
                        sr = pas.tile([1, 512], F32, tag="sr", bufs=2,
                                      name="sr")
                        nc.vector.tensor_tensor(out=sr, in0=ssr[:, 0, :],
                                                in1=ssr[:, 1, :], op=ALU.add)
                        nc.scalar.activation(out=sr, in_=sr, func=AF.Sqrt,
                                             bias=eps6[0:1, :])
                        srt = pas.tile([1, 512], F16, tag="srt", bufs=2,
                                       name="srt")
                        with nc.allow_low_precision("l2norm scale fp16"):
                            nc.vector.reciprocal(out=srt, in_=sr)
                        pbc = psA.tile([128, 512], F32, tag="pn2", bufs=2,
                                       name="psbc")
                        nc.tensor.matmul(pbc, onesrow16, srt,
                                         start=True, stop=True)
                        for dt in range(2):
                            ct = 2*h + dt
                            nc.vector.tensor_tensor(
                                out=dst[:, h, dt, osl], in0=xc[:, ct, lsl],
                                in1=pbc, op=ALU.mult)
                if tnm == "k":
                    for h in range(2):
                        for dt in range(2):
                            nc.sync.dma_start_transpose(
                                out=klc[:, h, dt], in_=kT[:, h, dt, :])

            # bn(v): |v| partial sums (finalized later)
            for lt in range(NLT):
                nc.vector.tensor_reduce(
                    out=bnv4[:, lt, :], in_=vlc[:, :, :, lt, :],
                    axis=mybir.AxisListType.X, op=ALU.add,
                    apply_absolute_value=True)

        # =================== PHASE B: multiscale conv + qmix ==============
        with tc.tile_pool(name="pb1", bufs=1) as pb1, \
             tc.tile_pool(name="pbm", bufs=2) as pbm, \
             tc.tile_pool(name="pbs", bufs=3) as pbs, \
             tc.tile_pool(name="psB", bufs=1, space="PSUM") as psB:
            y8 = arena[:, 0:12288].bitcast(F8).rearrange(
                "p (j l) -> p j l", l=L)
            vt8 = arena[:, 12288:16448].bitcast(F8).rearrange(
                "p (ct l) -> p ct l", l=PADV + L)
            with nc.allow_low_precision("fp8 conv input"):
                for ct in range(4):
                    nc.vector.tensor_copy(out=vt8[:, ct, :],
                                          in_=vt_bf[:, ct, :])

            def pair_ap(off, n, ct):
                a = vt8[:, ct, off:off+n]
                return bass.AP(tensor=a.tensor, offset=a.offset,
                               ap=[a.ap[0], [1, 2], a.ap[1]])

            for ct in range(4):
                msd8 = pbm.tile([128, 23, 2, 128], F8, tag=f"msd{ct % 2}",
                                bufs=1, name=f"msd8_{ct}")
                nc.gpsimd.dma_start(
                    out=msd8,
                    in_=io["mspair"][ct].rearrange("j t p f -> p j t f"))
                mss8 = pbm.tile([128, 3, 128], F8, tag=f"mss{ct % 2}",
                                bufs=1, name=f"mss8_{ct}")
                nc.gpsimd.dma_start(
                    out=mss8,
                    in_=io["mssing"][ct].rearrange("j p f -> p j f"))
                for lw in range(NLW):
                    base_pi = 0
                    for si, ks in enumerate(MSK):
                        npair = (ks - 1) // 2
                        py = psB.tile([128, 512], F32, tag="py", bufs=4,
                                      name="psy")
                        for p_ in range(npair):
                            off = PADV + lw*512 - (2*p_ + 1)
                            nc.tensor.matmul(
                                py, msd8[:, base_pi + p_, :, :],
                                pair_ap(off, 512, ct),
                                start=(p_ == 0), stop=False, perf_mode=DR)
                        off = PADV + lw*512 - (ks - 1)
                        nc.tensor.matmul(py, mss8[:, si, :],
                                         vt8[:, ct, off:off+512],
                                         start=False, stop=True)
                        nc.scalar.mul(
                            out=y8[:, si*4+ct, lw*512:(lw+1)*512],
                            in_=py, mul=1.0/FP8S)
                        base_pi += npair

            qmix8_sb = pb1.tile([128, 6, 2, D], F8)
            nc.gpsimd.dma_start(
                out=qmix8_sb,
                in_=io["qmix8"][:].rearrange("(pp j p) o -> p pp j o",
                                             j=2, p=128))
            cms_v = [arena[:, 33040+i*1024:33040+(i+1)*1024].rearrange(
                "p (o c) -> p o c", o=2) for i in range(3)]
            for lt in range(NLT):
                cms = cms_v[lt % 3]
                for oh in range(2):
                    pq = psB.tile([128, 512], F32, tag="pq", bufs=4,
                                  name="psq")
                    for p_ in range(6):
                        nc.tensor.matmul(
                            pq, y8[:, 2*p_:2*p_+2, lt*128:(lt+1)*128],
                            qmix8_sb[:, p_, :, oh*512:(oh+1)*512],
                            start=(p_ == 0), stop=(p_ == 5), perf_mode=DR)
                    if oh == 0:
                        nc.vector.tensor_scalar_mul(cms[:, oh, :], pq,
                                                    1.0/FP8S)
                    else:
                        nc.scalar.mul(out=cms[:, oh, :], in_=pq,
                                      mul=1.0/FP8S)
                nc.gpsimd.dma_start(
                    out=sc["cm_in"][:, lt*128:(lt+1)*128, :].rearrange(
                        "o l c -> l o c"),
                    in_=cms)
            nc.gpsimd.collective_compute(
                "ReduceScatter", ALU.add, replica_groups=RG,
                ins=[sc["cm_in"][:]], outs=[sc["cm_out"][:]])

        # =================== PHASE B3: delta rule =========================
        with tc.tile_pool(name="pd1", bufs=1) as pd1, \
             tc.tile_pool(name="pdc", bufs=1) as pdc, \
             tc.tile_pool(name="pdw", bufs=1) as pdw, \
             tc.tile_pool(name="psD", bufs=1, space="PSUM") as psD:

            def blk(name, bufs=2):
                return pdc.tile([128, 2, 128], F16, tag=name, name=name,
                                bufs=bufs)

            def pd_(name):
                return psD.tile([128, 2, 128], F32, tag="pd", bufs=3,
                                name=name)

            def mm2(pt, lhs_fn, rhs_fn, n_k=1):
                for h in range(2):
                    for kt in range(n_k):
                        nc.tensor.matmul(pt[:, h, :], lhs_fn(h, kt),
                                         rhs_fn(h, kt),
                                         start=(kt == 0), stop=(kt == n_k-1))

            GRP = 3
            specs = [("T2", "TdT", "Td"), ("T2T", "Td", "TdT"),
                     ("T4", "T2T", "T2"), ("T4T", "T2", "T2T"),
                     ("T8", "T4T", "T4"), ("T8T", "T4", "T4T"),
                     ("T16", "T8T", "T8")]

            def blkg(name):
                return pdc.tile([128, 2, 128], F16, tag=name, name=name,
                                bufs=3)

            fw1h_r = io["fw1h"][:].rearrange("(kt p) m -> p kt m", p=128)
            _fus_state = {"hst": None}

            def _emit_fusion(step):
                lw, mt = step // 8, step % 8
                if mt == 0:
                    hst = pd1.tile([128, 8, 512], F16, tag="hst", bufs=1,
                                   name="hst")
                    nc.sync.dma_start(out=hst,
                                      in_=hsT_r[:, :, lw*512:(lw+1)*512])
                    _fus_state["hst"] = hst
                hst = _fus_state["hst"]
                fwt = pd1.tile([128, 8, 128], F16, tag="fwt", bufs=3,
                               name=f"fwt{mt}")
                nc.sync.dma_start(out=fwt,
                                  in_=fw1h_r[:, :, mt*128:(mt+1)*128])
                ph = psD.tile([128, 512], F32, tag="pX5", bufs=3,
                              name="psh")
                for kt in range(8):
                    nc.tensor.matmul(ph, fwt[:, kt, :], hst[:, kt, :],
                                     start=(kt == 0), stop=(kt == 7))
                if mt % 2 == 0:
                    nc.scalar.copy(
                        out=hdn_pre[:, mt, lw*512:(lw+1)*512], in_=ph)
                else:
                    nc.vector.tensor_copy(
                        out=hdn_pre[:, mt, lw*512:(lw+1)*512], in_=ph)

            X_v = [arena[:, 5120+i*1024:5120+(i+1)*1024].rearrange(
                "p (h e) -> p h e", h=2) for i in range(4)]
            x1_v = [arena[:, 9216+i*1024:9216+(i+1)*1024].rearrange(
                "p (h e) -> p h e", h=2) for i in range(3)]
            y1_v = [arena[:, 12288+i*1024:12288+(i+1)*1024].rearrange(
                "p (h e) -> p h e", h=2) for i in range(3)]
            for g0 in range(0, NCH, GRP):
                cis = list(range(g0, min(g0 + GRP, NCH)))
                ls = {ci: slice(ci*128, (ci+1)*128) for ci in cis}
                t = {ci: {} for ci in cis}

                # step: G = K K^T, mask+beta -> Td, To
                for ci in cis:
                    pG = pd_("pG")
                    mm2(pG, lambda h, kt: kT[:, h, kt, ls[ci]],
                        lambda h, kt: kT[:, h, kt, ls[ci]], n_k=2)
                    t[ci]["Td"], t[ci]["To"] = blkg("Td"), blkg("To")
                    for h in range(2):
                        nc.vector.scalar_tensor_tensor(
                            out=t[ci]["Td"][:, h, :], in0=pG[:, h, :],
                            scalar=beta_lp[:, ci, h:h+1], in1=masks[:, 0, :],
                            op0=ALU.mult, op1=ALU.mult)
                        nc.vector.scalar_tensor_tensor(
                            out=t[ci]["To"][:, h, :], in0=pG[:, h, :],
                            scalar=beta_lp[:, ci, h:h+1], in1=masks[:, 1, :],
                            op0=ALU.mult, op1=ALU.mult)
                # step: TdT = transpose(Td)
                for ci in cis:
                    pT = psD.tile([128, 2, 128], F16, tag="pdT", bufs=1,
                                  name="pTdT")
                    for h in range(2):
                        nc.tensor.transpose(pT[:, h, :], t[ci]["Td"][:, h, :],
                                            ident16)
                    t[ci]["TdT"] = blkg("TdT")
                    nc.scalar.copy(out=t[ci]["TdT"], in_=pT)
                # steps: squaring chain
                for si_, (nm, ln, rn) in enumerate(specs):
                    for ci in cis:
                        pq2 = pd_("pq2")
                        mm2(pq2, lambda h, kt, a=t[ci][ln]: a[:, h, :],
                            lambda h, kt, b_=t[ci][rn]: b_[:, h, :])
                        t[ci][nm] = blkg(nm)
                        if (si_ + ci) % 2 == 0:
                            nc.scalar.copy(out=t[ci][nm], in_=pq2)
                        else:
                            nc.vector.tensor_copy(out=t[ci][nm], in_=pq2)
                # steps: MT product chain -> DT
                for ci in cis:
                    MT = pdc.tile([128, 2, 128], F16, tag="MT", name="MT",
                                  bufs=2 * GRP)
                    nc.vector.tensor_tensor(out=MT, in0=t[ci]["TdT"],
                                            in1=bc_mid(ident16, 2),
                                            op=ALU.add)
                    t[ci]["MT"] = MT
                for nm in ("T2", "T4", "T8", "T16"):
                    for ci in cis:
                        pm = pd_("pm")
                        for h in range(2):
                            nc.tensor.matmul(pm[:, h, :], t[ci][nm][:, h, :],
                                             t[ci]["MT"][:, h, :],
                                             start=True, stop=False)
                            nc.tensor.matmul(pm[:, h, :], ident16,
                                             t[ci]["MT"][:, h, :],
                                             start=False, stop=True)
                        MTn = pdc.tile([128, 2, 128], F16, tag="MT",
                                       name="MT", bufs=2 * GRP)
                        if ci % 2 == 0:
                            nc.scalar.copy(out=MTn, in_=pm)
                        else:
                            nc.vector.tensor_copy(out=MTn, in_=pm)
                        t[ci]["MT"] = MTn
                # steps: B, BT, B2T
                for ci in cis:
                    pB = pd_("pB")
                    mm2(pB, lambda h, kt: t[ci]["MT"][:, h, :],
                        lambda h, kt: t[ci]["To"][:, h, :])
                    t[ci]["Bm"] = blkg("Bm")
                    nc.scalar.copy(out=t[ci]["Bm"], in_=pB)
                for ci in cis:
                    pBT = pd_("pBT")
                    mm2(pBT, lambda h, kt: t[ci]["To"][:, h, :],
                        lambda h, kt: t[ci]["MT"][:, h, :])
                    t[ci]["BT"] = blkg("BT")
                    nc.vector.tensor_copy(out=t[ci]["BT"], in_=pBT)
                for ci in cis:
                    pB2 = pd_("pB2")
                    mm2(pB2, lambda h, kt: t[ci]["Bm"][:, h, :],
                        lambda h, kt: t[ci]["BT"][:, h, :])
                    t[ci]["B2T"] = blkg("B2T")
                    nc.scalar.copy(out=t[ci]["B2T"], in_=pB2)
                # step: aT
                for ci in cis:
                    pA4 = pd_("pA4")
                    mm2(pA4, lambda h, kt: kT[:, h, kt, ls[ci]],
                        lambda h, kt: qT[:, h, kt, ls[ci]], n_k=2)
                    nc.vector.tensor_tensor(out=aT_all[:, ci % 4], in0=pA4,
                                            in1=bc_mid(masks[:, 3, :], 2),
                                            op=ALU.mult)
                # steps: X = [beta*v | beta*k], 3-stage apply
                for ci in cis:
                    X = X_v[ci % 4]
                    for h in range(2):
                        nc.vector.tensor_scalar_mul(
                            X[:, h, 0:256].rearrange("p (a b) -> p a b", a=2),
                            vlc[:, h, :, ci, :], beta_lp[:, ci, h:h+1])
                        nc.vector.tensor_scalar_mul(
                            X[:, h, 256:512].rearrange(
                                "p (a b) -> p a b", a=2),
                            klc[:, h, :, ci, :], beta_lp[:, ci, h:h+1])
                    t[ci]["X"] = X
                for ci in cis:
                    x1t = x1_v[ci % 3]
                    for h in range(2):
                        px = psD.tile([128, 512], F32, tag="pX5", bufs=3,
                                      name="pX1")
                        nc.tensor.matmul(px, t[ci]["MT"][:, h, :],
                                         t[ci]["X"][:, h, :],
                                         start=True, stop=True)
                        if h == 0:
                            nc.scalar.copy(out=x1t[:, h, :], in_=px)
                        else:
                            nc.vector.tensor_copy(out=x1t[:, h, :], in_=px)
                    t[ci]["x1t"] = x1t
                for ci in cis:
                    y1t = y1_v[ci % 3]
                    for h in range(2):
                        px = psD.tile([128, 512], F32, tag="pX5", bufs=3,
                                      name="pX2")
                        nc.tensor.matmul(px, t[ci]["B2T"][:, h, :],
                                         t[ci]["x1t"][:, h, :],
                                         start=True, stop=False)
                        nc.tensor.matmul(px, ident16, t[ci]["x1t"][:, h, :],
                                         start=False, stop=True)
                        if h == 0:
                            nc.vector.tensor_copy(out=y1t[:, h, :], in_=px)
                        else:
                            nc.scalar.copy(out=y1t[:, h, :], in_=px)
                    t[ci]["y1t"] = y1t
                for ci in cis:
                    wtmp = pdw.tile([128, 2, 256], F16, tag="wtmp", bufs=3,
                                    name="wtmp")
                    for h in range(2):
                        px = psD.tile([128, 512], F32, tag="pX5", bufs=3,
                                      name="pX3")
                        nc.tensor.matmul(px, t[ci]["BT"][:, h, :],
                                         t[ci]["y1t"][:, h, :],
                                         start=True, stop=False)
                        nc.tensor.matmul(px, ident16, t[ci]["y1t"][:, h, :],
                                         start=False, stop=True)
                        nc.vector.tensor_copy(out=u_all[:, ci % 4, h, :],
                                              in_=px[:, 0:256])
                        nc.scalar.copy(out=wtmp[:, h, :], in_=px[:, 256:512])
                    t[ci]["wtmp"] = wtmp
                for ci in cis:
                    for kt in range(2):
                        ptw = psD.tile([128, 2, 128], F16, tag="pdT", bufs=1,
                                       name="ptw")
                        for h in range(2):
                            nc.tensor.transpose(
                                ptw[:, h, :],
                                t[ci]["wtmp"][:, h, kt*128:(kt+1)*128],
                                ident16)
                        nc.scalar.mul(out=wT_all[:, ci % 4, kt], in_=ptw,
                                      mul=-1.0)

                ngrp = (NCH + GRP - 1) // GRP
                gi = g0 // GRP
                for fstep in range(32 * gi // ngrp, 32 * (gi + 1) // ngrp):
                    _emit_fusion(fstep)

                # --- serial S part (per chunk) ---------------------------
                for ci in cis:
                    pup = psD.tile([128, 2, d], F32, tag="pS", bufs=1,
                                   name="pup")
                    for h in range(2):
                        for kt in range(2):
                            nc.tensor.matmul(pup[:, h, :],
                                             wT_all[:, ci % 4, kt, h, :],
                                             S16[:, h, kt, :],
                                             start=(kt == 0), stop=False)
                        nc.tensor.matmul(pup[:, h, :], ident16,
                                         u_all[:, ci % 4, h, :],
                                         start=False, stop=True)
                    # pup now holds u - w^T S = upr directly (wT is negated)
                    uprt = pdw.tile([128, 2, d], F16, tag="uprt", bufs=2,
                                    name="uprt")
                    nc.scalar.copy(out=uprt, in_=pup)

                    po = psD.tile([128, 2, d], F32, tag="pS", bufs=1,
                                  name="po")
                    for h in range(2):
                        for kt in range(2):
                            nc.tensor.matmul(po[:, h, :],
                                             qT[:, h, kt, ls[ci]],
                                             S16[:, h, kt, :],
                                             start=(kt == 0), stop=False)
                        nc.tensor.matmul(po[:, h, :], aT_all[:, ci % 4, h, :],
                                         uprt[:, h, :],
                                         start=False, stop=True)
                    dsb = pdw.tile([128, 2, d], F16, tag="dsb", bufs=1,
                                   name="dsb")
                    nc.scalar.copy(out=dsb, in_=po)
                    nc.gpsimd.dma_start(
                        out=sc["dout_s"][ls[ci], :],
                        in_=dsb.rearrange("p h e -> p (h e)"))
                    nc.vector.tensor_reduce(
                        out=bn_sb[:, ci, 2:4], in_=dsb,
                        axis=mybir.AxisListType.X, op=ALU.add,
                        apply_absolute_value=True)

                    for h in range(2):
                        pdS = psD.tile([128, 2, d], F32, tag="pS", bufs=1,
                                       name=f"pdS{h}")
                        for kt in range(2):
                            nc.tensor.matmul(pdS[:, kt, :],
                                             klc[:, h, kt, ci, :],
                                             uprt[:, h, :],
                                             start=True, stop=True)
                        nc.vector.scalar_tensor_tensor(
                            out=S16[:, h], in0=pdS, scalar=1.0,
                            in1=S16[:, h], op0=ALU.mult, op1=ALU.add)

            # ---- bn features finalize + AllGather (two L-halves) ---------
            for half, wait_ms in ((0, 0.34), (1, 0.43)):
                ctx_bn = tc.tile_wait_until(wait_ms)
                ctx_bn.__enter__()
                nm_i = "bn_in_a" if half == 0 else "bn_in_b"
                nm_o = "bn_out_a" if half == 0 else "bn_out_b"
                for lt in range(half * 8, half * 8 + 8):
                    cmt_b = pd1.tile([128, C], F16, tag="cmt_b", bufs=3,
                                     name="cmt_b")
                    nc.gpsimd.dma_start(
                        out=cmt_b, in_=sc["cm_out"][lt*128:(lt+1)*128, :])
                    nc.vector.tensor_reduce(
                        out=bn_sb[:, lt, 0:2],
                        in_=cmt_b.rearrange("p (h e) -> p h e", e=d),
                        axis=mybir.AxisListType.X, op=ALU.add,
                        apply_absolute_value=True)
                nc.vector.tensor_reduce(
                    out=bn_sb[:, half*8:half*8+8, 4:6],
                    in_=bnv4[:, half*8:half*8+8].rearrange(
                        "p lt (h t) -> p lt h t", t=2),
                    axis=mybir.AxisListType.X, op=ALU.add)
                nc.gpsimd.dma_start(
                    out=sc[nm_i][:].rearrange("(lt p) c -> p lt c", p=128),
                    in_=bn_sb[:, half*8:half*8+8, :])
                nc.gpsimd.collective_compute(
                    "AllGather", ALU.bypass, replica_groups=RG,
                    ins=[sc[nm_i][:]], outs=[sc[nm_o][:]])
                ctx_bn.__exit__(None, None, None)

            bnT = [pd1.tile([8, L], F16, name=f"bnT{m}") for m in range(2)]
            for m in range(2):
                for half in range(2):
                    nm_o = "bn_out_a" if half == 0 else "bn_out_b"
                    bng = pd1.tile([128, 8, 8], F32, tag="bng", bufs=2,
                                   name=f"bng{m}{half}")
                    nc.sync.dma_start(
                        out=bng,
                        in_=sc[nm_o][m].rearrange("(lt p) c -> p lt c",
                                                  p=128))
                    for lt8 in range(8):
                        lt = half * 8 + lt8
                        ptb = psD.tile([128, 512], F32, tag="pX5", bufs=3, name="ptb")
                        ptbv = ptb[0:8, 0:128]
                        nc.tensor.transpose(ptbv, bng[:, lt8, :], ident)
                        nc.scalar.mul(out=bnT[m][:, lt*128:(lt+1)*128],
                                      in_=ptbv, mul=1.0/d)

            # ---- fusion tail: bn part + gelu -----------------------------
            fb1_sb = pd1.tile([128, 8], F32)
            nc.sync.dma_start(out=fb1_sb,
                              in_=io["fb1"][:].rearrange("(m p) -> p m",
                                                         p=128))
            fw1b_sb = pd1.tile([8, 2, 1024], F16)
            nc.sync.dma_start(
                out=fw1b_sb,
                in_=io["fw1b"][:].rearrange("(m p) c -> p m c", p=8))
            for lw in range(NLW):
                for mt in range(8):
                    ph2 = psD.tile([128, 512], F32, tag="pX5", bufs=3, name="psh2")
                    for m in range(2):
                        nc.tensor.matmul(ph2,
                                         fw1b_sb[:, m, mt*128:(mt+1)*128],
                                         bnT[m][:, lw*512:(lw+1)*512],
                                         start=(m == 0), stop=(m == 1))
                    hdf = pdw.tile([128, 512], F16, tag="hdf", bufs=3,
                                   name="hdf")
                    nc.vector.scalar_tensor_tensor(
                        out=hdf, in0=ph2, scalar=1.0,
                        in1=hdn_pre[:, mt, lw*512:(lw+1)*512],
                        op0=ALU.mult, op1=ALU.add)
                    nc.scalar.activation(
                        out=hdn_pre[:, mt, lw*512:(lw+1)*512], in_=hdf,
                        func=AF.Gelu, bias=fb1_sb[:, mt:mt+1])

            fw2_sb = pd1.tile([128, 8, 12], F16)
            nc.sync.dma_start(
                out=fw2_sb,
                in_=io["fw2"][:].rearrange("(kt p) c -> p kt c", p=128))
            lg_sb = pd1.tile([128, NLT, 12], F32)
            for lt in range(NLT):
                pl = psD.tile([128, 512], F32, tag="pX5", bufs=3, name="psl")
                plv = pl[:, 0:12]
                for kt in range(8):
                    nc.tensor.matmul(plv, hdn_pre[:, kt, lt*128:(lt+1)*128],
                                     fw2_sb[:, kt, :],
                                     start=(kt == 0), stop=(kt == 7))
                nc.scalar.copy(out=lg_sb[:, lt, :], in_=plv)
            for m in range(2):
                nc.gpsimd.dma_start(
                    out=sc["lg_in"][m].rearrange("(lt p) c -> p lt c", p=128),
                    in_=lg_sb[:, :, m*6:(m+1)*6])
            nc.gpsimd.collective_compute(
                "ReduceScatter", ALU.add, replica_groups=RG,
                ins=[sc["lg_in"][:]], outs=[sc["lg_out"][:]])

        # =================== PHASE C ======================================
        with tc.tile_pool(name="pc1", bufs=1) as pc1, \
             tc.tile_pool(name="pc2", bufs=2) as pc2, \
             tc.tile_pool(name="pcs", bufs=4) as pcs, \
             tc.tile_pool(name="psC", bufs=1, space="PSUM") as psC:

            def psc(name, tag="pg"):
                return psC.tile([128, 512], F32, tag=tag, bufs=2, name=name)

            # ---- softmax gates -------------------------------------------
            b2_sb = pc1.tile([128, 6], F32)
            nc.sync.dma_start(out=b2_sb, in_=io["b2o"][:])
            lgo = pc1.tile([128, NLT, 2, 3], F32)
            nc.sync.dma_start(
                out=lgo,
                in_=sc["lg_out"][:].rearrange("(lt p) (h e) -> p lt h e",
                                              p=128, e=3))
            nc.vector.tensor_tensor(
                out=lgo, in0=lgo,
                in1=bass.AP(tensor=b2_sb.tensor, offset=b2_sb.offset,
                            ap=[b2_sb.ap[0], [0, NLT], [3, 2], [1, 3]]),
                op=ALU.add)
            rmax = pc1.tile([128, NLT, 2], F32)
            nc.vector.tensor_reduce(out=rmax, in_=lgo,
                                    axis=mybir.AxisListType.X, op=ALU.max)
            nc.vector.tensor_tensor(
                out=lgo, in0=lgo,
                in1=rmax[:, :, :, None].to_broadcast([128, NLT, 2, 3]),
                op=ALU.subtract)
            nc.scalar.activation(out=lgo, in_=lgo, func=AF.Exp)
            rsum = pc1.tile([128, NLT, 2], F32)
            nc.vector.tensor_reduce(out=rsum, in_=lgo,
                                    axis=mybir.AxisListType.X, op=ALU.add)
            nc.vector.reciprocal(out=rsum, in_=rsum)
            nc.vector.tensor_tensor(
                out=lgo, in0=lgo,
                in1=rsum[:, :, :, None].to_broadcast([128, NLT, 2, 3]),
                op=ALU.mult)

            # ---- gate mix + RMSNorm + Wo ---------------------------------
            wo_sb = pc1.tile([128, 4, D], F16)
            nc.sync.dma_start(
                out=wo_sb,
                in_=io["wo"][:].rearrange("(kt p) n -> p kt n", p=128))
            dout_sb = arena[:, 0:8192].rearrange("p (lt h e) -> p lt h e",
                                                 h=2, e=d)
            cm_sb = arena[:, 8192:16384].rearrange("p (lt c) -> p lt c",
                                                   c=C)
            nc.gpsimd.dma_start(
                out=cm_sb,
                in_=sc["cm_out"][:].rearrange("(lt p) c -> p lt c", p=128))
            nc.gpsimd.dma_start(
                out=dout_sb,
                in_=sc["dout_s"][:].rearrange("(lt p) (h e) -> p lt h e",
                                              p=128, e=d))
            for lt in range(NLT):
                dov = dout_sb[:, lt]
                o_t = pcs.tile([128, 2, d], F16, tag="o_t", name="o_t")
                ssq = pcs.tile([128, 2], F32, tag="ssq", name="ssq")
                scr = pcs.tile([128, d], F32, tag="scr", name="scr")
                for h in range(2):
                    nc.vector.tensor_scalar_mul(
                        o_t[:, h, :],
                        cm_sb[:, lt, h*256:(h+1)*256], lgo[:, lt, h, 0:1])
                    nc.vector.scalar_tensor_tensor(
                        out=o_t[:, h, :], in0=dov[:, h, :],
                        scalar=lgo[:, lt, h, 1:2], in1=o_t[:, h, :],
                        op0=ALU.mult, op1=ALU.add)
                    nc.vector.scalar_tensor_tensor(
                        out=o_t[:, h, :].rearrange("p (a b) -> p a b", a=2),
                        in0=vlc[:, h, :, lt, :],
                        scalar=lgo[:, lt, h, 2:3],
                        in1=o_t[:, h, :].rearrange("p (a b) -> p a b", a=2),
                        op0=ALU.mult, op1=ALU.add)
                    nc.scalar.activation(out=scr, in_=o_t[:, h, :],
                                         func=AF.Square,
                                         accum_out=ssq[:, h:h+1])
                nc.scalar.activation(out=ssq, in_=ssq, func=AF.Sqrt,
                                     scale=1.0/d, bias=eps5)
                nc.vector.reciprocal(out=ssq, in_=ssq)
                for h in range(2):
                    nc.vector.tensor_scalar_mul(o_t[:, h, :], o_t[:, h, :],
                                                ssq[:, h:h+1])
                pto = psC.tile([128, 4, 128], F16, tag="pto", bufs=2,
                               name="psto")
                for ct in range(4):
                    h, dt = ct // 2, ct % 2
                    nc.tensor.transpose(pto[:, ct, :],
                                        o_t[:, h, dt*128:(dt+1)*128],
                                        ident16)
                oT = pcs.tile([128, 4, 128], F16, tag="oT", name="oT")
                nc.vector.tensor_copy(out=oT, in_=pto)
                orow = pcs.tile([128, D], F32, tag="orow", name="orow")
                for nh in range(2):
                    pw = psC.tile([128, 512], F32, tag="pw", bufs=2, name="psw")
                    for ct in range(4):
                        nc.tensor.matmul(pw, oT[:, ct, :],
                                         wo_sb[:, ct, nh*512:(nh+1)*512],
                                         start=(ct == 0), stop=(ct == 3))
                    nc.scalar.copy(out=orow[:, nh*512:(nh+1)*512], in_=pw)
                nc.sync.dma_start(out=io["out_part"][lt*128:(lt+1)*128, :],
                                  in_=orow)


# ======================= host side =======================================

def _diag_tiles(w_own, taps, out_dtype):
    """w_own: (C, k) conv weights for this core's channels.
    Returns (4, k, 128, 128) diag tiles; tap dd uses column k-1-dd."""
    k = w_own.shape[1]
    out = np.zeros((4, k, 128, 128), dtype=out_dtype)
    for ct in range(4):
        for dd in range(k):
            np.fill_diagonal(out[ct, dd], w_own[ct*128:(ct+1)*128, k-1-dd])
    return out


def _host_inputs(inputs):
    hs = np.asarray(inputs["hidden_states"], np.float32)
    Wq = np.asarray(inputs["Wq"], np.float32)
    Wk = np.asarray(inputs["Wk"], np.float32)
    Wv = np.asarray(inputs["Wv"], np.float32)
    Wb = np.asarray(inputs["Wb"], np.float32)
    cq = np.asarray(inputs["conv_q_w"], np.float32)
    ck = np.asarray(inputs["conv_k_w"], np.float32)
    cv = np.asarray(inputs["conv_v_w"], np.float32)
    w3 = np.asarray(inputs["ms_w3"], np.float32)
    w15 = np.asarray(inputs["ms_w15"], np.float32)
    w31 = np.asarray(inputs["ms_w31"], np.float32)
    kmix = np.asarray(inputs["kernel_mix_w"], np.float32)
    cmix = np.asarray(inputs["channel_mixer_w"], np.float32)
    fw1 = np.asarray(inputs["fusion_w1"], np.float32)
    fb1 = np.asarray(inputs["fusion_b1"], np.float32)
    fw2 = np.asarray(inputs["fusion_w2"], np.float32)
    fb2 = np.asarray(inputs["fusion_b2"], np.float32)
    onw = np.asarray(inputs["o_norm_w"], np.float32)
    Wo = np.asarray(inputs["Wo"], np.float32)

    # combined kernel_mix -> channel_mixer matrix Q: (3D, D)
    Q = np.zeros((3 * D, D), np.float32)
    for h in range(H):
        Q[h*3*d:(h+1)*3*d] = kmix @ cmix[h*d:(h+1)*d]

    masks = np.zeros((5, 128, 128), np.float32)
    i_, j_ = np.mgrid[0:128, 0:128]
    blk = (i_ // 32) == (j_ // 32)
    masks[0] = -((i_ > j_) & blk).astype(np.float32)
    masks[1] = -((i_ > j_) & ~blk).astype(np.float32)
    masks[2] = -((j_ > i_) & blk).astype(np.float32)
    masks[3] = (j_ >= i_).astype(np.float32)
    masks[4] = np.eye(128, dtype=np.float32)

    Wo_s = Wo * np.tile(onw, H)[:, None]

    in_maps = []
    for c in range(8):
        b, r = divmod(c, 2)
        cs = slice(C*r, C*(r+1))
        qmix = np.concatenate(
            [Q[1024*s + C*r: 1024*s + C*r + C] for s in range(3)], 0)
        f8 = ml_dtypes.float8_e4m3
        mspair = np.zeros((4, 23, 2, 128, 128), np.float32)
        mssing = np.zeros((4, 3, 128, 128), np.float32)
        for ct in range(4):
            pi = 0
            for si, (w, ks) in enumerate(((w3, 3), (w15, 15), (w31, 31))):
                dg = _diag_tiles(w[cs], ks, np.float32)[ct] * 64.0
                for p_ in range((ks - 1) // 2):
                    mspair[ct, pi, 0] = dg[2*p_ + 1]
                    mspair[ct, pi, 1] = dg[2*p_]
                    pi += 1
                mssing[ct, si] = dg[ks - 1]
        cdiag = np.stack([_diag_tiles(w[cs], KQKV, np.float16)
                          for w in (cq, ck, cv)], 0)
        fw1b = np.zeros((16, 1024), np.float32)
        for m in range(2):
            for src in range(3):
                for h_ in range(2):
                    fw1b[m*8 + src*2 + h_] = \
                        fw1[D + src*4 + 2*m + h_, 1024*r:1024*(r+1)]
        fw2p = np.zeros((1024, 12), np.float32)
        b2o = np.zeros((6,), np.float32)
        for jm in range(2):
            for h_ in range(2):
                for br in range(3):
                    gcol = (2*jm + h_)*3 + br
                    fw2p[:, jm*6 + h_*3 + br] = fw2[1024*r:1024*(r+1), gcol]
        for h_ in range(2):
            for br in range(3):
                b2o[h_*3 + br] = fb2[(2*r + h_)*3 + br]
        m = {
            "hsT": np.ascontiguousarray(hs[b].T).astype(np.float16),
            "wq": np.ascontiguousarray(Wq[:, cs]).astype(np.float16),
            "wk": np.ascontiguousarray(Wk[:, cs]).astype(np.float16),
            "wv": np.ascontiguousarray(Wv[:, cs]).astype(np.float16),
            "wb": np.ascontiguousarray(Wb[:, 2*r:2*r+2]).astype(np.float16),
            "cdiag": cdiag,
            "mspair": mspair.astype(f8),
            "mssing": mssing.astype(f8),
            "qmix8": (qmix * 64.0).astype(f8),
            "fw1h": np.ascontiguousarray(
                fw1[:D, 1024*r:1024*(r+1)]).astype(np.float16),
            "fw1b": fw1b.astype(np.float16),
            "fb1": np.ascontiguousarray(fb1[1024*r:1024*(r+1)]),
            "fw2": fw2p.astype(np.float16),
            "b2o": np.tile(b2o, (128, 1)),
            "wo": np.ascontiguousarray(Wo_s[cs, :]).astype(np.float16),
            "masks": masks,
            "onesrow": np.ones((1, 128), np.float32),
            "onescol": np.ones((128, 1), np.float32),
            "ident16": np.eye(128, dtype=np.float16),
        }
        in_maps.append(m)
    return in_maps


_PROG = {}


def _get_program(debug=False):
    key = bool(debug)
    if key not in _PROG:
        _PROG[key] = build_program(debug=debug)
    return _PROG[key]


def run(inputs, debug=False, **kw):
    nc = _get_program(debug=debug)
    in_maps = _host_inputs(inputs)
    res = run_bass_kernel_spmd(nc, in_maps, list(range(8)), **kw)
    return res


def kernel(**inputs):
    res = run(inputs)
    out = np.zeros((B, L, D), np.float32)
    for b in range(B):
        out[b] = res.results[2*b]["out_part"] + res.results[2*b+1]["out_part"]
    return out


if __name__ == "__main__":
    nc = build_program()
    print("program built ok")


# revision 87
# speedup vs baseline: 1.0414x; 1.0026x over previous
"""DeltaNet-style block (nn_DeltaNet_31877247271438) on 8 trn2 NeuronCores.

Sharding: core c -> (batch b = c//2, pair-rank r = c%2).  Within a batch pair:
  - head-parallel: rank r owns heads {2r, 2r+1} (feature cols [512r, 512r+512))
  - cross-head mixes are K-split with pairwise collectives:
      * channel_mixer (folded with kernel_mix into one matrix Q): partial sums
        ReduceScatter'ed (fp16) so each core receives its own heads' ms_out
      * fusion-MLP hidden is column-split; logits partials ReduceScatter'ed
      * bn features AllGather'ed (tiny)
  - the final Wo matmul partials are summed on the host.

v2: inter-phase tensors stay in SBUF (no DRAM staging round-trips); l-major
forms of k/v come from DMA-xbar transposes; the delta rule's 16 chunk
inversions are batched (only the S recurrence is serial); beta is folded
into the mask multiply so the kb row form is never materialized; a manually
aliased SBUF arena lets phase-scoped tensors (hsT/xc, y, u/wT/aT, hdn_pre)
share the same bytes.
"""
import sys
sys.path.insert(0, '/opt/trn_rl_repo')

import numpy as np
import ml_dtypes

import concourse.bass as bass
import concourse.tile as tile
from concourse import bacc, mybir
from concourse.bass_utils import run_bass_kernel_spmd

F32 = mybir.dt.float32
F16 = mybir.dt.float16
F8 = mybir.dt.float8e4
DR = mybir.MatmulPerfMode.DoubleRow
FP8S = 64.0
AF = mybir.ActivationFunctionType
ALU = mybir.AluOpType

B, L, D, H = 4, 2048, 1024, 4
d = 256          # per-head dim
C = 512          # channels owned per core (2 heads)
NLT = 16         # l-tiles of 128
NLW = 4          # l-windows of 512
NCH = 16         # delta chunks of 128
KQKV = 4         # qkv conv taps
MSK = (3, 15, 31)
NTAPS = sum(MSK)  # 49
PADV = 32
RG = [[0, 1], [2, 3], [4, 5], [6, 7]]

ARENA = 36864    # f16 elems per partition in the aliased arena


def bc_mid(ap2, n):
    """[P, F] AP -> [P, n, F] with a 0-stride middle dim (free-dim bcast)."""
    assert len(ap2.ap) == 2
    return bass.AP(tensor=ap2.tensor, offset=ap2.offset,
                   ap=[ap2.ap[0], [0, n], ap2.ap[1]])


def build_program(debug=False):
    nc = bacc.Bacc("TRN2", target_bir_lowering=False, debug=False,
                   num_devices=8)

    io = {}
    io["hsT"] = nc.declare_dram_parameter("hsT", [D, L], F16, False)
    io["wq"] = nc.declare_dram_parameter("wq", [D, C], F16, False)
    io["wk"] = nc.declare_dram_parameter("wk", [D, C], F16, False)
    io["wv"] = nc.declare_dram_parameter("wv", [D, C], F16, False)
    io["wb"] = nc.declare_dram_parameter("wb", [D, 2], F16, False)
    io["cdiag"] = nc.declare_dram_parameter("cdiag", [3, 4, KQKV, 128, 128],
                                            F16, False)
    io["mspair"] = nc.declare_dram_parameter("mspair", [4, 23, 2, 128, 128],
                                             F8, False)
    io["mssing"] = nc.declare_dram_parameter("mssing", [4, 3, 128, 128],
                                             F8, False)
    io["qmix8"] = nc.declare_dram_parameter("qmix8", [12 * 128, D], F8, False)
    io["fw1h"] = nc.declare_dram_parameter("fw1h", [D, 1024], F16, False)
    io["fw1b"] = nc.declare_dram_parameter("fw1b", [16, 1024], F16, False)
    io["fb1"] = nc.declare_dram_parameter("fb1", [1024], F32, False)
    io["fw2"] = nc.declare_dram_parameter("fw2", [1024, 12], F16, False)
    io["b2o"] = nc.declare_dram_parameter("b2o", [128, 6], F32, False)
    io["wo"] = nc.declare_dram_parameter("wo", [C, D], F16, False)
    io["masks"] = nc.declare_dram_parameter("masks", [5, 128, 128], F32, False)
    io["onesrow"] = nc.declare_dram_parameter("onesrow", [1, 128], F32, False)
    io["onescol"] = nc.declare_dram_parameter("onescol", [128, 1], F32, False)
    io["ident16"] = nc.declare_dram_parameter("ident16", [128, 128], F16, False)
    io["out_part"] = nc.declare_dram_parameter("out_part", [L, D], F32, True)

    sc = {}
    sc["dout_s"] = nc.dram_tensor("dout_s", [L, C], F16)
    sc["cm_in"] = nc.dram_tensor("cm_in", [2, L, C], F16)
    sc["cm_out"] = nc.dram_tensor("cm_out", [L, C], F16)
    sc["bn_in_a"] = nc.dram_tensor("bn_in_a", [1024, 8], F32)
    sc["bn_out_a"] = nc.dram_tensor("bn_out_a", [2, 1024, 8], F32)
    sc["bn_in_b"] = nc.dram_tensor("bn_in_b", [512, 8], F32)
    sc["bn_out_b"] = nc.dram_tensor("bn_out_b", [2, 512, 8], F32)
    sc["bn_in_c"] = nc.dram_tensor("bn_in_c", [512, 8], F32)
    sc["bn_out_c"] = nc.dram_tensor("bn_out_c", [2, 512, 8], F32)
    sc["lg_in"] = nc.dram_tensor("lg_in", [2, L, 6], F32)
    sc["lg_out"] = nc.dram_tensor("lg_out", [L, 6], F32)

    with tile.TileContext(nc) as tc:
        _body(nc, tc, io, sc)
    nc.compile()
    return nc


def _body(nc, tc, io, sc):
    from contextlib import ExitStack
    ctx = ExitStack()
    with ctx:
        consts = ctx.enter_context(tc.tile_pool(name="consts", bufs=1))
        outer = ctx.enter_context(tc.tile_pool(name="outer", bufs=1))

        masks = consts.tile([128, 5, 128], F32)
        nc.sync.dma_start(out=masks,
                          in_=io["masks"][:].rearrange("m p f -> p m f"))
        ident = masks[:, 4, :]
        ident16 = consts.tile([128, 128], F16)
        nc.sync.dma_start(out=ident16, in_=io["ident16"][:])
        onescol16 = consts.tile([128, 1], F16)
        nc.vector.memset(onescol16, 1.0)
        onesrow16 = consts.tile([1, 128], F16)
        nc.vector.memset(onesrow16, 1.0)
        eps6 = consts.tile([128, 1], F32)
        nc.vector.memset(eps6, 1e-6)
        eps5 = consts.tile([128, 1], F32)
        nc.vector.memset(eps5, 1e-5)

        beta_lp = outer.tile([128, NLT, 2], F32)
        S16 = outer.tile([128, 2, 2, d], F16)
        nc.vector.memset(S16, 0.0)
        qT = outer.tile([128, 2, 2, L], F16)       # (h, dtile, l) row form
        kT = outer.tile([128, 2, 2, L], F16)
        klc = outer.tile([128, 2, 2, NLT, 128], F16)  # (h, dtile, lt, dk)
        vlc = outer.tile([128, 2, 2, NLT, 128], F16)
        bn_sb = outer.tile([128, NLT, 8], F32)
        bnv4 = outer.tile([128, NLT, 4], F32)
        arena = outer.tile([128, ARENA], F16)

        # arena regions (manually aliased, lifetimes disjoint)
        hsT = arena[:, 0:16384].rearrange("p (kt l) -> p kt l", l=L)
        xc = arena[:, 16384:24592].rearrange("p (ct l) -> p ct l", l=L + 4)
        vt_bf = arena[:, 24592:32912].rearrange("p (ct l) -> p ct l",
                                                l=PADV + L)
        y_bf = arena[:, 0:24576].rearrange("p (j l) -> p j l", l=L)
        u_all = arena[:, 0:2048].rearrange("p (n h e) -> p n h e", h=2, e=d)
        wT_all = arena[:, 2048:4096].rearrange("p (n kt h e) -> p n kt h e",
                                               kt=2, h=2, e=128)
        aT_all = arena[:, 4096:5120].rearrange("p (n h e) -> p n h e",
                                               h=2, e=128)
        hdn_pre = arena[:, 20480:36864].rearrange("p (mt l) -> p mt l", l=L)

        hsT_r = io["hsT"][:].rearrange("(kt p) l -> p kt l", p=128)

        # =================== PHASE A ======================================
        with tc.tile_pool(name="pa1", bufs=1) as pa1, \
             tc.tile_pool(name="pa2", bufs=2) as pa2, \
             tc.tile_pool(name="pas", bufs=3) as pas, \
             tc.tile_pool(name="psA", bufs=1, space="PSUM") as psA:
            nc.sync.dma_start(out=hsT, in_=hsT_r)

            # ---- beta (l-partition form) ---------------------------------
            wb_sb = pa1.tile([128, 8, 2], F16)
            nc.sync.dma_start(
                out=wb_sb, in_=io["wb"][:].rearrange("(kt p) c -> p kt c",
                                                     p=128))
            for lt in range(NLT):
                pb = psA.tile([128, 2], F32, tag="pb", bufs=1, name="psb")
                for kt in range(8):
                    nc.tensor.matmul(pb, hsT[:, kt, lt*128:(lt+1)*128],
                                     wb_sb[:, kt, :],
                                     start=(kt == 0), stop=(kt == 7))
                nc.scalar.activation(out=beta_lp[:, lt, :], in_=pb,
                                     func=AF.Sigmoid)

            # ---- q, k, v: proj -> conv -> silu -> (norm) -----------------
            for tnm, ti in (("v", 2), ("k", 1), ("q", 0)):
                PAD = 4
                w_sb = pa2.tile([128, 8, C], F16, tag="w_sb", bufs=2,
                                name=f"w_{tnm}")
                nc.sync.dma_start(
                    out=w_sb,
                    in_=io["w" + tnm][:].rearrange("(kt p) c -> p kt c",
                                                   p=128))
                cdg = pa2.tile([128, 4, KQKV, 128], F16, tag="cdg", bufs=2,
                               name=f"cdg_{tnm}")
                nc.sync.dma_start(
                    out=cdg,
                    in_=io["cdiag"][ti].rearrange("ct tap p f -> p ct tap f"))
                if tnm != "v":
                    nc.vector.memset(xc[:, :, 0:4], 0.0)
                else:
                    nc.vector.memset(vt_bf[:, :, 0:PADV], 0.0)

                for ct in range(4):
                    xp = pa2.tile([128, 4 + L], F16, tag="xp", bufs=2,
                                  name=f"xp_{tnm}{ct}")
                    nc.vector.memset(xp[:, 0:4], 0.0)
                    for lw in range(NLW):
                        pp = psA.tile([128, 512], F32, tag="pp", bufs=2,
                                      name="psp")
                        for kt in range(8):
                            nc.tensor.matmul(
                                pp, w_sb[:, kt, ct*128:(ct+1)*128],
                                hsT[:, kt, lw*512:(lw+1)*512],
                                start=(kt == 0), stop=(kt == 7))
                        nc.scalar.copy(out=xp[:, 4+lw*512:4+(lw+1)*512],
                                       in_=pp)
                    for lw in range(NLW):
                        pc = psA.tile([128, 512], F32, tag="pc", bufs=2,
                                      name="psc")
                        for dd in range(KQKV):
                            off = 4 + lw*512 - dd
                            nc.tensor.matmul(
                                pc, cdg[:, ct, dd, :], xp[:, off:off+512],
                                start=(dd == 0), stop=(dd == KQKV-1))
                        if tnm == "v":
                            nc.scalar.activation(
                                out=vt_bf[:, ct, PADV+lw*512:PADV+(lw+1)*512],
                                in_=pc, func=AF.Silu)
                        else:
                            nc.scalar.activation(
                                out=xc[:, ct, PAD+lw*512:PAD+(lw+1)*512],
                                in_=pc, func=AF.Silu)
                if tnm == "v":
                    for ct in range(4):
                        nc.sync.dma_start_transpose(
                            out=vlc[:, ct // 2, ct % 2],
                            in_=vt_bf[:, ct, PADV:PADV+L])
                    continue

                # ---- l2norm -> qT / kT row forms -------------------------
                dst = qT if tnm == "q" else kT
                for h in range(2):
                    for lw in range(NLW):
                        lsl = slice(PAD+lw*512, PAD+(lw+1)*512)
                        osl = slice(lw*512, (lw+1)*512)
                        sqs = pas.tile([128, 2, 512], F16, tag="sq", bufs=3,
                                       name="sq")
                        for i, ct in enumerate((2*h, 2*h+1)):
                            nc.vector.tensor_tensor(
                                out=sqs[:, i, :], in0=xc[:, ct, lsl],
                                in1=xc[:, ct, lsl], op=ALU.mult)
                        ssr = pas.tile([1, 2, 512], F32, tag="ssr", bufs=2,
                                       name="ssr")
                        nc.gpsimd.tensor_reduce(
                            out=ssr, in_=sqs, axis=mybir.AxisListType.C,
                            op=ALU.add)
                        sr = pas.tile([1, 512], F32, tag="sr", bufs=2,
                                      name="sr")
                        nc.vector.tensor_tensor(out=sr, in0=ssr[:, 0, :],
                                                in1=ssr[:, 1, :], op=ALU.add)
                        nc.scalar.activation(out=sr, in_=sr, func=AF.Sqrt,
                                             bias=eps6[0:1, :])
                        srt = pas.tile([1, 512], F16, tag="srt", bufs=2,
                                       name="srt")
                        with nc.allow_low_precision("l2norm scale fp16"):
                            nc.vector.reciprocal(out=srt, in_=sr)
                        pbc = psA.tile([128, 512], F32, tag="pn2", bufs=2,
                                       name="psbc")
                        nc.tensor.matmul(pbc, onesrow16, srt,
                                         start=True, stop=True)
                        for dt in range(2):
                            ct = 2*h + dt
                            nc.vector.tensor_tensor(
                                out=dst[:, h, dt, osl], in0=xc[:, ct, lsl],
                                in1=pbc, op=ALU.mult)
                if tnm == "k":
                    for h in range(2):
                        for dt in range(2):
                            nc.sync.dma_start_transpose(
                                out=klc[:, h, dt], in_=kT[:, h, dt, :])

            # bn(v): |v| partial sums (finalized later)
            for lt in range(NLT):
                nc.vector.tensor_reduce(
                    out=bnv4[:, lt, :], in_=vlc[:, :, :, lt, :],
                    axis=mybir.AxisListType.X, op=ALU.add,
                    apply_absolute_value=True)

        # =================== PHASE B: multiscale conv + qmix ==============
        with tc.tile_pool(name="pb1", bufs=1) as pb1, \
             tc.tile_pool(name="pbm", bufs=2) as pbm, \
             tc.tile_pool(name="pbs", bufs=3) as pbs, \
             tc.tile_pool(name="psB", bufs=1, space="PSUM") as psB:
            y8 = arena[:, 0:12288].bitcast(F8).rearrange(
                "p (j l) -> p j l", l=L)
            vt8 = arena[:, 12288:16448].bitcast(F8).rearrange(
                "p (ct l) -> p ct l", l=PADV + L)
            with nc.allow_low_precision("fp8 conv input"):
                for ct in range(4):
                    nc.vector.tensor_copy(out=vt8[:, ct, :],
                                          in_=vt_bf[:, ct, :])

            def pair_ap(off, n, ct):
                a = vt8[:, ct, off:off+n]
                return bass.AP(tensor=a.tensor, offset=a.offset,
                               ap=[a.ap[0], [1, 2], a.ap[1]])

            for ct in range(4):
                msd8 = pbm.tile([128, 23, 2, 128], F8, tag=f"msd{ct % 2}",
                                bufs=1, name=f"msd8_{ct}")
                nc.gpsimd.dma_start(
                    out=msd8,
                    in_=io["mspair"][ct].rearrange("j t p f -> p j t f"))
                mss8 = pbm.tile([128, 3, 128], F8, tag=f"mss{ct % 2}",
                                bufs=1, name=f"mss8_{ct}")
                nc.gpsimd.dma_start(
                    out=mss8,
                    in_=io["mssing"][ct].rearrange("j p f -> p j f"))
                for lw in range(NLW):
                    base_pi = 0
                    for si, ks in enumerate(MSK):
                        npair = (ks - 1) // 2
                        py = psB.tile([128, 512], F32, tag="py", bufs=4,
                                      name="psy")
                        for p_ in range(npair):
                            off = PADV + lw*512 - (2*p_ + 1)
                            nc.tensor.matmul(
                                py, msd8[:, base_pi + p_, :, :],
                                pair_ap(off, 512, ct),
                                start=(p_ == 0), stop=False, perf_mode=DR)
                        off = PADV + lw*512 - (ks - 1)
                        nc.tensor.matmul(py, mss8[:, si, :],
                                         vt8[:, ct, off:off+512],
                                         start=False, stop=True)
                        nc.scalar.mul(
                            out=y8[:, si*4+ct, lw*512:(lw+1)*512],
                            in_=py, mul=1.0/FP8S)
                        base_pi += npair

            qmix8_sb = pb1.tile([128, 6, 2, D], F8)
            nc.gpsimd.dma_start(
                out=qmix8_sb,
                in_=io["qmix8"][:].rearrange("(pp j p) o -> p pp j o",
                                             j=2, p=128))
            cms_v = [arena[:, 33040+i*1024:33040+(i+1)*1024].rearrange(
                "p (o c) -> p o c", o=2) for i in range(3)]
            for lt in range(NLT):
                cms = cms_v[lt % 3]
                for oh in range(2):
                    pq = psB.tile([128, 512], F32, tag="pq", bufs=4,
                                  name="psq")
                    for p_ in range(6):
                        nc.tensor.matmul(
                            pq, y8[:, 2*p_:2*p_+2, lt*128:(lt+1)*128],
                            qmix8_sb[:, p_, :, oh*512:(oh+1)*512],
                            start=(p_ == 0), stop=(p_ == 5), perf_mode=DR)
                    if oh == 0:
                        nc.vector.tensor_scalar_mul(cms[:, oh, :], pq,
                                                    1.0/FP8S)
                    else:
                        nc.scalar.mul(out=cms[:, oh, :], in_=pq,
                                      mul=1.0/FP8S)
                nc.gpsimd.dma_start(
                    out=sc["cm_in"][:, lt*128:(lt+1)*128, :].rearrange(
                        "o l c -> l o c"),
                    in_=cms)
            nc.gpsimd.collective_compute(
                "ReduceScatter", ALU.add, replica_groups=RG,
                ins=[sc["cm_in"][:]], outs=[sc["cm_out"][:]])

        # =================== PHASE B3: delta rule =========================
        with tc.tile_pool(name="pd1", bufs=1) as pd1, \
             tc.tile_pool(name="pdc", bufs=1) as pdc, \
             tc.tile_pool(name="pdw", bufs=1) as pdw, \
             tc.tile_pool(name="psD", bufs=1, space="PSUM") as psD:

            def blk(name, bufs=2):
                return pdc.tile([128, 2, 128], F16, tag=name, name=name,
                                bufs=bufs)

            def pd_(name):
                return psD.tile([128, 2, 128], F32, tag="pd", bufs=3,
                                name=name)

            def mm2(pt, lhs_fn, rhs_fn, n_k=1):
                for h in range(2):
                    for kt in range(n_k):
                        nc.tensor.matmul(pt[:, h, :], lhs_fn(h, kt),
                                         rhs_fn(h, kt),
                                         start=(kt == 0), stop=(kt == n_k-1))

            GRP = 3
            specs = [("T2", "TdT", "Td"), ("T2T", "Td", "TdT"),
                     ("T4", "T2T", "T2"), ("T4T", "T2", "T2T"),
                     ("T8", "T4T", "T4"), ("T8T", "T4", "T4T"),
                     ("T16", "T8T", "T8")]

            def blkg(name):
                return pdc.tile([128, 2, 128], F16, tag=name, name=name,
                                bufs=3)

            fw1h_r = io["fw1h"][:].rearrange("(kt p) m -> p kt m", p=128)
            _fus_state = {"hst": None}

            def _emit_fusion(step):
                lw, mt = step // 8, step % 8
                if mt == 0:
                    hst = pd1.tile([128, 8, 512], F16, tag="hst", bufs=1,
                                   name="hst")
                    nc.sync.dma_start(out=hst,
                                      in_=hsT_r[:, :, lw*512:(lw+1)*512])
                    _fus_state["hst"] = hst
                hst = _fus_state["hst"]
                fwt = pd1.tile([128, 8, 128], F16, tag="fwt", bufs=3,
                               name=f"fwt{mt}")
                nc.sync.dma_start(out=fwt,
                                  in_=fw1h_r[:, :, mt*128:(mt+1)*128])
                ph = psD.tile([128, 512], F32, tag="pX5", bufs=3,
                              name="psh")
                for kt in range(8):
                    nc.tensor.matmul(ph, fwt[:, kt, :], hst[:, kt, :],
                                     start=(kt == 0), stop=(kt == 7))
                if mt % 2 == 0:
                    nc.scalar.copy(
                        out=hdn_pre[:, mt, lw*512:(lw+1)*512], in_=ph)
                else:
                    nc.vector.tensor_copy(
                        out=hdn_pre[:, mt, lw*512:(lw+1)*512], in_=ph)

            X_v = [arena[:, 5120+i*1024:5120+(i+1)*1024].rearrange(
                "p (h e) -> p h e", h=2) for i in range(4)]
            x1_v = [arena[:, 9216+i*1024:9216+(i+1)*1024].rearrange(
                "p (h e) -> p h e", h=2) for i in range(3)]
            y1_v = [arena[:, 12288+i*1024:12288+(i+1)*1024].rearrange(
                "p (h e) -> p h e", h=2) for i in range(3)]
            for g0 in range(0, NCH, GRP):
                cis = list(range(g0, min(g0 + GRP, NCH)))
                ls = {ci: slice(ci*128, (ci+1)*128) for ci in cis}
                t = {ci: {} for ci in cis}

                # step: G = K K^T, mask+beta -> Td, To
                for ci in cis:
                    pG = pd_("pG")
                    mm2(pG, lambda h, kt: kT[:, h, kt, ls[ci]],
                        lambda h, kt: kT[:, h, kt, ls[ci]], n_k=2)
                    t[ci]["Td"], t[ci]["To"] = blkg("Td"), blkg("To")
                    for h in range(2):
                        nc.vector.scalar_tensor_tensor(
                            out=t[ci]["Td"][:, h, :], in0=pG[:, h, :],
                            scalar=beta_lp[:, ci, h:h+1], in1=masks[:, 0, :],
                            op0=ALU.mult, op1=ALU.mult)
                        nc.vector.scalar_tensor_tensor(
                            out=t[ci]["To"][:, h, :], in0=pG[:, h, :],
                            scalar=beta_lp[:, ci, h:h+1], in1=masks[:, 1, :],
                            op0=ALU.mult, op1=ALU.mult)
                # step: TdT = transpose(Td)
                for ci in cis:
                    pT = psD.tile([128, 2, 128], F16, tag="pdT", bufs=1,
                                  name="pTdT")
                    for h in range(2):
                        nc.tensor.transpose(pT[:, h, :], t[ci]["Td"][:, h, :],
                                            ident16)
                    t[ci]["TdT"] = blkg("TdT")
                    nc.scalar.copy(out=t[ci]["TdT"], in_=pT)
                # steps: squaring chain
                for si_, (nm, ln, rn) in enumerate(specs):
                    for ci in cis:
                        pq2 = pd_("pq2")
                        mm2(pq2, lambda h, kt, a=t[ci][ln]: a[:, h, :],
                            lambda h, kt, b_=t[ci][rn]: b_[:, h, :])
                        t[ci][nm] = blkg(nm)
                        if (si_ + ci) % 2 == 0:
                            nc.scalar.copy(out=t[ci][nm], in_=pq2)
                        else:
                            nc.vector.tensor_copy(out=t[ci][nm], in_=pq2)
                # steps: MT product chain -> DT
                for ci in cis:
                    MT = pdc.tile([128, 2, 128], F16, tag="MT", name="MT",
                                  bufs=2 * GRP)
                    nc.vector.tensor_tensor(out=MT, in0=t[ci]["TdT"],
                                            in1=bc_mid(ident16, 2),
                                            op=ALU.add)
                    t[ci]["MT"] = MT
                for nm in ("T2", "T4", "T8", "T16"):
                    for ci in cis:
                        pm = pd_("pm")
                        for h in range(2):
                            nc.tensor.matmul(pm[:, h, :], t[ci][nm][:, h, :],
                                             t[ci]["MT"][:, h, :],
                                             start=True, stop=False)
                            nc.tensor.matmul(pm[:, h, :], ident16,
                                             t[ci]["MT"][:, h, :],
                                             start=False, stop=True)
                        MTn = pdc.tile([128, 2, 128], F16, tag="MT",
                                       name="MT", bufs=2 * GRP)
                        if ci % 2 == 0:
                            nc.scalar.copy(out=MTn, in_=pm)
                        else:
                            nc.vector.tensor_copy(out=MTn, in_=pm)
                        t[ci]["MT"] = MTn
                # steps: B, BT, B2T
                for ci in cis:
                    pB = pd_("pB")
                    mm2(pB, lambda h, kt: t[ci]["MT"][:, h, :],
                        lambda h, kt: t[ci]["To"][:, h, :])
                    t[ci]["Bm"] = blkg("Bm")
                    nc.scalar.copy(out=t[ci]["Bm"], in_=pB)
                for ci in cis:
                    pBT = pd_("pBT")
                    mm2(pBT, lambda h, kt: t[ci]["To"][:, h, :],
                        lambda h, kt: t[ci]["MT"][:, h, :])
                    t[ci]["BT"] = blkg("BT")
                    nc.vector.tensor_copy(out=t[ci]["BT"], in_=pBT)
                for ci in cis:
                    pB2 = pd_("pB2")
                    mm2(pB2, lambda h, kt: t[ci]["Bm"][:, h, :],
                        lambda h, kt: t[ci]["BT"][:, h, :])
                    t[ci]["B2T"] = blkg("B2T")
                    nc.scalar.copy(out=t[ci]["B2T"], in_=pB2)
                # step: aT
                for ci in cis:
                    pA4 = pd_("pA4")
                    mm2(pA4, lambda h, kt: kT[:, h, kt, ls[ci]],
                        lambda h, kt: qT[:, h, kt, ls[ci]], n_k=2)
                    nc.vector.tensor_tensor(out=aT_all[:, ci % 4], in0=pA4,
                                            in1=bc_mid(masks[:, 3, :], 2),
                                            op=ALU.mult)
                # steps: X = [beta*v | beta*k], 3-stage apply
                for ci in cis:
                    X = X_v[ci % 4]
                    for h in range(2):
                        nc.vector.tensor_scalar_mul(
                            X[:, h, 0:256].rearrange("p (a b) -> p a b", a=2),
                            vlc[:, h, :, ci, :], beta_lp[:, ci, h:h+1])
                        nc.vector.tensor_scalar_mul(
                            X[:, h, 256:512].rearrange(
                                "p (a b) -> p a b", a=2),
                            klc[:, h, :, ci, :], beta_lp[:, ci, h:h+1])
                    t[ci]["X"] = X
                for ci in cis:
                    x1t = x1_v[ci % 3]
                    for h in range(2):
                        px = psD.tile([128, 512], F32, tag="pX5", bufs=3,
                                      name="pX1")
                        nc.tensor.matmul(px, t[ci]["MT"][:, h, :],
                                         t[ci]["X"][:, h, :],
                                         start=True, stop=True)
                        if h == 0:
                            nc.scalar.copy(out=x1t[:, h, :], in_=px)
                        else:
                            nc.vector.tensor_copy(out=x1t[:, h, :], in_=px)
                    t[ci]["x1t"] = x1t
                for ci in cis:
                    y1t = y1_v[ci % 3]
                    for h in range(2):
                        px = psD.tile([128, 512], F32, tag="pX5", bufs=3,
                                      name="pX2")
                        nc.tensor.matmul(px, t[ci]["B2T"][:, h, :],
                                         t[ci]["x1t"][:, h, :],
                                         start=True, stop=False)
                        nc.tensor.matmul(px, ident16, t[ci]["x1t"][:, h, :],
                                         start=False, stop=True)
                        if h == 0:
                            nc.vector.tensor_copy(out=y1t[:, h, :], in_=px)
                        else:
                            nc.scalar.copy(out=y1t[:, h, :], in_=px)
                    t[ci]["y1t"] = y1t
                for ci in cis:
                    wtmp = pdw.tile([128, 2, 256], F16, tag="wtmp", bufs=3,
                                    name="wtmp")
                    for h in range(2):
                        px = psD.tile([128, 512], F32, tag="pX5", bufs=3,
                                      name="pX3")
                        nc.tensor.matmul(px, t[ci]["BT"][:, h, :],
                                         t[ci]["y1t"][:, h, :],
                                         start=True, stop=False)
                        nc.tensor.matmul(px, ident16, t[ci]["y1t"][:, h, :],
                                         start=False, stop=True)
                        nc.vector.tensor_copy(out=u_all[:, ci % 4, h, :],
                                              in_=px[:, 0:256])
                        nc.scalar.copy(out=wtmp[:, h, :], in_=px[:, 256:512])
                    t[ci]["wtmp"] = wtmp
                for ci in cis:
                    for kt in range(2):
                        ptw = psD.tile([128, 2, 128], F16, tag="pdT", bufs=1,
                                       name="ptw")
                        for h in range(2):
                            nc.tensor.transpose(
                                ptw[:, h, :],
                                t[ci]["wtmp"][:, h, kt*128:(kt+1)*128],
                                ident16)
                        nc.scalar.mul(out=wT_all[:, ci % 4, kt], in_=ptw,
                                      mul=-1.0)

                ngrp = (NCH + GRP - 1) // GRP
                gi = g0 // GRP
                for fstep in range(32 * gi // ngrp, 32 * (gi + 1) // ngrp):
                    _emit_fusion(fstep)

                # --- serial S part (per chunk) ---------------------------
                for ci in cis:
                    pup = psD.tile([128, 2, d], F32, tag="pS", bufs=1,
                                   name="pup")
                    for h in range(2):
                        for kt in range(2):
                            nc.tensor.matmul(pup[:, h, :],
                                             wT_all[:, ci % 4, kt, h, :],
                                             S16[:, h, kt, :],
                                             start=(kt == 0), stop=False)
                        nc.tensor.matmul(pup[:, h, :], ident16,
                                         u_all[:, ci % 4, h, :],
                                         start=False, stop=True)
                    # pup now holds u - w^T S = upr directly (wT is negated)
                    uprt = pdw.tile([128, 2, d], F16, tag="uprt", bufs=2,
                                    name="uprt")
                    nc.scalar.copy(out=uprt, in_=pup)

                    po = psD.tile([128, 2, d], F32, tag="pS", bufs=1,
                                  name="po")
                    for h in range(2):
                        for kt in range(2):
                            nc.tensor.matmul(po[:, h, :],
                                             qT[:, h, kt, ls[ci]],
                                             S16[:, h, kt, :],
                                             start=(kt == 0), stop=False)
                        nc.tensor.matmul(po[:, h, :], aT_all[:, ci % 4, h, :],
                                         uprt[:, h, :],
                                         start=False, stop=True)
                    dsb = pdw.tile([128, 2, d], F16, tag="dsb", bufs=1,
                                   name="dsb")
                    nc.scalar.copy(out=dsb, in_=po)
                    nc.gpsimd.dma_start(
                        out=sc["dout_s"][ls[ci], :],
                        in_=dsb.rearrange("p h e -> p (h e)"))
                    nc.vector.tensor_reduce(
                        out=bn_sb[:, ci, 2:4], in_=dsb,
                        axis=mybir.AxisListType.X, op=ALU.add,
                        apply_absolute_value=True)

                    for h in range(2):
                        pdS = psD.tile([128, 2, d], F32, tag="pS", bufs=1,
                                       name=f"pdS{h}")
                        for kt in range(2):
                            nc.tensor.matmul(pdS[:, kt, :],
                                             klc[:, h, kt, ci, :],
                                             uprt[:, h, :],
                                             start=True, stop=True)
                        nc.vector.scalar_tensor_tensor(
                            out=S16[:, h], in0=pdS, scalar=1.0,
                            in1=S16[:, h], op0=ALU.mult, op1=ALU.add)

            # ---- bn features finalize + AllGather (three L-segments) -----
            SEGS = ((0, 0.34, 0, 8, "a"), (1, 0.41, 8, 12, "b"),
                    (2, 0.44, 12, 16, "c"))
            for _seg, wait_ms, lt0, lt1, sfx in SEGS:
                ctx_bn = tc.tile_wait_until(wait_ms)
                ctx_bn.__enter__()
                for lt in range(lt0, lt1):
                    cmt_b = pd1.tile([128, C], F16, tag="cmt_b", bufs=3,
                                     name="cmt_b")
                    nc.gpsimd.dma_start(
                        out=cmt_b, in_=sc["cm_out"][lt*128:(lt+1)*128, :])
                    nc.vector.tensor_reduce(
                        out=bn_sb[:, lt, 0:2],
                        in_=cmt_b.rearrange("p (h e) -> p h e", e=d),
                        axis=mybir.AxisListType.X, op=ALU.add,
                        apply_absolute_value=True)
                nc.vector.tensor_reduce(
                    out=bn_sb[:, lt0:lt1, 4:6],
                    in_=bnv4[:, lt0:lt1].rearrange(
                        "p lt (h t) -> p lt h t", t=2),
                    axis=mybir.AxisListType.X, op=ALU.add)
                nc.gpsimd.dma_start(
                    out=sc["bn_in_" + sfx][:].rearrange(
                        "(lt p) c -> p lt c", p=128),
                    in_=bn_sb[:, lt0:lt1, :])
                nc.gpsimd.collective_compute(
                    "AllGather", ALU.bypass, replica_groups=RG,
                    ins=[sc["bn_in_" + sfx][:]],
                    outs=[sc["bn_out_" + sfx][:]])
                ctx_bn.__exit__(None, None, None)

            bnT = [pd1.tile([8, L], F16, name=f"bnT{m}") for m in range(2)]
            for m in range(2):
                for _seg, _w, lt0, lt1, sfx in SEGS:
                    nseg = lt1 - lt0
                    bng = pd1.tile([128, 8, 8], F32, tag="bng", bufs=3,
                                   name=f"bng{m}{sfx}")
                    nc.sync.dma_start(
                        out=bng[:, 0:nseg, :],
                        in_=sc["bn_out_" + sfx][m].rearrange(
                            "(lt p) c -> p lt c", p=128))
                    for lts in range(nseg):
                        lt = lt0 + lts
                        ptb = psD.tile([128, 512], F32, tag="pX5", bufs=3,
                                       name="ptb")
                        ptbv = ptb[0:8, 0:128]
                        nc.tensor.transpose(ptbv, bng[:, lts, :], ident)
                        nc.scalar.mul(out=bnT[m][:, lt*128:(lt+1)*128],
                                      in_=ptbv, mul=1.0/d)
            # ---- fusion tail: bn part + gelu -----------------------------
            fb1_sb = pd1.tile([128, 8], F32)
            nc.sync.dma_start(out=fb1_sb,
                              in_=io["fb1"][:].rearrange("(m p) -> p m",
                                                         p=128))
            fw1b_sb = pd1.tile([8, 2, 1024], F16)
            nc.sync.dma_start(
                out=fw1b_sb,
                in_=io["fw1b"][:].rearrange("(m p) c -> p m c", p=8))
            for lw in range(NLW):
                for mt in range(8):
                    ph2 = psD.tile([128, 512], F32, tag="pX5", bufs=3, name="psh2")
                    for m in range(2):
                        nc.tensor.matmul(ph2,
                                         fw1b_sb[:, m, mt*128:(mt+1)*128],
                                         bnT[m][:, lw*512:(lw+1)*512],
                                         start=(m == 0), stop=(m == 1))
                    hdf = pdw.tile([128, 512], F16, tag="hdf", bufs=3,
                                   name="hdf")
                    nc.vector.scalar_tensor_tensor(
                        out=hdf, in0=ph2, scalar=1.0,
                        in1=hdn_pre[:, mt, lw*512:(lw+1)*512],
                        op0=ALU.mult, op1=ALU.add)
                    nc.scalar.activation(
                        out=hdn_pre[:, mt, lw*512:(lw+1)*512], in_=hdf,
                        func=AF.Gelu, bias=fb1_sb[:, mt:mt+1])

            fw2_sb = pd1.tile([128, 8, 12], F16)
            nc.sync.dma_start(
                out=fw2_sb,
                in_=io["fw2"][:].rearrange("(kt p) c -> p kt c", p=128))
            lg_sb = pd1.tile([128, NLT, 12], F32)
            for lt in range(NLT):
                pl = psD.tile([128, 512], F32, tag="pX5", bufs=3, name="psl")
                plv = pl[:, 0:12]
                for kt in range(8):
                    nc.tensor.matmul(plv, hdn_pre[:, kt, lt*128:(lt+1)*128],
                                     fw2_sb[:, kt, :],
                                     start=(kt == 0), stop=(kt == 7))
                nc.scalar.copy(out=lg_sb[:, lt, :], in_=plv)
            for m in range(2):
                nc.gpsimd.dma_start(
                    out=sc["lg_in"][m].rearrange("(lt p) c -> p lt c", p=128),
                    in_=lg_sb[:, :, m*6:(m+1)*6])
            nc.gpsimd.collective_compute(
                "ReduceScatter", ALU.add, replica_groups=RG,
                ins=[sc["lg_in"][:]], outs=[sc["lg_out"][:]])

        # =================== PHASE C ======================================
        with tc.tile_pool(name="pc1", bufs=1) as pc1, \
             tc.tile_pool(name="pc2", bufs=2) as pc2, \
             tc.tile_pool(name="pcs", bufs=4) as pcs, \
             tc.tile_pool(name="psC", bufs=1, space="PSUM") as psC:

            def psc(name, tag="pg"):
                return psC.tile([128, 512], F32, tag=tag, bufs=2, name=name)

            # ---- softmax gates -------------------------------------------
            b2_sb = pc1.tile([128, 6], F32)
            nc.sync.dma_start(out=b2_sb, in_=io["b2o"][:])
            lgo = pc1.tile([128, NLT, 2, 3], F32)
            nc.sync.dma_start(
                out=lgo,
                in_=sc["lg_out"][:].rearrange("(lt p) (h e) -> p lt h e",
                                              p=128, e=3))
            nc.vector.tensor_tensor(
                out=lgo, in0=lgo,
                in1=bass.AP(tensor=b2_sb.tensor, offset=b2_sb.offset,
                            ap=[b2_sb.ap[0], [0, NLT], [3, 2], [1, 3]]),
                op=ALU.add)
            rmax = pc1.tile([128, NLT, 2], F32)
            nc.vector.tensor_reduce(out=rmax, in_=lgo,
                                    axis=mybir.AxisListType.X, op=ALU.max)
            nc.vector.tensor_tensor(
                out=lgo, in0=lgo,
                in1=rmax[:, :, :, None].to_broadcast([128, NLT, 2, 3]),
                op=ALU.subtract)
            nc.scalar.activation(out=lgo, in_=lgo, func=AF.Exp)
            rsum = pc1.tile([128, NLT, 2], F32)
            nc.vector.tensor_reduce(out=rsum, in_=lgo,
                                    axis=mybir.AxisListType.X, op=ALU.add)
            nc.vector.reciprocal(out=rsum, in_=rsum)
            nc.vector.tensor_tensor(
                out=lgo, in0=lgo,
                in1=rsum[:, :, :, None].to_broadcast([128, NLT, 2, 3]),
                op=ALU.mult)

            # ---- gate mix + RMSNorm + Wo ---------------------------------
            wo_sb = pc1.tile([128, 4, D], F16)
            nc.sync.dma_start(
                out=wo_sb,
                in_=io["wo"][:].rearrange("(kt p) n -> p kt n", p=128))
            dout_sb = arena[:, 0:8192].rearrange("p (lt h e) -> p lt h e",
                                                 h=2, e=d)
            cm_sb = arena[:, 8192:16384].rearrange("p (lt c) -> p lt c",
                                                   c=C)
            nc.gpsimd.dma_start(
                out=cm_sb,
                in_=sc["cm_out"][:].rearrange("(lt p) c -> p lt c", p=128))
            nc.gpsimd.dma_start(
                out=dout_sb,
                in_=sc["dout_s"][:].rearrange("(lt p) (h e) -> p lt h e",
                                              p=128, e=d))
            for lt in range(NLT):
                dov = dout_sb[:, lt]
                o_t = pcs.tile([128, 2, d], F16, tag="o_t", name="o_t")
                ssq = pcs.tile([128, 2], F32, tag="ssq", name="ssq")
                scr = pcs.tile([128, d], F32, tag="scr", name="scr")
                for h in range(2):
                    nc.vector.tensor_scalar_mul(
                        o_t[:, h, :],
                        cm_sb[:, lt, h*256:(h+1)*256], lgo[:, lt, h, 0:1])
                    nc.vector.scalar_tensor_tensor(
                        out=o_t[:, h, :], in0=dov[:, h, :],
                        scalar=lgo[:, lt, h, 1:2], in1=o_t[:, h, :],
                        op0=ALU.mult, op1=ALU.add)
                    nc.vector.scalar_tensor_tensor(
                        out=o_t[:, h, :].rearrange("p (a b) -> p a b", a=2),
                        in0=vlc[:, h, :, lt, :],
                        scalar=lgo[:, lt, h, 2:3],
                        in1=o_t[:, h, :].rearrange("p (a b) -> p a b", a=2),
                        op0=ALU.mult, op1=ALU.add)
                    nc.scalar.activation(out=scr, in_=o_t[:, h, :],
                                         func=AF.Square,
                                         accum_out=ssq[:, h:h+1])
                nc.scalar.activation(out=ssq, in_=ssq, func=AF.Sqrt,
                                     scale=1.0/d, bias=eps5)
                nc.vector.reciprocal(out=ssq, in_=ssq)
                for h in range(2):
                    nc.vector.tensor_scalar_mul(o_t[:, h, :], o_t[:, h, :],
                                                ssq[:, h:h+1])
                pto = psC.tile([128, 4, 128], F16, tag="pto", bufs=2,
                               name="psto")
                for ct in range(4):
                    h, dt = ct // 2, ct % 2
                    nc.tensor.transpose(pto[:, ct, :],
                                        o_t[:, h, dt*128:(dt+1)*128],
                                        ident16)
                oT = pcs.tile([128, 4, 128], F16, tag="oT", name="oT")
                nc.vector.tensor_copy(out=oT, in_=pto)
                orow = pcs.tile([128, D], F32, tag="orow", name="orow")
                for nh in range(2):
                    pw = psC.tile([128, 512], F32, tag="pw", bufs=2, name="psw")
                    for ct in range(4):
                        nc.tensor.matmul(pw, oT[:, ct, :],
                                         wo_sb[:, ct, nh*512:(nh+1)*512],
                                         start=(ct == 0), stop=(ct == 3))
                    nc.scalar.copy(out=orow[:, nh*512:(nh+1)*512], in_=pw)
                nc.sync.dma_start(out=io["out_part"][lt*128:(lt+1)*128, :],
                                  in_=orow)


# ======================= host side =======================================

def _diag_tiles(w_own, taps, out_dtype):
    """w_own: (C, k) conv weights for this core's channels.
    Returns (4, k, 128, 128) diag tiles; tap dd uses column k-1-dd."""
    k = w_own.shape[1]
    out = np.zeros((4, k, 128, 128), dtype=out_dtype)
    for ct in range(4):
        for dd in range(k):
            np.fill_diagonal(out[ct, dd], w_own[ct*128:(ct+1)*128, k-1-dd])
    return out


def _host_inputs(inputs):
    hs = np.asarray(inputs["hidden_states"], np.float32)
    Wq = np.asarray(inputs["Wq"], np.float32)
    Wk = np.asarray(inputs["Wk"], np.float32)
    Wv = np.asarray(inputs["Wv"], np.float32)
    Wb = np.asarray(inputs["Wb"], np.float32)
    cq = np.asarray(inputs["conv_q_w"], np.float32)
    ck = np.asarray(inputs["conv_k_w"], np.float32)
    cv = np.asarray(inputs["conv_v_w"], np.float32)
    w3 = np.asarray(inputs["ms_w3"], np.float32)
    w15 = np.asarray(inputs["ms_w15"], np.float32)
    w31 = np.asarray(inputs["ms_w31"], np.float32)
    kmix = np.asarray(inputs["kernel_mix_w"], np.float32)
    cmix = np.asarray(inputs["channel_mixer_w"], np.float32)
    fw1 = np.asarray(inputs["fusion_w1"], np.float32)
    fb1 = np.asarray(inputs["fusion_b1"], np.float32)
    fw2 = np.asarray(inputs["fusion_w2"], np.float32)
    fb2 = np.asarray(inputs["fusion_b2"], np.float32)
    onw = np.asarray(inputs["o_norm_w"], np.float32)
    Wo = np.asarray(inputs["Wo"], np.float32)

    # combined kernel_mix -> channel_mixer matrix Q: (3D, D)
    Q = np.zeros((3 * D, D), np.float32)
    for h in range(H):
        Q[h*3*d:(h+1)*3*d] = kmix @ cmix[h*d:(h+1)*d]

    masks = np.zeros((5, 128, 128), np.float32)
    i_, j_ = np.mgrid[0:128, 0:128]
    blk = (i_ // 32) == (j_ // 32)
    masks[0] = -((i_ > j_) & blk).astype(np.float32)
    masks[1] = -((i_ > j_) & ~blk).astype(np.float32)
    masks[2] = -((j_ > i_) & blk).astype(np.float32)
    masks[3] = (j_ >= i_).astype(np.float32)
    masks[4] = np.eye(128, dtype=np.float32)

    Wo_s = Wo * np.tile(onw, H)[:, None]

    in_maps = []
    for c in range(8):
        b, r = divmod(c, 2)
        cs = slice(C*r, C*(r+1))
        qmix = np.concatenate(
            [Q[1024*s + C*r: 1024*s + C*r + C] for s in range(3)], 0)
        f8 = ml_dtypes.float8_e4m3
        mspair = np.zeros((4, 23, 2, 128, 128), np.float32)
        mssing = np.zeros((4, 3, 128, 128), np.float32)
        for ct in range(4):
            pi = 0
            for si, (w, ks) in enumerate(((w3, 3), (w15, 15), (w31, 31))):
                dg = _diag_tiles(w[cs], ks, np.float32)[ct] * 64.0
                for p_ in range((ks - 1) // 2):
                    mspair[ct, pi, 0] = dg[2*p_ + 1]
                    mspair[ct, pi, 1] = dg[2*p_]
                    pi += 1
                mssing[ct, si] = dg[ks - 1]
        cdiag = np.stack([_diag_tiles(w[cs], KQKV, np.float16)
                          for w in (cq, ck, cv)], 0)
        fw1b = np.zeros((16, 1024), np.float32)
        for m in range(2):
            for src in range(3):
                for h_ in range(2):
                    fw1b[m*8 + src*2 + h_] = \
                        fw1[D + src*4 + 2*m + h_, 1024*r:1024*(r+1)]
        fw2p = np.zeros((1024, 12), np.float32)
        b2o = np.zeros((6,), np.float32)
        for jm in range(2):
            for h_ in range(2):
                for br in range(3):
                    gcol = (2*jm + h_)*3 + br
                    fw2p[:, jm*6 + h_*3 + br] = fw2[1024*r:1024*(r+1), gcol]
        for h_ in range(2):
            for br in range(3):
                b2o[h_*3 + br] = fb2[(2*r + h_)*3 + br]
        m = {
            "hsT": np.ascontiguousarray(hs[b].T).astype(np.float16),
            "wq": np.ascontiguousarray(Wq[:, cs]).astype(np.float16),
            "wk": np.ascontiguousarray(Wk[:, cs]).astype(np.float16),
            "wv": np.ascontiguousarray(Wv[:, cs]).astype(np.float16),
            "wb": np.ascontiguousarray(Wb[:, 2*r:2*r+2]).astype(np.float16),
            "cdiag": cdiag,
            "mspair": mspair.astype(f8),
            "mssing": mssing.astype(f8),
            "qmix8": (qmix * 64.0).astype(f8),
            "fw1h": np.ascontiguousarray(
                fw1[:D, 1024*r:1024*(r+1)]).astype(np.float16),
            "fw1b": fw1b.astype(np.float16),
            "fb1": np.ascontiguousarray(fb1[1024*r:1024*(r+1)]),
            "fw2": fw2p.astype(np.float16),
            "b2o": np.tile(b2o, (128, 1)),
            "wo": np.ascontiguousarray(Wo_s[cs, :]).astype(np.float16),
            "masks": masks,
            "onesrow": np.ones((1, 128), np.float32),
            "onescol": np.ones((128, 1), np.float32),
            "ident16": np.eye(128, dtype=np.float16),
        }
        in_maps.append(m)
    return in_maps


_PROG = {}


def _get_program(debug=False):
    key = bool(debug)
    if key not in _PROG:
        _PROG[key] = build_program(debug=debug)
    return _PROG[key]


def run(inputs, debug=False, **kw):
    nc = _get_program(debug=debug)
    in_maps = _host_inputs(inputs)
    res = run_bass_kernel_spmd(nc, in_maps, list(range(8)), **kw)
    return res


def kernel(**inputs):
    res = run(inputs)
    out = np.zeros((B, L, D), np.float32)
    for b in range(B):
        out[b] = res.results[2*b]["out_part"] + res.results[2*b+1]["out_part"]
    return out


if __name__ == "__main__":
    nc = build_program()
    print("program built ok")


# revision 97
# speedup vs baseline: 1.0569x; 1.0149x over previous
"""DeltaNet-style block (nn_DeltaNet_31877247271438) on 8 trn2 NeuronCores.

Sharding: core c -> (batch b = c//2, pair-rank r = c%2).  Within a batch pair:
  - head-parallel: rank r owns heads {2r, 2r+1} (feature cols [512r, 512r+512))
  - cross-head mixes are K-split with pairwise collectives:
      * channel_mixer (folded with kernel_mix into one matrix Q): partial sums
        ReduceScatter'ed (fp16) so each core receives its own heads' ms_out
      * fusion-MLP hidden is column-split; logits partials ReduceScatter'ed
      * bn features AllGather'ed (tiny)
  - the final Wo matmul partials are summed on the host.

v2: inter-phase tensors stay in SBUF (no DRAM staging round-trips); l-major
forms of k/v come from DMA-xbar transposes; the delta rule's 16 chunk
inversions are batched (only the S recurrence is serial); beta is folded
into the mask multiply so the kb row form is never materialized; a manually
aliased SBUF arena lets phase-scoped tensors (hsT/xc, y, u/wT/aT, hdn_pre)
share the same bytes.
"""
import sys
sys.path.insert(0, '/opt/trn_rl_repo')

import numpy as np
import ml_dtypes

import concourse.bass as bass
import concourse.tile as tile
from concourse import bacc, mybir
from concourse.bass_utils import run_bass_kernel_spmd

F32 = mybir.dt.float32
F16 = mybir.dt.float16
F8 = mybir.dt.float8e4
DR = mybir.MatmulPerfMode.DoubleRow
FP8S = 64.0
AF = mybir.ActivationFunctionType
ALU = mybir.AluOpType

B, L, D, H = 4, 2048, 1024, 4
d = 256          # per-head dim
C = 512          # channels owned per core (2 heads)
NLT = 16         # l-tiles of 128
NLW = 4          # l-windows of 512
NCH = 16         # delta chunks of 128
KQKV = 4         # qkv conv taps
MSK = (3, 15, 31)
NTAPS = sum(MSK)  # 49
PADV = 32
RG = [[0, 1], [2, 3], [4, 5], [6, 7]]

ARENA = 36864    # f16 elems per partition in the aliased arena


def bc_mid(ap2, n):
    """[P, F] AP -> [P, n, F] with a 0-stride middle dim (free-dim bcast)."""
    assert len(ap2.ap) == 2
    return bass.AP(tensor=ap2.tensor, offset=ap2.offset,
                   ap=[ap2.ap[0], [0, n], ap2.ap[1]])


def build_program(debug=False):
    nc = bacc.Bacc("TRN2", target_bir_lowering=False, debug=False,
                   num_devices=8)

    io = {}
    io["hsT"] = nc.declare_dram_parameter("hsT", [D, L], F16, False)
    io["wq"] = nc.declare_dram_parameter("wq", [D, C], F16, False)
    io["wk"] = nc.declare_dram_parameter("wk", [D, C], F16, False)
    io["wv"] = nc.declare_dram_parameter("wv", [D, C], F16, False)
    io["wb"] = nc.declare_dram_parameter("wb", [D, 2], F16, False)
    io["cdiag"] = nc.declare_dram_parameter("cdiag", [3, 4, KQKV, 128, 128],
                                            F16, False)
    io["mspair"] = nc.declare_dram_parameter("mspair", [4, 23, 2, 128, 128],
                                             F8, False)
    io["mssing"] = nc.declare_dram_parameter("mssing", [4, 3, 128, 128],
                                             F8, False)
    io["qmix8"] = nc.declare_dram_parameter("qmix8", [12 * 128, D], F8, False)
    io["fw1h"] = nc.declare_dram_parameter("fw1h", [D, 1024], F16, False)
    io["fw1b"] = nc.declare_dram_parameter("fw1b", [16, 1024], F16, False)
    io["fb1"] = nc.declare_dram_parameter("fb1", [1024], F32, False)
    io["fw2"] = nc.declare_dram_parameter("fw2", [1024, 12], F16, False)
    io["b2o"] = nc.declare_dram_parameter("b2o", [128, 6], F32, False)
    io["wo"] = nc.declare_dram_parameter("wo", [C, D], F16, False)
    io["masks"] = nc.declare_dram_parameter("masks", [5, 128, 128], F32, False)
    io["onesrow"] = nc.declare_dram_parameter("onesrow", [1, 128], F32, False)
    io["onescol"] = nc.declare_dram_parameter("onescol", [128, 1], F32, False)
    io["ident16"] = nc.declare_dram_parameter("ident16", [128, 128], F16, False)
    io["out_part"] = nc.declare_dram_parameter("out_part", [L, D], F32, True)

    sc = {}
    sc["dout_s"] = nc.dram_tensor("dout_s", [L, C], F16)
    sc["cm_in"] = nc.dram_tensor("cm_in", [2, L, C], F16)
    sc["cm_out"] = nc.dram_tensor("cm_out", [L, C], F16)
    sc["bn_in_a"] = nc.dram_tensor("bn_in_a", [1024, 8], F32)
    sc["bn_out_a"] = nc.dram_tensor("bn_out_a", [2, 1024, 8], F32)
    sc["bn_in_b"] = nc.dram_tensor("bn_in_b", [512, 8], F32)
    sc["bn_out_b"] = nc.dram_tensor("bn_out_b", [2, 512, 8], F32)
    sc["bn_in_c"] = nc.dram_tensor("bn_in_c", [512, 8], F32)
    sc["bn_out_c"] = nc.dram_tensor("bn_out_c", [2, 512, 8], F32)
    sc["lg_in"] = nc.dram_tensor("lg_in", [2, L, 6], F32)
    sc["lg_out"] = nc.dram_tensor("lg_out", [L, 6], F32)

    with tile.TileContext(nc) as tc:
        _body(nc, tc, io, sc)
    nc.compile()
    return nc


def _body(nc, tc, io, sc):
    from contextlib import ExitStack
    ctx = ExitStack()
    with ctx:
        consts = ctx.enter_context(tc.tile_pool(name="consts", bufs=1))
        outer = ctx.enter_context(tc.tile_pool(name="outer", bufs=1))

        masks = consts.tile([128, 5, 128], F32)
        nc.sync.dma_start(out=masks,
                          in_=io["masks"][:].rearrange("m p f -> p m f"))
        ident = masks[:, 4, :]
        ident16 = consts.tile([128, 128], F16)
        nc.sync.dma_start(out=ident16, in_=io["ident16"][:])
        onescol16 = consts.tile([128, 1], F16)
        nc.vector.memset(onescol16, 1.0)
        onesrow16 = consts.tile([1, 128], F16)
        nc.vector.memset(onesrow16, 1.0)
        eps6 = consts.tile([128, 1], F32)
        nc.vector.memset(eps6, 1e-6)
        eps5 = consts.tile([128, 1], F32)
        nc.vector.memset(eps5, 1e-5)

        beta_lp = outer.tile([128, NLT, 2], F32)
        S16 = outer.tile([128, 2, 2, d], F16)
        nc.vector.memset(S16, 0.0)
        qT = outer.tile([128, 2, 2, L], F16)       # (h, dtile, l) row form
        kT = outer.tile([128, 2, 2, L], F16)
        klc = outer.tile([128, 2, 2, NLT, 128], F16)  # (h, dtile, lt, dk)
        vlc = outer.tile([128, 2, 2, NLT, 128], F16)
        bn_sb = outer.tile([128, NLT, 8], F32)
        bnv4 = outer.tile([128, NLT, 4], F32)
        arena = outer.tile([128, ARENA], F16)

        # arena regions (manually aliased, lifetimes disjoint)
        hsT = arena[:, 0:16384].rearrange("p (kt l) -> p kt l", l=L)
        xc = arena[:, 16384:24592].rearrange("p (ct l) -> p ct l", l=L + 4)
        vt_bf = arena[:, 24592:32912].rearrange("p (ct l) -> p ct l",
                                                l=PADV + L)
        y_bf = arena[:, 0:24576].rearrange("p (j l) -> p j l", l=L)
        u_all = arena[:, 0:2048].rearrange("p (n h e) -> p n h e", h=2, e=d)
        wT_all = arena[:, 2048:4096].rearrange("p (n kt h e) -> p n kt h e",
                                               kt=2, h=2, e=128)
        aT_all = arena[:, 4096:5120].rearrange("p (n h e) -> p n h e",
                                               h=2, e=128)
        hdn_pre = arena[:, 20480:36864].rearrange("p (mt l) -> p mt l", l=L)

        hsT_r = io["hsT"][:].rearrange("(kt p) l -> p kt l", p=128)

        # =================== PHASE A ======================================
        with tc.tile_pool(name="pa1", bufs=1) as pa1, \
             tc.tile_pool(name="pa2", bufs=2) as pa2, \
             tc.tile_pool(name="pas", bufs=3) as pas, \
             tc.tile_pool(name="psA", bufs=1, space="PSUM") as psA:
            nc.sync.dma_start(out=hsT, in_=hsT_r)

            # ---- beta (l-partition form) ---------------------------------
            wb_sb = pa1.tile([128, 8, 2], F16)
            nc.sync.dma_start(
                out=wb_sb, in_=io["wb"][:].rearrange("(kt p) c -> p kt c",
                                                     p=128))
            for lt in range(NLT):
                pb = psA.tile([128, 2], F32, tag="pb", bufs=1, name="psb")
                for kt in range(8):
                    nc.tensor.matmul(pb, hsT[:, kt, lt*128:(lt+1)*128],
                                     wb_sb[:, kt, :],
                                     start=(kt == 0), stop=(kt == 7))
                nc.scalar.activation(out=beta_lp[:, lt, :], in_=pb,
                                     func=AF.Sigmoid)

            # ---- q, k, v: proj -> conv -> silu -> (norm) -----------------
            for tnm, ti in (("v", 2), ("k", 1), ("q", 0)):
                PAD = 4
                w_sb = pa2.tile([128, 8, C], F16, tag="w_sb", bufs=2,
                                name=f"w_{tnm}")
                nc.sync.dma_start(
                    out=w_sb,
                    in_=io["w" + tnm][:].rearrange("(kt p) c -> p kt c",
                                                   p=128))
                cdg = pa2.tile([128, 4, KQKV, 128], F16, tag="cdg", bufs=2,
                               name=f"cdg_{tnm}")
                nc.sync.dma_start(
                    out=cdg,
                    in_=io["cdiag"][ti].rearrange("ct tap p f -> p ct tap f"))
                if tnm != "v":
                    nc.vector.memset(xc[:, :, 0:4], 0.0)
                else:
                    nc.vector.memset(vt_bf[:, :, 0:PADV], 0.0)

                for ct in range(4):
                    xp = pa2.tile([128, 4 + L], F16, tag="xp", bufs=2,
                                  name=f"xp_{tnm}{ct}")
                    nc.vector.memset(xp[:, 0:4], 0.0)
                    for lw in range(NLW):
                        pp = psA.tile([128, 512], F32, tag="pp", bufs=2,
                                      name="psp")
                        for kt in range(8):
                            nc.tensor.matmul(
                                pp, w_sb[:, kt, ct*128:(ct+1)*128],
                                hsT[:, kt, lw*512:(lw+1)*512],
                                start=(kt == 0), stop=(kt == 7))
                        nc.scalar.copy(out=xp[:, 4+lw*512:4+(lw+1)*512],
                                       in_=pp)
                    for lw in range(NLW):
                        pc = psA.tile([128, 512], F32, tag="pc", bufs=2,
                                      name="psc")
                        for dd in range(KQKV):
                            off = 4 + lw*512 - dd
                            nc.tensor.matmul(
                                pc, cdg[:, ct, dd, :], xp[:, off:off+512],
                                start=(dd == 0), stop=(dd == KQKV-1))
                        if tnm == "v":
                            nc.scalar.activation(
                                out=vt_bf[:, ct, PADV+lw*512:PADV+(lw+1)*512],
                                in_=pc, func=AF.Silu)
                        else:
                            nc.scalar.activation(
                                out=xc[:, ct, PAD+lw*512:PAD+(lw+1)*512],
                                in_=pc, func=AF.Silu)
                if tnm == "v":
                    for ct in range(4):
                        nc.sync.dma_start_transpose(
                            out=vlc[:, ct // 2, ct % 2],
                            in_=vt_bf[:, ct, PADV:PADV+L])
                    continue

                # ---- l2norm -> qT / kT row forms -------------------------
                dst = qT if tnm == "q" else kT
                for h in range(2):
                    for lw in range(NLW):
                        lsl = slice(PAD+lw*512, PAD+(lw+1)*512)
                        osl = slice(lw*512, (lw+1)*512)
                        sqs = pas.tile([128, 2, 512], F16, tag="sq", bufs=3,
                                       name="sq")
                        for i, ct in enumerate((2*h, 2*h+1)):
                            nc.vector.tensor_tensor(
                                out=sqs[:, i, :], in0=xc[:, ct, lsl],
                                in1=xc[:, ct, lsl], op=ALU.mult)
                        ssr = pas.tile([1, 2, 512], F32, tag="ssr", bufs=2,
                                       name="ssr")
                        nc.gpsimd.tensor_reduce(
                            out=ssr, in_=sqs, axis=mybir.AxisListType.C,
                            op=ALU.add)
                        sr = pas.tile([1, 512], F32, tag="sr", bufs=2,
                                      name="sr")
                        nc.vector.tensor_tensor(out=sr, in0=ssr[:, 0, :],
                                                in1=ssr[:, 1, :], op=ALU.add)
                        nc.scalar.activation(out=sr, in_=sr, func=AF.Sqrt,
                                             bias=eps6[0:1, :])
                        srt = pas.tile([1, 512], F16, tag="srt", bufs=2,
                                       name="srt")
                        with nc.allow_low_precision("l2norm scale fp16"):
                            nc.vector.reciprocal(out=srt, in_=sr)
                        pbc = psA.tile([128, 512], F32, tag="pn2", bufs=2,
                                       name="psbc")
                        nc.tensor.matmul(pbc, onesrow16, srt,
                                         start=True, stop=True)
                        for dt in range(2):
                            ct = 2*h + dt
                            nc.vector.tensor_tensor(
                                out=dst[:, h, dt, osl], in0=xc[:, ct, lsl],
                                in1=pbc, op=ALU.mult)
                if tnm == "k":
                    for h in range(2):
                        for dt in range(2):
                            nc.sync.dma_start_transpose(
                                out=klc[:, h, dt], in_=kT[:, h, dt, :])

            # bn(v): |v| partial sums (finalized later)
            for lt in range(NLT):
                nc.vector.tensor_reduce(
                    out=bnv4[:, lt, :], in_=vlc[:, :, :, lt, :],
                    axis=mybir.AxisListType.X, op=ALU.add,
                    apply_absolute_value=True)

        # =================== PHASE B: multiscale conv + qmix ==============
        with tc.tile_pool(name="pb1", bufs=1) as pb1, \
             tc.tile_pool(name="pbm", bufs=2) as pbm, \
             tc.tile_pool(name="pbs", bufs=3) as pbs, \
             tc.tile_pool(name="psB", bufs=1, space="PSUM") as psB:
            y8 = arena[:, 0:12288].bitcast(F8).rearrange(
                "p (j l) -> p j l", l=L)
            vt8 = arena[:, 12288:16448].bitcast(F8).rearrange(
                "p (ct l) -> p ct l", l=PADV + L)
            with nc.allow_low_precision("fp8 conv input"):
                for ct in range(4):
                    nc.vector.tensor_copy(out=vt8[:, ct, :],
                                          in_=vt_bf[:, ct, :])

            def pair_ap(off, n, ct):
                a = vt8[:, ct, off:off+n]
                return bass.AP(tensor=a.tensor, offset=a.offset,
                               ap=[a.ap[0], [1, 2], a.ap[1]])

            for ct in range(4):
                msd8 = pbm.tile([128, 23, 2, 128], F8, tag=f"msd{ct % 2}",
                                bufs=1, name=f"msd8_{ct}")
                nc.gpsimd.dma_start(
                    out=msd8,
                    in_=io["mspair"][ct].rearrange("j t p f -> p j t f"))
                mss8 = pbm.tile([128, 3, 128], F8, tag=f"mss{ct % 2}",
                                bufs=1, name=f"mss8_{ct}")
                nc.gpsimd.dma_start(
                    out=mss8,
                    in_=io["mssing"][ct].rearrange("j p f -> p j f"))
                for lw in range(NLW):
                    base_pi = 0
                    for si, ks in enumerate(MSK):
                        npair = (ks - 1) // 2
                        py = psB.tile([128, 512], F32, tag="py", bufs=4,
                                      name="psy")
                        for p_ in range(npair):
                            off = PADV + lw*512 - (2*p_ + 1)
                            nc.tensor.matmul(
                                py, msd8[:, base_pi + p_, :, :],
                                pair_ap(off, 512, ct),
                                start=(p_ == 0), stop=False, perf_mode=DR)
                        off = PADV + lw*512 - (ks - 1)
                        nc.tensor.matmul(py, mss8[:, si, :],
                                         vt8[:, ct, off:off+512],
                                         start=False, stop=True)
                        nc.scalar.mul(
                            out=y8[:, si*4+ct, lw*512:(lw+1)*512],
                            in_=py, mul=1.0/FP8S)
                        base_pi += npair

            qmix8_sb = pb1.tile([128, 6, 2, D], F8)
            nc.gpsimd.dma_start(
                out=qmix8_sb,
                in_=io["qmix8"][:].rearrange("(pp j p) o -> p pp j o",
                                             j=2, p=128))
            cms_v = [arena[:, 33040+i*1024:33040+(i+1)*1024].rearrange(
                "p (o c) -> p o c", o=2) for i in range(3)]
            for lt in range(NLT):
                cms = cms_v[lt % 3]
                for oh in range(2):
                    pq = psB.tile([128, 512], F32, tag="pq", bufs=4,
                                  name="psq")
                    for p_ in range(6):
                        nc.tensor.matmul(
                            pq, y8[:, 2*p_:2*p_+2, lt*128:(lt+1)*128],
                            qmix8_sb[:, p_, :, oh*512:(oh+1)*512],
                            start=(p_ == 0), stop=(p_ == 5), perf_mode=DR)
                    if oh == 0:
                        nc.vector.tensor_scalar_mul(cms[:, oh, :], pq,
                                                    1.0/FP8S)
                    else:
                        nc.scalar.mul(out=cms[:, oh, :], in_=pq,
                                      mul=1.0/FP8S)
                nc.gpsimd.dma_start(
                    out=sc["cm_in"][:, lt*128:(lt+1)*128, :].rearrange(
                        "o l c -> l o c"),
                    in_=cms)
            nc.gpsimd.collective_compute(
                "ReduceScatter", ALU.add, replica_groups=RG,
                ins=[sc["cm_in"][:]], outs=[sc["cm_out"][:]])

        # =================== PHASE B3: delta rule =========================
        with tc.tile_pool(name="pd1", bufs=1) as pd1, \
             tc.tile_pool(name="pdc", bufs=1) as pdc, \
             tc.tile_pool(name="pdw", bufs=1) as pdw, \
             tc.tile_pool(name="psD", bufs=1, space="PSUM") as psD:

            def blk(name, bufs=2):
                return pdc.tile([128, 2, 128], F16, tag=name, name=name,
                                bufs=bufs)

            def pd_(name):
                return psD.tile([128, 2, 128], F32, tag="pd", bufs=3,
                                name=name)

            def mm2(pt, lhs_fn, rhs_fn, n_k=1):
                for h in range(2):
                    for kt in range(n_k):
                        nc.tensor.matmul(pt[:, h, :], lhs_fn(h, kt),
                                         rhs_fn(h, kt),
                                         start=(kt == 0), stop=(kt == n_k-1))

            GRP = 3
            specs = [("T2", "TdT", "Td"), ("T2T", "Td", "TdT"),
                     ("T4", "T2T", "T2"), ("T4T", "T2", "T2T"),
                     ("T8", "T4T", "T4"), ("T8T", "T4", "T4T"),
                     ("T16", "T8T", "T8")]

            def blkg(name):
                return pdc.tile([128, 2, 128], F16, tag=name, name=name,
                                bufs=3)

            fw1h_r = io["fw1h"][:].rearrange("(kt p) m -> p kt m", p=128)
            _fus_state = {"hst": None}

            def _emit_fusion(step):
                lw, mt = step // 8, step % 8
                if mt == 0:
                    hst = pd1.tile([128, 8, 512], F16, tag="hst", bufs=1,
                                   name="hst")
                    nc.sync.dma_start(out=hst,
                                      in_=hsT_r[:, :, lw*512:(lw+1)*512])
                    _fus_state["hst"] = hst
                hst = _fus_state["hst"]
                fwt = pd1.tile([128, 8, 128], F16, tag="fwt", bufs=4,
                               name=f"fwt{mt}")
                nc.sync.dma_start(out=fwt,
                                  in_=fw1h_r[:, :, mt*128:(mt+1)*128])
                ph = psD.tile([128, 512], F32, tag="pX5", bufs=3,
                              name="psh")
                for kt in range(8):
                    nc.tensor.matmul(ph, fwt[:, kt, :], hst[:, kt, :],
                                     start=(kt == 0), stop=(kt == 7))
                if mt % 2 == 0:
                    nc.scalar.copy(
                        out=hdn_pre[:, mt, lw*512:(lw+1)*512], in_=ph)
                else:
                    nc.vector.tensor_copy(
                        out=hdn_pre[:, mt, lw*512:(lw+1)*512], in_=ph)

            X_v = [arena[:, 5120+i*1024:5120+(i+1)*1024].rearrange(
                "p (h e) -> p h e", h=2) for i in range(4)]
            x1_v = [arena[:, 9216+i*1024:9216+(i+1)*1024].rearrange(
                "p (h e) -> p h e", h=2) for i in range(3)]
            y1_v = [arena[:, 12288+i*1024:12288+(i+1)*1024].rearrange(
                "p (h e) -> p h e", h=2) for i in range(3)]
            for g0 in range(0, NCH, GRP):
                cis = list(range(g0, min(g0 + GRP, NCH)))
                ls = {ci: slice(ci*128, (ci+1)*128) for ci in cis}
                t = {ci: {} for ci in cis}

                # step: G = K K^T, mask+beta -> Td, To
                for ci in cis:
                    pG = pd_("pG")
                    mm2(pG, lambda h, kt: kT[:, h, kt, ls[ci]],
                        lambda h, kt: kT[:, h, kt, ls[ci]], n_k=2)
                    t[ci]["Td"], t[ci]["To"] = blkg("Td"), blkg("To")
                    for h in range(2):
                        nc.vector.scalar_tensor_tensor(
                            out=t[ci]["Td"][:, h, :], in0=pG[:, h, :],
                            scalar=beta_lp[:, ci, h:h+1], in1=masks[:, 0, :],
                            op0=ALU.mult, op1=ALU.mult)
                        nc.vector.scalar_tensor_tensor(
                            out=t[ci]["To"][:, h, :], in0=pG[:, h, :],
                            scalar=beta_lp[:, ci, h:h+1], in1=masks[:, 1, :],
                            op0=ALU.mult, op1=ALU.mult)
                # step: TdT = transpose(Td)
                for ci in cis:
                    pT = psD.tile([128, 2, 128], F16, tag="pdT", bufs=1,
                                  name="pTdT")
                    for h in range(2):
                        nc.tensor.transpose(pT[:, h, :], t[ci]["Td"][:, h, :],
                                            ident16)
                    t[ci]["TdT"] = blkg("TdT")
                    nc.scalar.copy(out=t[ci]["TdT"], in_=pT)
                # steps: squaring chain
                for si_, (nm, ln, rn) in enumerate(specs):
                    for ci in cis:
                        pq2 = pd_("pq2")
                        mm2(pq2, lambda h, kt, a=t[ci][ln]: a[:, h, :],
                            lambda h, kt, b_=t[ci][rn]: b_[:, h, :])
                        t[ci][nm] = blkg(nm)
                        if (si_ + ci) % 2 == 0:
                            nc.scalar.copy(out=t[ci][nm], in_=pq2)
                        else:
                            nc.vector.tensor_copy(out=t[ci][nm], in_=pq2)
                # steps: MT product chain -> DT
                for ci in cis:
                    MT = pdc.tile([128, 2, 128], F16, tag="MT", name="MT",
                                  bufs=2 * GRP)
                    nc.vector.tensor_tensor(out=MT, in0=t[ci]["TdT"],
                                            in1=bc_mid(ident16, 2),
                                            op=ALU.add)
                    t[ci]["MT"] = MT
                for nm in ("T2", "T4", "T8", "T16"):
                    for ci in cis:
                        pm = pd_("pm")
                        for h in range(2):
                            nc.tensor.matmul(pm[:, h, :], t[ci][nm][:, h, :],
                                             t[ci]["MT"][:, h, :],
                                             start=True, stop=False)
                            nc.tensor.matmul(pm[:, h, :], ident16,
                                             t[ci]["MT"][:, h, :],
                                             start=False, stop=True)
                        MTn = pdc.tile([128, 2, 128], F16, tag="MT",
                                       name="MT", bufs=2 * GRP)
                        if ci % 2 == 0:
                            nc.scalar.copy(out=MTn, in_=pm)
                        else:
                            nc.vector.tensor_copy(out=MTn, in_=pm)
                        t[ci]["MT"] = MTn
                # steps: B, BT, B2T
                for ci in cis:
                    pB = pd_("pB")
                    mm2(pB, lambda h, kt: t[ci]["MT"][:, h, :],
                        lambda h, kt: t[ci]["To"][:, h, :])
                    t[ci]["Bm"] = blkg("Bm")
                    nc.scalar.copy(out=t[ci]["Bm"], in_=pB)
                for ci in cis:
                    pBT = pd_("pBT")
                    mm2(pBT, lambda h, kt: t[ci]["To"][:, h, :],
                        lambda h, kt: t[ci]["MT"][:, h, :])
                    t[ci]["BT"] = blkg("BT")
                    nc.vector.tensor_copy(out=t[ci]["BT"], in_=pBT)
                for ci in cis:
                    pB2 = pd_("pB2")
                    mm2(pB2, lambda h, kt: t[ci]["Bm"][:, h, :],
                        lambda h, kt: t[ci]["BT"][:, h, :])
                    t[ci]["B2T"] = blkg("B2T")
                    nc.scalar.copy(out=t[ci]["B2T"], in_=pB2)
                # step: aT
                for ci in cis:
                    pA4 = pd_("pA4")
                    mm2(pA4, lambda h, kt: kT[:, h, kt, ls[ci]],
                        lambda h, kt: qT[:, h, kt, ls[ci]], n_k=2)
                    nc.vector.tensor_tensor(out=aT_all[:, ci % 4], in0=pA4,
                                            in1=bc_mid(masks[:, 3, :], 2),
                                            op=ALU.mult)
                # steps: X = [beta*v | beta*k], 3-stage apply
                for ci in cis:
                    X = X_v[ci % 4]
                    for h in range(2):
                        nc.vector.tensor_scalar_mul(
                            X[:, h, 0:256].rearrange("p (a b) -> p a b", a=2),
                            vlc[:, h, :, ci, :], beta_lp[:, ci, h:h+1])
                        nc.vector.tensor_scalar_mul(
                            X[:, h, 256:512].rearrange(
                                "p (a b) -> p a b", a=2),
                            klc[:, h, :, ci, :], beta_lp[:, ci, h:h+1])
                    t[ci]["X"] = X
                for ci in cis:
                    x1t = x1_v[ci % 3]
                    for h in range(2):
                        px = psD.tile([128, 512], F32, tag="pX5", bufs=3,
                                      name="pX1")
                        nc.tensor.matmul(px, t[ci]["MT"][:, h, :],
                                         t[ci]["X"][:, h, :],
                                         start=True, stop=True)
                        if h == 0:
                            nc.scalar.copy(out=x1t[:, h, :], in_=px)
                        else:
                            nc.vector.tensor_copy(out=x1t[:, h, :], in_=px)
                    t[ci]["x1t"] = x1t
                for ci in cis:
                    y1t = y1_v[ci % 3]
                    for h in range(2):
                        px = psD.tile([128, 512], F32, tag="pX5", bufs=3,
                                      name="pX2")
                        nc.tensor.matmul(px, t[ci]["B2T"][:, h, :],
                                         t[ci]["x1t"][:, h, :],
                                         start=True, stop=False)
                        nc.tensor.matmul(px, ident16, t[ci]["x1t"][:, h, :],
                                         start=False, stop=True)
                        if h == 0:
                            nc.vector.tensor_copy(out=y1t[:, h, :], in_=px)
                        else:
                            nc.scalar.copy(out=y1t[:, h, :], in_=px)
                    t[ci]["y1t"] = y1t
                for ci in cis:
                    wtmp = pdw.tile([128, 2, 256], F16, tag="wtmp", bufs=3,
                                    name="wtmp")
                    for h in range(2):
                        px = psD.tile([128, 512], F32, tag="pX5", bufs=3,
                                      name="pX3")
                        nc.tensor.matmul(px, t[ci]["BT"][:, h, :],
                                         t[ci]["y1t"][:, h, :],
                                         start=True, stop=False)
                        nc.tensor.matmul(px, ident16, t[ci]["y1t"][:, h, :],
                                         start=False, stop=True)
                        nc.vector.tensor_copy(out=u_all[:, ci % 4, h, :],
                                              in_=px[:, 0:256])
                        nc.scalar.copy(out=wtmp[:, h, :], in_=px[:, 256:512])
                    t[ci]["wtmp"] = wtmp
                for ci in cis:
                    for kt in range(2):
                        ptw = psD.tile([128, 2, 128], F16, tag="pdT", bufs=1,
                                       name="ptw")
                        for h in range(2):
                            nc.tensor.transpose(
                                ptw[:, h, :],
                                t[ci]["wtmp"][:, h, kt*128:(kt+1)*128],
                                ident16)
                        nc.scalar.mul(out=wT_all[:, ci % 4, kt], in_=ptw,
                                      mul=-1.0)

                ngrp = (NCH + GRP - 1) // GRP
                gi = g0 // GRP
                for fstep in range(32 * gi // ngrp, 32 * (gi + 1) // ngrp):
                    _emit_fusion(fstep)

                # --- serial S part (per chunk) ---------------------------
                for ci in cis:
                    pup = psD.tile([128, 2, d], F32, tag="pS", bufs=1,
                                   name="pup")
                    for h in range(2):
                        for kt in range(2):
                            nc.tensor.matmul(pup[:, h, :],
                                             wT_all[:, ci % 4, kt, h, :],
                                             S16[:, h, kt, :],
                                             start=(kt == 0), stop=False)
                        nc.tensor.matmul(pup[:, h, :], ident16,
                                         u_all[:, ci % 4, h, :],
                                         start=False, stop=True)
                    # pup now holds u - w^T S = upr directly (wT is negated)
                    uprt = pdw.tile([128, 2, d], F16, tag="uprt", bufs=2,
                                    name="uprt")
                    nc.scalar.copy(out=uprt, in_=pup)

                    po = psD.tile([128, 2, d], F32, tag="pS", bufs=1,
                                  name="po")
                    for h in range(2):
                        for kt in range(2):
                            nc.tensor.matmul(po[:, h, :],
                                             qT[:, h, kt, ls[ci]],
                                             S16[:, h, kt, :],
                                             start=(kt == 0), stop=False)
                        nc.tensor.matmul(po[:, h, :], aT_all[:, ci % 4, h, :],
                                         uprt[:, h, :],
                                         start=False, stop=True)
                    dsb = pdw.tile([128, 2, d], F16, tag="dsb", bufs=1,
                                   name="dsb")
                    nc.scalar.copy(out=dsb, in_=po)
                    nc.gpsimd.dma_start(
                        out=sc["dout_s"][ls[ci], :],
                        in_=dsb.rearrange("p h e -> p (h e)"))
                    nc.vector.tensor_reduce(
                        out=bn_sb[:, ci, 2:4], in_=dsb,
                        axis=mybir.AxisListType.X, op=ALU.add,
                        apply_absolute_value=True)

                    for h in range(2):
                        pdS = psD.tile([128, 2, d], F32, tag="pS", bufs=1,
                                       name=f"pdS{h}")
                        for kt in range(2):
                            nc.tensor.matmul(pdS[:, kt, :],
                                             klc[:, h, kt, ci, :],
                                             uprt[:, h, :],
                                             start=True, stop=True)
                        nc.vector.scalar_tensor_tensor(
                            out=S16[:, h], in0=pdS, scalar=1.0,
                            in1=S16[:, h], op0=ALU.mult, op1=ALU.add)

            # ---- bn features finalize + AllGather (three L-segments) -----
            SEGS = ((0, 0.34, 0, 8, "a"), (1, 0.41, 8, 12, "b"),
                    (2, 0.44, 12, 16, "c"))
            for _seg, wait_ms, lt0, lt1, sfx in SEGS:
                ctx_bn = tc.tile_wait_until(wait_ms)
                ctx_bn.__enter__()
                for lt in range(lt0, lt1):
                    cmt_b = pd1.tile([128, C], F16, tag="cmt_b", bufs=3,
                                     name="cmt_b")
                    nc.gpsimd.dma_start(
                        out=cmt_b, in_=sc["cm_out"][lt*128:(lt+1)*128, :])
                    nc.vector.tensor_reduce(
                        out=bn_sb[:, lt, 0:2],
                        in_=cmt_b.rearrange("p (h e) -> p h e", e=d),
                        axis=mybir.AxisListType.X, op=ALU.add,
                        apply_absolute_value=True)
                nc.vector.tensor_reduce(
                    out=bn_sb[:, lt0:lt1, 4:6],
                    in_=bnv4[:, lt0:lt1].rearrange(
                        "p lt (h t) -> p lt h t", t=2),
                    axis=mybir.AxisListType.X, op=ALU.add)
                nc.gpsimd.dma_start(
                    out=sc["bn_in_" + sfx][:].rearrange(
                        "(lt p) c -> p lt c", p=128),
                    in_=bn_sb[:, lt0:lt1, :])
                nc.gpsimd.collective_compute(
                    "AllGather", ALU.bypass, replica_groups=RG,
                    ins=[sc["bn_in_" + sfx][:]],
                    outs=[sc["bn_out_" + sfx][:]])
                ctx_bn.__exit__(None, None, None)

            bnT = [pd1.tile([8, L], F16, name=f"bnT{m}") for m in range(2)]
            for m in range(2):
                for _seg, _w, lt0, lt1, sfx in SEGS:
                    nseg = lt1 - lt0
                    bng = pd1.tile([128, 8, 8], F32, tag="bng", bufs=3,
                                   name=f"bng{m}{sfx}")
                    nc.sync.dma_start(
                        out=bng[:, 0:nseg, :],
                        in_=sc["bn_out_" + sfx][m].rearrange(
                            "(lt p) c -> p lt c", p=128))
                    for lts in range(nseg):
                        lt = lt0 + lts
                        ptb = psD.tile([128, 512], F32, tag="pX5", bufs=3,
                                       name="ptb")
                        ptbv = ptb[0:8, 0:128]
                        nc.tensor.transpose(ptbv, bng[:, lts, :], ident)
                        nc.scalar.mul(out=bnT[m][:, lt*128:(lt+1)*128],
                                      in_=ptbv, mul=1.0/d)
            # ---- fusion tail: bn part + gelu -----------------------------
            fb1_sb = pd1.tile([128, 8], F32)
            nc.sync.dma_start(out=fb1_sb,
                              in_=io["fb1"][:].rearrange("(m p) -> p m",
                                                         p=128))
            fw1b_sb = pd1.tile([8, 2, 1024], F16)
            nc.sync.dma_start(
                out=fw1b_sb,
                in_=io["fw1b"][:].rearrange("(m p) c -> p m c", p=8))
            for lw in range(NLW):
                for mt in range(8):
                    ph2 = psD.tile([128, 512], F32, tag="pX5", bufs=3, name="psh2")
                    for m in range(2):
                        nc.tensor.matmul(ph2,
                                         fw1b_sb[:, m, mt*128:(mt+1)*128],
                                         bnT[m][:, lw*512:(lw+1)*512],
                                         start=(m == 0), stop=(m == 1))
                    hdf = pdw.tile([128, 512], F16, tag="hdf", bufs=3,
                                   name="hdf")
                    nc.vector.scalar_tensor_tensor(
                        out=hdf, in0=ph2, scalar=1.0,
                        in1=hdn_pre[:, mt, lw*512:(lw+1)*512],
                        op0=ALU.mult, op1=ALU.add)
                    nc.scalar.activation(
                        out=hdn_pre[:, mt, lw*512:(lw+1)*512], in_=hdf,
                        func=AF.Gelu, bias=fb1_sb[:, mt:mt+1])

            fw2_sb = pd1.tile([128, 8, 12], F16)
            nc.sync.dma_start(
                out=fw2_sb,
                in_=io["fw2"][:].rearrange("(kt p) c -> p kt c", p=128))
            lg_sb = pd1.tile([128, NLT, 12], F32)
            for lt in range(NLT):
                pl = psD.tile([128, 512], F32, tag="pX5", bufs=3, name="psl")
                plv = pl[:, 0:12]
                for kt in range(8):
                    nc.tensor.matmul(plv, hdn_pre[:, kt, lt*128:(lt+1)*128],
                                     fw2_sb[:, kt, :],
                                     start=(kt == 0), stop=(kt == 7))
                nc.scalar.copy(out=lg_sb[:, lt, :], in_=plv)
            for m in range(2):
                nc.gpsimd.dma_start(
                    out=sc["lg_in"][m].rearrange("(lt p) c -> p lt c", p=128),
                    in_=lg_sb[:, :, m*6:(m+1)*6])
            nc.gpsimd.collective_compute(
                "ReduceScatter", ALU.add, replica_groups=RG,
                ins=[sc["lg_in"][:]], outs=[sc["lg_out"][:]])

        # =================== PHASE C ======================================
        with tc.tile_pool(name="pc1", bufs=1) as pc1, \
             tc.tile_pool(name="pc2", bufs=2) as pc2, \
             tc.tile_pool(name="pcs", bufs=4) as pcs, \
             tc.tile_pool(name="psC", bufs=1, space="PSUM") as psC:

            def psc(name, tag="pg"):
                return psC.tile([128, 512], F32, tag=tag, bufs=2, name=name)

            # ---- softmax gates -------------------------------------------
            b2_sb = pc1.tile([128, 6], F32)
            nc.sync.dma_start(out=b2_sb, in_=io["b2o"][:])
            lgo = pc1.tile([128, NLT, 2, 3], F32)
            nc.sync.dma_start(
                out=lgo,
                in_=sc["lg_out"][:].rearrange("(lt p) (h e) -> p lt h e",
                                              p=128, e=3))
            nc.vector.tensor_tensor(
                out=lgo, in0=lgo,
                in1=bass.AP(tensor=b2_sb.tensor, offset=b2_sb.offset,
                            ap=[b2_sb.ap[0], [0, NLT], [3, 2], [1, 3]]),
                op=ALU.add)
            rmax = pc1.tile([128, NLT, 2], F32)
            nc.vector.tensor_reduce(out=rmax, in_=lgo,
                                    axis=mybir.AxisListType.X, op=ALU.max)
            nc.vector.tensor_tensor(
                out=lgo, in0=lgo,
                in1=rmax[:, :, :, None].to_broadcast([128, NLT, 2, 3]),
                op=ALU.subtract)
            nc.scalar.activation(out=lgo, in_=lgo, func=AF.Exp)
            rsum = pc1.tile([128, NLT, 2], F32)
            nc.vector.tensor_reduce(out=rsum, in_=lgo,
                                    axis=mybir.AxisListType.X, op=ALU.add)
            nc.vector.reciprocal(out=rsum, in_=rsum)
            nc.vector.tensor_tensor(
                out=lgo, in0=lgo,
                in1=rsum[:, :, :, None].to_broadcast([128, NLT, 2, 3]),
                op=ALU.mult)

            # ---- gate mix + RMSNorm + Wo ---------------------------------
            wo_sb = pc1.tile([128, 4, D], F16)
            nc.sync.dma_start(
                out=wo_sb,
                in_=io["wo"][:].rearrange("(kt p) n -> p kt n", p=128))
            dout_sb = arena[:, 0:8192].rearrange("p (lt h e) -> p lt h e",
                                                 h=2, e=d)
            cm_sb = arena[:, 8192:16384].rearrange("p (lt c) -> p lt c",
                                                   c=C)
            nc.gpsimd.dma_start(
                out=cm_sb,
                in_=sc["cm_out"][:].rearrange("(lt p) c -> p lt c", p=128))
            nc.gpsimd.dma_start(
                out=dout_sb,
                in_=sc["dout_s"][:].rearrange("(lt p) (h e) -> p lt h e",
                                              p=128, e=d))
            for lt in range(NLT):
                dov = dout_sb[:, lt]
                o_t = pcs.tile([128, 2, d], F16, tag="o_t", name="o_t")
                ssq = pcs.tile([128, 2], F32, tag="ssq", name="ssq")
                scr = pcs.tile([128, d], F32, tag="scr", name="scr")
                for h in range(2):
                    nc.vector.tensor_scalar_mul(
                        o_t[:, h, :],
                        cm_sb[:, lt, h*256:(h+1)*256], lgo[:, lt, h, 0:1])
                    nc.vector.scalar_tensor_tensor(
                        out=o_t[:, h, :], in0=dov[:, h, :],
                        scalar=lgo[:, lt, h, 1:2], in1=o_t[:, h, :],
                        op0=ALU.mult, op1=ALU.add)
                    nc.vector.scalar_tensor_tensor(
                        out=o_t[:, h, :].rearrange("p (a b) -> p a b", a=2),
                        in0=vlc[:, h, :, lt, :],
                        scalar=lgo[:, lt, h, 2:3],
                        in1=o_t[:, h, :].rearrange("p (a b) -> p a b", a=2),
                        op0=ALU.mult, op1=ALU.add)
                    nc.scalar.activation(out=scr, in_=o_t[:, h, :],
                                         func=AF.Square,
                                         accum_out=ssq[:, h:h+1])
                nc.scalar.activation(out=ssq, in_=ssq, func=AF.Sqrt,
                                     scale=1.0/d, bias=eps5)
                nc.vector.reciprocal(out=ssq, in_=ssq)
                for h in range(2):
                    nc.vector.tensor_scalar_mul(o_t[:, h, :], o_t[:, h, :],
                                                ssq[:, h:h+1])
                pto = psC.tile([128, 4, 128], F16, tag="pto", bufs=2,
                               name="psto")
                for ct in range(4):
                    h, dt = ct // 2, ct % 2
                    nc.tensor.transpose(pto[:, ct, :],
                                        o_t[:, h, dt*128:(dt+1)*128],
                                        ident16)
                oT = pcs.tile([128, 4, 128], F16, tag="oT", name="oT")
                nc.vector.tensor_copy(out=oT, in_=pto)
                orow = pcs.tile([128, D], F32, tag="orow", name="orow")
                for nh in range(2):
                    pw = psC.tile([128, 512], F32, tag="pw", bufs=2, name="psw")
                    for ct in range(4):
                        nc.tensor.matmul(pw, oT[:, ct, :],
                                         wo_sb[:, ct, nh*512:(nh+1)*512],
                                         start=(ct == 0), stop=(ct == 3))
                    nc.scalar.copy(out=orow[:, nh*512:(nh+1)*512], in_=pw)
                nc.sync.dma_start(out=io["out_part"][lt*128:(lt+1)*128, :],
                                  in_=orow)


# ======================= host side =======================================

def _diag_tiles(w_own, taps, out_dtype):
    """w_own: (C, k) conv weights for this core's channels.
    Returns (4, k, 128, 128) diag tiles; tap dd uses column k-1-dd."""
    k = w_own.shape[1]
    out = np.zeros((4, k, 128, 128), dtype=out_dtype)
    for ct in range(4):
        for dd in range(k):
            np.fill_diagonal(out[ct, dd], w_own[ct*128:(ct+1)*128, k-1-dd])
    return out


def _host_inputs(inputs):
    hs = np.asarray(inputs["hidden_states"], np.float32)
    Wq = np.asarray(inputs["Wq"], np.float32)
    Wk = np.asarray(inputs["Wk"], np.float32)
    Wv = np.asarray(inputs["Wv"], np.float32)
    Wb = np.asarray(inputs["Wb"], np.float32)
    cq = np.asarray(inputs["conv_q_w"], np.float32)
    ck = np.asarray(inputs["conv_k_w"], np.float32)
    cv = np.asarray(inputs["conv_v_w"], np.float32)
    w3 = np.asarray(inputs["ms_w3"], np.float32)
    w15 = np.asarray(inputs["ms_w15"], np.float32)
    w31 = np.asarray(inputs["ms_w31"], np.float32)
    kmix = np.asarray(inputs["kernel_mix_w"], np.float32)
    cmix = np.asarray(inputs["channel_mixer_w"], np.float32)
    fw1 = np.asarray(inputs["fusion_w1"], np.float32)
    fb1 = np.asarray(inputs["fusion_b1"], np.float32)
    fw2 = np.asarray(inputs["fusion_w2"], np.float32)
    fb2 = np.asarray(inputs["fusion_b2"], np.float32)
    onw = np.asarray(inputs["o_norm_w"], np.float32)
    Wo = np.asarray(inputs["Wo"], np.float32)

    # combined kernel_mix -> channel_mixer matrix Q: (3D, D)
    Q = np.zeros((3 * D, D), np.float32)
    for h in range(H):
        Q[h*3*d:(h+1)*3*d] = kmix @ cmix[h*d:(h+1)*d]

    masks = np.zeros((5, 128, 128), np.float32)
    i_, j_ = np.mgrid[0:128, 0:128]
    blk = (i_ // 32) == (j_ // 32)
    masks[0] = -((i_ > j_) & blk).astype(np.float32)
    masks[1] = -((i_ > j_) & ~blk).astype(np.float32)
    masks[2] = -((j_ > i_) & blk).astype(np.float32)
    masks[3] = (j_ >= i_).astype(np.float32)
    masks[4] = np.eye(128, dtype=np.float32)

    Wo_s = Wo * np.tile(onw, H)[:, None]

    in_maps = []
    for c in range(8):
        b, r = divmod(c, 2)
        cs = slice(C*r, C*(r+1))
        qmix = np.concatenate(
            [Q[1024*s + C*r: 1024*s + C*r + C] for s in range(3)], 0)
        f8 = ml_dtypes.float8_e4m3
        mspair = np.zeros((4, 23, 2, 128, 128), np.float32)
        mssing = np.zeros((4, 3, 128, 128), np.float32)
        for ct in range(4):
            pi = 0
            for si, (w, ks) in enumerate(((w3, 3), (w15, 15), (w31, 31))):
                dg = _diag_tiles(w[cs], ks, np.float32)[ct] * 64.0
                for p_ in range((ks - 1) // 2):
                    mspair[ct, pi, 0] = dg[2*p_ + 1]
                    mspair[ct, pi, 1] = dg[2*p_]
                    pi += 1
                mssing[ct, si] = dg[ks - 1]
        cdiag = np.stack([_diag_tiles(w[cs], KQKV, np.float16)
                          for w in (cq, ck, cv)], 0)
        fw1b = np.zeros((16, 1024), np.float32)
        for m in range(2):
            for src in range(3):
                for h_ in range(2):
                    fw1b[m*8 + src*2 + h_] = \
                        fw1[D + src*4 + 2*m + h_, 1024*r:1024*(r+1)]
        fw2p = np.zeros((1024, 12), np.float32)
        b2o = np.zeros((6,), np.float32)
        for jm in range(2):
            for h_ in range(2):
                for br in range(3):
                    gcol = (2*jm + h_)*3 + br
                    fw2p[:, jm*6 + h_*3 + br] = fw2[1024*r:1024*(r+1), gcol]
        for h_ in range(2):
            for br in range(3):
                b2o[h_*3 + br] = fb2[(2*r + h_)*3 + br]
        m = {
            "hsT": np.ascontiguousarray(hs[b].T).astype(np.float16),
            "wq": np.ascontiguousarray(Wq[:, cs]).astype(np.float16),
            "wk": np.ascontiguousarray(Wk[:, cs]).astype(np.float16),
            "wv": np.ascontiguousarray(Wv[:, cs]).astype(np.float16),
            "wb": np.ascontiguousarray(Wb[:, 2*r:2*r+2]).astype(np.float16),
            "cdiag": cdiag,
            "mspair": mspair.astype(f8),
            "mssing": mssing.astype(f8),
            "qmix8": (qmix * 64.0).astype(f8),
            "fw1h": np.ascontiguousarray(
                fw1[:D, 1024*r:1024*(r+1)]).astype(np.float16),
            "fw1b": fw1b.astype(np.float16),
            "fb1": np.ascontiguousarray(fb1[1024*r:1024*(r+1)]),
            "fw2": fw2p.astype(np.float16),
            "b2o": np.tile(b2o, (128, 1)),
            "wo": np.ascontiguousarray(Wo_s[cs, :]).astype(np.float16),
            "masks": masks,
            "onesrow": np.ones((1, 128), np.float32),
            "onescol": np.ones((128, 1), np.float32),
            "ident16": np.eye(128, dtype=np.float16),
        }
        in_maps.append(m)
    return in_maps


_PROG = {}


def _get_program(debug=False):
    key = bool(debug)
    if key not in _PROG:
        _PROG[key] = build_program(debug=debug)
    return _PROG[key]


def run(inputs, debug=False, **kw):
    nc = _get_program(debug=debug)
    in_maps = _host_inputs(inputs)
    res = run_bass_kernel_spmd(nc, in_maps, list(range(8)), **kw)
    return res


def kernel(**inputs):
    res = run(inputs)
    out = np.zeros((B, L, D), np.float32)
    for b in range(B):
        out[b] = res.results[2*b]["out_part"] + res.results[2*b+1]["out_part"]
    return out


if __name__ == "__main__":
    nc = build_program()
    print("program built ok")


# revision 101
# speedup vs baseline: 1.0592x; 1.0022x over previous
"""DeltaNet-style block (nn_DeltaNet_31877247271438) on 8 trn2 NeuronCores.

Sharding: core c -> (batch b = c//2, pair-rank r = c%2).  Within a batch pair:
  - head-parallel: rank r owns heads {2r, 2r+1} (feature cols [512r, 512r+512))
  - cross-head mixes are K-split with pairwise collectives:
      * channel_mixer (folded with kernel_mix into one matrix Q): partial sums
        ReduceScatter'ed (fp16) so each core receives its own heads' ms_out
      * fusion-MLP hidden is column-split; logits partials ReduceScatter'ed
      * bn features AllGather'ed (tiny)
  - the final Wo matmul partials are summed on the host.

v2: inter-phase tensors stay in SBUF (no DRAM staging round-trips); l-major
forms of k/v come from DMA-xbar transposes; the delta rule's 16 chunk
inversions are batched (only the S recurrence is serial); beta is folded
into the mask multiply so the kb row form is never materialized; a manually
aliased SBUF arena lets phase-scoped tensors (hsT/xc, y, u/wT/aT, hdn_pre)
share the same bytes.
"""
import sys
sys.path.insert(0, '/opt/trn_rl_repo')

import numpy as np
import ml_dtypes

import concourse.bass as bass
import concourse.tile as tile
from concourse import bacc, mybir
from concourse.bass_utils import run_bass_kernel_spmd

F32 = mybir.dt.float32
F16 = mybir.dt.float16
F8 = mybir.dt.float8e4
DR = mybir.MatmulPerfMode.DoubleRow
FP8S = 64.0
AF = mybir.ActivationFunctionType
ALU = mybir.AluOpType

B, L, D, H = 4, 2048, 1024, 4
d = 256          # per-head dim
C = 512          # channels owned per core (2 heads)
NLT = 16         # l-tiles of 128
NLW = 4          # l-windows of 512
NCH = 16         # delta chunks of 128
KQKV = 4         # qkv conv taps
MSK = (3, 15, 31)
NTAPS = sum(MSK)  # 49
PADV = 32
RG = [[0, 1], [2, 3], [4, 5], [6, 7]]

ARENA = 36864    # f16 elems per partition in the aliased arena


def bc_mid(ap2, n):
    """[P, F] AP -> [P, n, F] with a 0-stride middle dim (free-dim bcast)."""
    assert len(ap2.ap) == 2
    return bass.AP(tensor=ap2.tensor, offset=ap2.offset,
                   ap=[ap2.ap[0], [0, n], ap2.ap[1]])


def build_program(debug=False):
    nc = bacc.Bacc("TRN2", target_bir_lowering=False, debug=False,
                   num_devices=8)

    io = {}
    io["hsT"] = nc.declare_dram_parameter("hsT", [D, L], F16, False)
    io["wq"] = nc.declare_dram_parameter("wq", [D, C], F16, False)
    io["wk"] = nc.declare_dram_parameter("wk", [D, C], F16, False)
    io["wv"] = nc.declare_dram_parameter("wv", [D, C], F16, False)
    io["wb"] = nc.declare_dram_parameter("wb", [D, 2], F16, False)
    io["cdiag"] = nc.declare_dram_parameter("cdiag", [3, 4, KQKV, 128, 128],
                                            F16, False)
    io["mspair"] = nc.declare_dram_parameter("mspair", [4, 23, 2, 128, 128],
                                             F8, False)
    io["mssing"] = nc.declare_dram_parameter("mssing", [4, 3, 128, 128],
                                             F8, False)
    io["qmix8"] = nc.declare_dram_parameter("qmix8", [12 * 128, D], F8, False)
    io["fw1h"] = nc.declare_dram_parameter("fw1h", [D, 1024], F16, False)
    io["fw1b"] = nc.declare_dram_parameter("fw1b", [16, 1024], F16, False)
    io["fb1"] = nc.declare_dram_parameter("fb1", [1024], F32, False)
    io["fw2"] = nc.declare_dram_parameter("fw2", [1024, 12], F16, False)
    io["b2o"] = nc.declare_dram_parameter("b2o", [128, 6], F32, False)
    io["wo"] = nc.declare_dram_parameter("wo", [C, D], F16, False)
    io["masks"] = nc.declare_dram_parameter("masks", [5, 128, 128], F32, False)
    io["onesrow"] = nc.declare_dram_parameter("onesrow", [1, 128], F32, False)
    io["onescol"] = nc.declare_dram_parameter("onescol", [128, 1], F32, False)
    io["ident16"] = nc.declare_dram_parameter("ident16", [128, 128], F16, False)
    io["out_part"] = nc.declare_dram_parameter("out_part", [L, D], F32, True)

    sc = {}
    sc["dout_s"] = nc.dram_tensor("dout_s", [L, C], F16)
    sc["cm_in"] = nc.dram_tensor("cm_in", [2, L, C], F16)
    sc["cm_out"] = nc.dram_tensor("cm_out", [L, C], F16)
    sc["bn_in_a"] = nc.dram_tensor("bn_in_a", [1024, 8], F32)
    sc["bn_out_a"] = nc.dram_tensor("bn_out_a", [2, 1024, 8], F32)
    sc["bn_in_b"] = nc.dram_tensor("bn_in_b", [512, 8], F32)
    sc["bn_out_b"] = nc.dram_tensor("bn_out_b", [2, 512, 8], F32)
    sc["bn_in_c"] = nc.dram_tensor("bn_in_c", [512, 8], F32)
    sc["bn_out_c"] = nc.dram_tensor("bn_out_c", [2, 512, 8], F32)
    sc["lg_in"] = nc.dram_tensor("lg_in", [2, L, 6], F32)
    sc["lg_out"] = nc.dram_tensor("lg_out", [L, 6], F32)

    with tile.TileContext(nc) as tc:
        _body(nc, tc, io, sc)
    nc.compile()
    return nc


def _body(nc, tc, io, sc):
    from contextlib import ExitStack
    ctx = ExitStack()
    with ctx:
        consts = ctx.enter_context(tc.tile_pool(name="consts", bufs=1))
        outer = ctx.enter_context(tc.tile_pool(name="outer", bufs=1))

        masks = consts.tile([128, 5, 128], F32)
        nc.sync.dma_start(out=masks,
                          in_=io["masks"][:].rearrange("m p f -> p m f"))
        ident = masks[:, 4, :]
        ident16 = consts.tile([128, 128], F16)
        nc.sync.dma_start(out=ident16, in_=io["ident16"][:])
        onescol16 = consts.tile([128, 1], F16)
        nc.vector.memset(onescol16, 1.0)
        onesrow16 = consts.tile([1, 128], F16)
        nc.vector.memset(onesrow16, 1.0)
        eps6 = consts.tile([128, 1], F32)
        nc.vector.memset(eps6, 1e-6)
        eps5 = consts.tile([128, 1], F32)
        nc.vector.memset(eps5, 1e-5)

        beta_lp = outer.tile([128, NLT, 2], F32)
        S16 = outer.tile([128, 2, 2, d], F16)
        nc.vector.memset(S16, 0.0)
        qT = outer.tile([128, 2, 2, L], F16)       # (h, dtile, l) row form
        kT = outer.tile([128, 2, 2, L], F16)
        klc = outer.tile([128, 2, 2, NLT, 128], F16)  # (h, dtile, lt, dk)
        vlc = outer.tile([128, 2, 2, NLT, 128], F16)
        bn_sb = outer.tile([128, NLT, 8], F32)
        bnv4 = outer.tile([128, NLT, 4], F32)
        arena = outer.tile([128, ARENA], F16)

        # arena regions (manually aliased, lifetimes disjoint)
        hsT = arena[:, 0:16384].rearrange("p (kt l) -> p kt l", l=L)
        xc = arena[:, 16384:24592].rearrange("p (ct l) -> p ct l", l=L + 4)
        vt_bf = arena[:, 24592:32912].rearrange("p (ct l) -> p ct l",
                                                l=PADV + L)
        y_bf = arena[:, 0:24576].rearrange("p (j l) -> p j l", l=L)
        u_all = arena[:, 0:2048].rearrange("p (n h e) -> p n h e", h=2, e=d)
        wT_all = arena[:, 2048:4096].rearrange("p (n kt h e) -> p n kt h e",
                                               kt=2, h=2, e=128)
        aT_all = arena[:, 4096:5120].rearrange("p (n h e) -> p n h e",
                                               h=2, e=128)
        hdn_pre = arena[:, 20480:36864].rearrange("p (mt l) -> p mt l", l=L)

        hsT_r = io["hsT"][:].rearrange("(kt p) l -> p kt l", p=128)

        # =================== PHASE A ======================================
        with tc.tile_pool(name="pa1", bufs=1) as pa1, \
             tc.tile_pool(name="pa2", bufs=2) as pa2, \
             tc.tile_pool(name="pas", bufs=3) as pas, \
             tc.tile_pool(name="psA", bufs=1, space="PSUM") as psA:
            nc.sync.dma_start(out=hsT, in_=hsT_r)

            # ---- beta (l-partition form) ---------------------------------
            wb_sb = pa1.tile([128, 8, 2], F16)
            nc.sync.dma_start(
                out=wb_sb, in_=io["wb"][:].rearrange("(kt p) c -> p kt c",
                                                     p=128))
            for lt in range(NLT):
                pb = psA.tile([128, 2], F32, tag="pb", bufs=1, name="psb")
                for kt in range(8):
                    nc.tensor.matmul(pb, hsT[:, kt, lt*128:(lt+1)*128],
                                     wb_sb[:, kt, :],
                                     start=(kt == 0), stop=(kt == 7))
                nc.scalar.activation(out=beta_lp[:, lt, :], in_=pb,
                                     func=AF.Sigmoid)

            # ---- q, k, v: proj -> conv -> silu -> (norm) -----------------
            for tnm, ti in (("v", 2), ("k", 1), ("q", 0)):
                PAD = 4
                w_sb = pa2.tile([128, 8, C], F16, tag="w_sb", bufs=2,
                                name=f"w_{tnm}")
                nc.sync.dma_start(
                    out=w_sb,
                    in_=io["w" + tnm][:].rearrange("(kt p) c -> p kt c",
                                                   p=128))
                cdg = pa2.tile([128, 4, KQKV, 128], F16, tag="cdg", bufs=2,
                               name=f"cdg_{tnm}")
                nc.sync.dma_start(
                    out=cdg,
                    in_=io["cdiag"][ti].rearrange("ct tap p f -> p ct tap f"))
                if tnm != "v":
                    nc.vector.memset(xc[:, :, 0:4], 0.0)
                else:
                    nc.vector.memset(vt_bf[:, :, 0:PADV], 0.0)

                for ct in range(4):
                    xp = pa2.tile([128, 4 + L], F16, tag="xp", bufs=2,
                                  name=f"xp_{tnm}{ct}")
                    nc.vector.memset(xp[:, 0:4], 0.0)
                    for lw in range(NLW):
                        pp = psA.tile([128, 512], F32, tag="pp", bufs=2,
                                      name="psp")
                        for kt in range(8):
                            nc.tensor.matmul(
                                pp, w_sb[:, kt, ct*128:(ct+1)*128],
                                hsT[:, kt, lw*512:(lw+1)*512],
                                start=(kt == 0), stop=(kt == 7))
                        nc.scalar.copy(out=xp[:, 4+lw*512:4+(lw+1)*512],
                                       in_=pp)
                    for lw in range(NLW):
                        pc = psA.tile([128, 512], F32, tag="pc", bufs=2,
                                      name="psc")
                        for dd in range(KQKV):
                            off = 4 + lw*512 - dd
                            nc.tensor.matmul(
                                pc, cdg[:, ct, dd, :], xp[:, off:off+512],
                                start=(dd == 0), stop=(dd == KQKV-1))
                        if tnm == "v":
                            nc.scalar.activation(
                                out=vt_bf[:, ct, PADV+lw*512:PADV+(lw+1)*512],
                                in_=pc, func=AF.Silu)
                        else:
                            nc.scalar.activation(
                                out=xc[:, ct, PAD+lw*512:PAD+(lw+1)*512],
                                in_=pc, func=AF.Silu)
                if tnm == "v":
                    for ct in range(4):
                        nc.sync.dma_start_transpose(
                            out=vlc[:, ct // 2, ct % 2],
                            in_=vt_bf[:, ct, PADV:PADV+L])
                    continue

                # ---- l2norm -> qT / kT row forms -------------------------
                dst = qT if tnm == "q" else kT
                for h in range(2):
                    for lw in range(NLW):
                        lsl = slice(PAD+lw*512, PAD+(lw+1)*512)
                        osl = slice(lw*512, (lw+1)*512)
                        sqs = pas.tile([128, 2, 512], F16, tag="sq", bufs=3,
                                       name="sq")
                        for i, ct in enumerate((2*h, 2*h+1)):
                            nc.vector.tensor_tensor(
                                out=sqs[:, i, :], in0=xc[:, ct, lsl],
                                in1=xc[:, ct, lsl], op=ALU.mult)
                        ssr = pas.tile([1, 2, 512], F32, tag="ssr", bufs=2,
                                       name="ssr")
                        nc.gpsimd.tensor_reduce(
                            out=ssr, in_=sqs, axis=mybir.AxisListType.C,
                            op=ALU.add)
                        sr = pas.tile([1, 512], F32, tag="sr", bufs=2,
                                      name="sr")
                        nc.vector.tensor_tensor(out=sr, in0=ssr[:, 0, :],
                                                in1=ssr[:, 1, :], op=ALU.add)
                        nc.scalar.activation(out=sr, in_=sr, func=AF.Sqrt,
                                             bias=eps6[0:1, :])
                        srt = pas.tile([1, 512], F16, tag="srt", bufs=2,
                                       name="srt")
                        with nc.allow_low_precision("l2norm scale fp16"):
                            nc.vector.reciprocal(out=srt, in_=sr)
                        pbc = psA.tile([128, 512], F32, tag="pn2", bufs=2,
                                       name="psbc")
                        nc.tensor.matmul(pbc, onesrow16, srt,
                                         start=True, stop=True)
                        for dt in range(2):
                            ct = 2*h + dt
                            nc.vector.tensor_tensor(
                                out=dst[:, h, dt, osl], in0=xc[:, ct, lsl],
                                in1=pbc, op=ALU.mult)
                if tnm == "k":
                    for h in range(2):
                        for dt in range(2):
                            nc.sync.dma_start_transpose(
                                out=klc[:, h, dt], in_=kT[:, h, dt, :])

            # bn(v): |v| partial sums (finalized later)
            for lt in range(NLT):
                nc.vector.tensor_reduce(
                    out=bnv4[:, lt, :], in_=vlc[:, :, :, lt, :],
                    axis=mybir.AxisListType.X, op=ALU.add,
                    apply_absolute_value=True)

        # =================== PHASE B: multiscale conv + qmix ==============
        with tc.tile_pool(name="pb1", bufs=1) as pb1, \
             tc.tile_pool(name="pbm", bufs=2) as pbm, \
             tc.tile_pool(name="pbs", bufs=3) as pbs, \
             tc.tile_pool(name="psB", bufs=1, space="PSUM") as psB:
            y8 = arena[:, 0:12288].bitcast(F8).rearrange(
                "p (j l) -> p j l", l=L)
            vt8 = arena[:, 12288:16448].bitcast(F8).rearrange(
                "p (ct l) -> p ct l", l=PADV + L)
            with nc.allow_low_precision("fp8 conv input"):
                for ct in range(4):
                    nc.vector.tensor_copy(out=vt8[:, ct, :],
                                          in_=vt_bf[:, ct, :])

            def pair_ap(off, n, ct):
                a = vt8[:, ct, off:off+n]
                return bass.AP(tensor=a.tensor, offset=a.offset,
                               ap=[a.ap[0], [1, 2], a.ap[1]])

            for ct in range(4):
                msd8 = pbm.tile([128, 23, 2, 128], F8, tag=f"msd{ct % 2}",
                                bufs=1, name=f"msd8_{ct}")
                nc.gpsimd.dma_start(
                    out=msd8,
                    in_=io["mspair"][ct].rearrange("j t p f -> p j t f"))
                mss8 = pbm.tile([128, 3, 128], F8, tag=f"mss{ct % 2}",
                                bufs=1, name=f"mss8_{ct}")
                nc.gpsimd.dma_start(
                    out=mss8,
                    in_=io["mssing"][ct].rearrange("j p f -> p j f"))
                for lw in range(NLW):
                    base_pi = 0
                    for si, ks in enumerate(MSK):
                        npair = (ks - 1) // 2
                        py = psB.tile([128, 512], F32, tag="py", bufs=4,
                                      name="psy")
                        for p_ in range(npair):
                            off = PADV + lw*512 - (2*p_ + 1)
                            nc.tensor.matmul(
                                py, msd8[:, base_pi + p_, :, :],
                                pair_ap(off, 512, ct),
                                start=(p_ == 0), stop=False, perf_mode=DR)
                        off = PADV + lw*512 - (ks - 1)
                        nc.tensor.matmul(py, mss8[:, si, :],
                                         vt8[:, ct, off:off+512],
                                         start=False, stop=True)
                        nc.scalar.mul(
                            out=y8[:, si*4+ct, lw*512:(lw+1)*512],
                            in_=py, mul=1.0/FP8S)
                        base_pi += npair

            qmix8_sb = pb1.tile([128, 6, 2, D], F8)
            nc.gpsimd.dma_start(
                out=qmix8_sb,
                in_=io["qmix8"][:].rearrange("(pp j p) o -> p pp j o",
                                             j=2, p=128))
            cms_v = [arena[:, 33040+i*1024:33040+(i+1)*1024].rearrange(
                "p (o c) -> p o c", o=2) for i in range(3)]
            for lt in range(NLT):
                cms = cms_v[lt % 3]
                for oh in range(2):
                    pq = psB.tile([128, 512], F32, tag="pq", bufs=4,
                                  name="psq")
                    for p_ in range(6):
                        nc.tensor.matmul(
                            pq, y8[:, 2*p_:2*p_+2, lt*128:(lt+1)*128],
                            qmix8_sb[:, p_, :, oh*512:(oh+1)*512],
                            start=(p_ == 0), stop=(p_ == 5), perf_mode=DR)
                    if oh == 0:
                        nc.vector.tensor_scalar_mul(cms[:, oh, :], pq,
                                                    1.0/FP8S)
                    else:
                        nc.scalar.mul(out=cms[:, oh, :], in_=pq,
                                      mul=1.0/FP8S)
                nc.gpsimd.dma_start(
                    out=sc["cm_in"][:, lt*128:(lt+1)*128, :].rearrange(
                        "o l c -> l o c"),
                    in_=cms)
            nc.gpsimd.collective_compute(
                "ReduceScatter", ALU.add, replica_groups=RG,
                ins=[sc["cm_in"][:]], outs=[sc["cm_out"][:]])

        # =================== PHASE B3: delta rule =========================
        with tc.tile_pool(name="pd1", bufs=1) as pd1, \
             tc.tile_pool(name="pdc", bufs=1) as pdc, \
             tc.tile_pool(name="pdw", bufs=1) as pdw, \
             tc.tile_pool(name="psD", bufs=1, space="PSUM") as psD:

            def blk(name, bufs=2):
                return pdc.tile([128, 2, 128], F16, tag=name, name=name,
                                bufs=bufs)

            def pd_(name):
                return psD.tile([128, 2, 128], F32, tag="pd", bufs=3,
                                name=name)

            def mm2(pt, lhs_fn, rhs_fn, n_k=1):
                for h in range(2):
                    for kt in range(n_k):
                        nc.tensor.matmul(pt[:, h, :], lhs_fn(h, kt),
                                         rhs_fn(h, kt),
                                         start=(kt == 0), stop=(kt == n_k-1))

            GRP = 3
            specs = [("T2", "TdT", "Td"), ("T2T", "Td", "TdT"),
                     ("T4", "T2T", "T2"), ("T4T", "T2", "T2T"),
                     ("T8", "T4T", "T4"), ("T8T", "T4", "T4T"),
                     ("T16", "T8T", "T8")]

            def blkg(name):
                return pdc.tile([128, 2, 128], F16, tag=name, name=name,
                                bufs=3)

            fw1h_r = io["fw1h"][:].rearrange("(kt p) m -> p kt m", p=128)
            _fus_state = {"hst": None}

            def _emit_fusion(step):
                lw, mt = step // 8, step % 8
                if mt == 0:
                    hst = pd1.tile([128, 8, 512], F16, tag="hst", bufs=1,
                                   name="hst")
                    nc.sync.dma_start(out=hst,
                                      in_=hsT_r[:, :, lw*512:(lw+1)*512])
                    _fus_state["hst"] = hst
                hst = _fus_state["hst"]
                fwt = pd1.tile([128, 8, 128], F16, tag="fwt", bufs=4,
                               name=f"fwt{mt}")
                nc.sync.dma_start(out=fwt,
                                  in_=fw1h_r[:, :, mt*128:(mt+1)*128])
                ph = psD.tile([128, 512], F32, tag="pX5", bufs=3,
                              name="psh")
                for kt in range(8):
                    nc.tensor.matmul(ph, fwt[:, kt, :], hst[:, kt, :],
                                     start=(kt == 0), stop=(kt == 7))
                if mt % 2 == 0:
                    nc.scalar.copy(
                        out=hdn_pre[:, mt, lw*512:(lw+1)*512], in_=ph)
                else:
                    nc.vector.tensor_copy(
                        out=hdn_pre[:, mt, lw*512:(lw+1)*512], in_=ph)

            X_v = [arena[:, 5120+i*1024:5120+(i+1)*1024].rearrange(
                "p (h e) -> p h e", h=2) for i in range(4)]
            x1_v = [arena[:, 9216+i*1024:9216+(i+1)*1024].rearrange(
                "p (h e) -> p h e", h=2) for i in range(3)]
            x1_v.append(arena[:, 15360:16384].rearrange(
                "p (h e) -> p h e", h=2))
            y1_v = [arena[:, 12288+i*1024:12288+(i+1)*1024].rearrange(
                "p (h e) -> p h e", h=2) for i in range(3)]
            y1_v.append(arena[:, 16384:17408].rearrange(
                "p (h e) -> p h e", h=2))
            for g0 in range(0, NCH, GRP):
                cis = list(range(g0, min(g0 + GRP, NCH)))
                ls = {ci: slice(ci*128, (ci+1)*128) for ci in cis}
                t = {ci: {} for ci in cis}

                # step: G = K K^T, mask+beta -> Td, To
                for ci in cis:
                    pG = pd_("pG")
                    mm2(pG, lambda h, kt: kT[:, h, kt, ls[ci]],
                        lambda h, kt: kT[:, h, kt, ls[ci]], n_k=2)
                    t[ci]["Td"], t[ci]["To"] = blkg("Td"), blkg("To")
                    for h in range(2):
                        nc.vector.scalar_tensor_tensor(
                            out=t[ci]["Td"][:, h, :], in0=pG[:, h, :],
                            scalar=beta_lp[:, ci, h:h+1], in1=masks[:, 0, :],
                            op0=ALU.mult, op1=ALU.mult)
                        nc.vector.scalar_tensor_tensor(
                            out=t[ci]["To"][:, h, :], in0=pG[:, h, :],
                            scalar=beta_lp[:, ci, h:h+1], in1=masks[:, 1, :],
                            op0=ALU.mult, op1=ALU.mult)
                # step: TdT = transpose(Td)
                for ci in cis:
                    pT = psD.tile([128, 2, 128], F16, tag="pdT", bufs=1,
                                  name="pTdT")
                    for h in range(2):
                        nc.tensor.transpose(pT[:, h, :], t[ci]["Td"][:, h, :],
                                            ident16)
                    t[ci]["TdT"] = blkg("TdT")
                    nc.scalar.copy(out=t[ci]["TdT"], in_=pT)
                # steps: squaring chain
                for si_, (nm, ln, rn) in enumerate(specs):
                    for ci in cis:
                        pq2 = pd_("pq2")
                        mm2(pq2, lambda h, kt, a=t[ci][ln]: a[:, h, :],
                            lambda h, kt, b_=t[ci][rn]: b_[:, h, :])
                        t[ci][nm] = blkg(nm)
                        if (si_ + ci) % 2 == 0:
                            nc.scalar.copy(out=t[ci][nm], in_=pq2)
                        else:
                            nc.vector.tensor_copy(out=t[ci][nm], in_=pq2)
                # steps: MT product chain -> DT
                for ci in cis:
                    MT = pdc.tile([128, 2, 128], F16, tag="MT", name="MT",
                                  bufs=2 * GRP)
                    nc.vector.tensor_tensor(out=MT, in0=t[ci]["TdT"],
                                            in1=bc_mid(ident16, 2),
                                            op=ALU.add)
                    t[ci]["MT"] = MT
                for nm in ("T2", "T4", "T8", "T16"):
                    for ci in cis:
                        pm = pd_("pm")
                        for h in range(2):
                            nc.tensor.matmul(pm[:, h, :], t[ci][nm][:, h, :],
                                             t[ci]["MT"][:, h, :],
                                             start=True, stop=False)
                            nc.tensor.matmul(pm[:, h, :], ident16,
                                             t[ci]["MT"][:, h, :],
                                             start=False, stop=True)
                        MTn = pdc.tile([128, 2, 128], F16, tag="MT",
                                       name="MT", bufs=2 * GRP)
                        if ci % 2 == 0:
                            nc.scalar.copy(out=MTn, in_=pm)
                        else:
                            nc.vector.tensor_copy(out=MTn, in_=pm)
                        t[ci]["MT"] = MTn
                # steps: B, BT, B2T
                for ci in cis:
                    pB = pd_("pB")
                    mm2(pB, lambda h, kt: t[ci]["MT"][:, h, :],
                        lambda h, kt: t[ci]["To"][:, h, :])
                    t[ci]["Bm"] = blkg("Bm")
                    nc.scalar.copy(out=t[ci]["Bm"], in_=pB)
                for ci in cis:
                    pBT = pd_("pBT")
                    mm2(pBT, lambda h, kt: t[ci]["To"][:, h, :],
                        lambda h, kt: t[ci]["MT"][:, h, :])
                    t[ci]["BT"] = blkg("BT")
                    nc.vector.tensor_copy(out=t[ci]["BT"], in_=pBT)
                for ci in cis:
                    pB2 = pd_("pB2")
                    mm2(pB2, lambda h, kt: t[ci]["Bm"][:, h, :],
                        lambda h, kt: t[ci]["BT"][:, h, :])
                    t[ci]["B2T"] = blkg("B2T")
                    nc.scalar.copy(out=t[ci]["B2T"], in_=pB2)
                # step: aT
                for ci in cis:
                    pA4 = pd_("pA4")
                    mm2(pA4, lambda h, kt: kT[:, h, kt, ls[ci]],
                        lambda h, kt: qT[:, h, kt, ls[ci]], n_k=2)
                    nc.vector.tensor_tensor(out=aT_all[:, ci % 4], in0=pA4,
                                            in1=bc_mid(masks[:, 3, :], 2),
                                            op=ALU.mult)
                # steps: X = [beta*v | beta*k], 3-stage apply
                for ci in cis:
                    X = X_v[ci % 4]
                    for h in range(2):
                        nc.vector.tensor_scalar_mul(
                            X[:, h, 0:256].rearrange("p (a b) -> p a b", a=2),
                            vlc[:, h, :, ci, :], beta_lp[:, ci, h:h+1])
                        nc.vector.tensor_scalar_mul(
                            X[:, h, 256:512].rearrange(
                                "p (a b) -> p a b", a=2),
                            klc[:, h, :, ci, :], beta_lp[:, ci, h:h+1])
                    t[ci]["X"] = X
                for ci in cis:
                    x1t = x1_v[ci % 4]
                    for h in range(2):
                        px = psD.tile([128, 512], F32, tag="pX5", bufs=3,
                                      name="pX1")
                        nc.tensor.matmul(px, t[ci]["MT"][:, h, :],
                                         t[ci]["X"][:, h, :],
                                         start=True, stop=True)
                        if h == 0:
                            nc.scalar.copy(out=x1t[:, h, :], in_=px)
                        else:
                            nc.vector.tensor_copy(out=x1t[:, h, :], in_=px)
                    t[ci]["x1t"] = x1t
                for ci in cis:
                    y1t = y1_v[ci % 4]
                    for h in range(2):
                        px = psD.tile([128, 512], F32, tag="pX5", bufs=3,
                                      name="pX2")
                        nc.tensor.matmul(px, t[ci]["B2T"][:, h, :],
                                         t[ci]["x1t"][:, h, :],
                                         start=True, stop=False)
                        nc.tensor.matmul(px, ident16, t[ci]["x1t"][:, h, :],
                                         start=False, stop=True)
                        if h == 0:
                            nc.vector.tensor_copy(out=y1t[:, h, :], in_=px)
                        else:
                            nc.scalar.copy(out=y1t[:, h, :], in_=px)
                    t[ci]["y1t"] = y1t
                for ci in cis:
                    wtmp = pdw.tile([128, 2, 256], F16, tag="wtmp", bufs=3,
                                    name="wtmp")
                    for h in range(2):
                        px = psD.tile([128, 512], F32, tag="pX5", bufs=3,
                                      name="pX3")
                        nc.tensor.matmul(px, t[ci]["BT"][:, h, :],
                                         t[ci]["y1t"][:, h, :],
                                         start=True, stop=False)
                        nc.tensor.matmul(px, ident16, t[ci]["y1t"][:, h, :],
                                         start=False, stop=True)
                        nc.vector.tensor_copy(out=u_all[:, ci % 4, h, :],
                                              in_=px[:, 0:256])
                        nc.scalar.copy(out=wtmp[:, h, :], in_=px[:, 256:512])
                    t[ci]["wtmp"] = wtmp
                for ci in cis:
                    for kt in range(2):
                        ptw = psD.tile([128, 2, 128], F16, tag="pdT", bufs=1,
                                       name="ptw")
                        for h in range(2):
                            nc.tensor.transpose(
                                ptw[:, h, :],
                                t[ci]["wtmp"][:, h, kt*128:(kt+1)*128],
                                ident16)
                        nc.scalar.mul(out=wT_all[:, ci % 4, kt], in_=ptw,
                                      mul=-1.0)

                ngrp = (NCH + GRP - 1) // GRP
                gi = g0 // GRP
                for fstep in range(32 * gi // ngrp, 32 * (gi + 1) // ngrp):
                    _emit_fusion(fstep)

                # --- serial S part (per chunk) ---------------------------
                for ci in cis:
                    pup = psD.tile([128, 2, d], F32, tag="pS", bufs=1,
                                   name="pup")
                    for h in range(2):
                        for kt in range(2):
                            nc.tensor.matmul(pup[:, h, :],
                                             wT_all[:, ci % 4, kt, h, :],
                                             S16[:, h, kt, :],
                                             start=(kt == 0), stop=False)
                        nc.tensor.matmul(pup[:, h, :], ident16,
                                         u_all[:, ci % 4, h, :],
                                         start=False, stop=True)
                    # pup now holds u - w^T S = upr directly (wT is negated)
                    uprt = pdw.tile([128, 2, d], F16, tag="uprt", bufs=2,
                                    name="uprt")
                    nc.scalar.copy(out=uprt, in_=pup)

                    po = psD.tile([128, 2, d], F32, tag="pS", bufs=1,
                                  name="po")
                    for h in range(2):
                        for kt in range(2):
                            nc.tensor.matmul(po[:, h, :],
                                             qT[:, h, kt, ls[ci]],
                                             S16[:, h, kt, :],
                                             start=(kt == 0), stop=False)
                        nc.tensor.matmul(po[:, h, :], aT_all[:, ci % 4, h, :],
                                         uprt[:, h, :],
                                         start=False, stop=True)
                    dsb = pdw.tile([128, 2, d], F16, tag="dsb", bufs=1,
                                   name="dsb")
                    nc.scalar.copy(out=dsb, in_=po)
                    nc.gpsimd.dma_start(
                        out=sc["dout_s"][ls[ci], :],
                        in_=dsb.rearrange("p h e -> p (h e)"))
                    nc.vector.tensor_reduce(
                        out=bn_sb[:, ci, 2:4], in_=dsb,
                        axis=mybir.AxisListType.X, op=ALU.add,
                        apply_absolute_value=True)

                    for h in range(2):
                        pdS = psD.tile([128, 2, d], F32, tag="pS", bufs=1,
                                       name=f"pdS{h}")
                        for kt in range(2):
                            nc.tensor.matmul(pdS[:, kt, :],
                                             klc[:, h, kt, ci, :],
                                             uprt[:, h, :],
                                             start=True, stop=True)
                        nc.vector.scalar_tensor_tensor(
                            out=S16[:, h], in0=pdS, scalar=1.0,
                            in1=S16[:, h], op0=ALU.mult, op1=ALU.add)

            # ---- bn features finalize + AllGather (three L-segments) -----
            SEGS = ((0, 0.34, 0, 8, "a"), (1, 0.41, 8, 12, "b"),
                    (2, 0.44, 12, 16, "c"))
            for _seg, wait_ms, lt0, lt1, sfx in SEGS:
                ctx_bn = tc.tile_wait_until(wait_ms)
                ctx_bn.__enter__()
                for lt in range(lt0, lt1):
                    cmt_b = pd1.tile([128, C], F16, tag="cmt_b", bufs=3,
                                     name="cmt_b")
                    nc.gpsimd.dma_start(
                        out=cmt_b, in_=sc["cm_out"][lt*128:(lt+1)*128, :])
                    nc.vector.tensor_reduce(
                        out=bn_sb[:, lt, 0:2],
                        in_=cmt_b.rearrange("p (h e) -> p h e", e=d),
                        axis=mybir.AxisListType.X, op=ALU.add,
                        apply_absolute_value=True)
                nc.vector.tensor_reduce(
                    out=bn_sb[:, lt0:lt1, 4:6],
                    in_=bnv4[:, lt0:lt1].rearrange(
                        "p lt (h t) -> p lt h t", t=2),
                    axis=mybir.AxisListType.X, op=ALU.add)
                nc.gpsimd.dma_start(
                    out=sc["bn_in_" + sfx][:].rearrange(
                        "(lt p) c -> p lt c", p=128),
                    in_=bn_sb[:, lt0:lt1, :])
                nc.gpsimd.collective_compute(
                    "AllGather", ALU.bypass, replica_groups=RG,
                    ins=[sc["bn_in_" + sfx][:]],
                    outs=[sc["bn_out_" + sfx][:]])
                ctx_bn.__exit__(None, None, None)

            bnT = [pd1.tile([8, L], F16, name=f"bnT{m}") for m in range(2)]
            for m in range(2):
                for _seg, _w, lt0, lt1, sfx in SEGS:
                    nseg = lt1 - lt0
                    bng = pd1.tile([128, 8, 8], F32, tag="bng", bufs=3,
                                   name=f"bng{m}{sfx}")
                    nc.sync.dma_start(
                        out=bng[:, 0:nseg, :],
                        in_=sc["bn_out_" + sfx][m].rearrange(
                            "(lt p) c -> p lt c", p=128))
                    for lts in range(nseg):
                        lt = lt0 + lts
                        ptb = psD.tile([128, 512], F32, tag="pX5", bufs=3,
                                       name="ptb")
                        ptbv = ptb[0:8, 0:128]
                        nc.tensor.transpose(ptbv, bng[:, lts, :], ident)
                        nc.scalar.mul(out=bnT[m][:, lt*128:(lt+1)*128],
                                      in_=ptbv, mul=1.0/d)
            # ---- fusion tail: bn part + gelu -----------------------------
            fb1_sb = pd1.tile([128, 8], F32)
            nc.sync.dma_start(out=fb1_sb,
                              in_=io["fb1"][:].rearrange("(m p) -> p m",
                                                         p=128))
            fw1b_sb = pd1.tile([8, 2, 1024], F16)
            nc.sync.dma_start(
                out=fw1b_sb,
                in_=io["fw1b"][:].rearrange("(m p) c -> p m c", p=8))
            for lw in range(NLW):
                for mt in range(8):
                    ph2 = psD.tile([128, 512], F32, tag="pX5", bufs=3, name="psh2")
                    for m in range(2):
                        nc.tensor.matmul(ph2,
                                         fw1b_sb[:, m, mt*128:(mt+1)*128],
                                         bnT[m][:, lw*512:(lw+1)*512],
                                         start=(m == 0), stop=(m == 1))
                    hdf = pdw.tile([128, 512], F16, tag="hdf", bufs=3,
                                   name="hdf")
                    nc.vector.scalar_tensor_tensor(
                        out=hdf, in0=ph2, scalar=1.0,
                        in1=hdn_pre[:, mt, lw*512:(lw+1)*512],
                        op0=ALU.mult, op1=ALU.add)
                    nc.scalar.activation(
                        out=hdn_pre[:, mt, lw*512:(lw+1)*512], in_=hdf,
                        func=AF.Gelu, bias=fb1_sb[:, mt:mt+1])

            fw2_sb = pd1.tile([128, 8, 12], F16)
            nc.sync.dma_start(
                out=fw2_sb,
                in_=io["fw2"][:].rearrange("(kt p) c -> p kt c", p=128))
            lg_sb = pd1.tile([128, NLT, 12], F32)
            for lt in range(NLT):
                pl = psD.tile([128, 512], F32, tag="pX5", bufs=3, name="psl")
                plv = pl[:, 0:12]
                for kt in range(8):
                    nc.tensor.matmul(plv, hdn_pre[:, kt, lt*128:(lt+1)*128],
                                     fw2_sb[:, kt, :],
                                     start=(kt == 0), stop=(kt == 7))
                nc.scalar.copy(out=lg_sb[:, lt, :], in_=plv)
            for m in range(2):
                nc.gpsimd.dma_start(
                    out=sc["lg_in"][m].rearrange("(lt p) c -> p lt c", p=128),
                    in_=lg_sb[:, :, m*6:(m+1)*6])
            nc.gpsimd.collective_compute(
                "ReduceScatter", ALU.add, replica_groups=RG,
                ins=[sc["lg_in"][:]], outs=[sc["lg_out"][:]])

        # =================== PHASE C ======================================
        with tc.tile_pool(name="pc1", bufs=1) as pc1, \
             tc.tile_pool(name="pc2", bufs=2) as pc2, \
             tc.tile_pool(name="pcs", bufs=4) as pcs, \
             tc.tile_pool(name="psC", bufs=1, space="PSUM") as psC:

            def psc(name, tag="pg"):
                return psC.tile([128, 512], F32, tag=tag, bufs=2, name=name)

            # ---- softmax gates -------------------------------------------
            b2_sb = pc1.tile([128, 6], F32)
            nc.sync.dma_start(out=b2_sb, in_=io["b2o"][:])
            lgo = pc1.tile([128, NLT, 2, 3], F32)
            nc.sync.dma_start(
                out=lgo,
                in_=sc["lg_out"][:].rearrange("(lt p) (h e) -> p lt h e",
                                              p=128, e=3))
            nc.vector.tensor_tensor(
                out=lgo, in0=lgo,
                in1=bass.AP(tensor=b2_sb.tensor, offset=b2_sb.offset,
                            ap=[b2_sb.ap[0], [0, NLT], [3, 2], [1, 3]]),
                op=ALU.add)
            rmax = pc1.tile([128, NLT, 2], F32)
            nc.vector.tensor_reduce(out=rmax, in_=lgo,
                                    axis=mybir.AxisListType.X, op=ALU.max)
            nc.vector.tensor_tensor(
                out=lgo, in0=lgo,
                in1=rmax[:, :, :, None].to_broadcast([128, NLT, 2, 3]),
                op=ALU.subtract)
            nc.scalar.activation(out=lgo, in_=lgo, func=AF.Exp)
            rsum = pc1.tile([128, NLT, 2], F32)
            nc.vector.tensor_reduce(out=rsum, in_=lgo,
                                    axis=mybir.AxisListType.X, op=ALU.add)
            nc.vector.reciprocal(out=rsum, in_=rsum)
            nc.vector.tensor_tensor(
                out=lgo, in0=lgo,
                in1=rsum[:, :, :, None].to_broadcast([128, NLT, 2, 3]),
                op=ALU.mult)

            # ---- gate mix + RMSNorm + Wo ---------------------------------
            wo_sb = pc1.tile([128, 4, D], F16)
            nc.sync.dma_start(
                out=wo_sb,
                in_=io["wo"][:].rearrange("(kt p) n -> p kt n", p=128))
            dout_sb = arena[:, 0:8192].rearrange("p (lt h e) -> p lt h e",
                                                 h=2, e=d)
            cm_sb = arena[:, 8192:16384].rearrange("p (lt c) -> p lt c",
                                                   c=C)
            nc.gpsimd.dma_start(
                out=cm_sb,
                in_=sc["cm_out"][:].rearrange("(lt p) c -> p lt c", p=128))
            nc.gpsimd.dma_start(
                out=dout_sb,
                in_=sc["dout_s"][:].rearrange("(lt p) (h e) -> p lt h e",
                                              p=128, e=d))
            for lt in range(NLT):
                dov = dout_sb[:, lt]
                o_t = pcs.tile([128, 2, d], F16, tag="o_t", name="o_t")
                ssq = pcs.tile([128, 2], F32, tag="ssq", name="ssq")
                scr = pcs.tile([128, d], F32, tag="scr", name="scr")
                for h in range(2):
                    nc.vector.tensor_scalar_mul(
                        o_t[:, h, :],
                        cm_sb[:, lt, h*256:(h+1)*256], lgo[:, lt, h, 0:1])
                    nc.vector.scalar_tensor_tensor(
                        out=o_t[:, h, :], in0=dov[:, h, :],
                        scalar=lgo[:, lt, h, 1:2], in1=o_t[:, h, :],
                        op0=ALU.mult, op1=ALU.add)
                    nc.vector.scalar_tensor_tensor(
                        out=o_t[:, h, :].rearrange("p (a b) -> p a b", a=2),
                        in0=vlc[:, h, :, lt, :],
                        scalar=lgo[:, lt, h, 2:3],
                        in1=o_t[:, h, :].rearrange("p (a b) -> p a b", a=2),
                        op0=ALU.mult, op1=ALU.add)
                    nc.scalar.activation(out=scr, in_=o_t[:, h, :],
                                         func=AF.Square,
                                         accum_out=ssq[:, h:h+1])
                nc.scalar.activation(out=ssq, in_=ssq, func=AF.Sqrt,
                                     scale=1.0/d, bias=eps5)
                nc.vector.reciprocal(out=ssq, in_=ssq)
                for h in range(2):
                    nc.vector.tensor_scalar_mul(o_t[:, h, :], o_t[:, h, :],
                                                ssq[:, h:h+1])
                pto = psC.tile([128, 4, 128], F16, tag="pto", bufs=2,
                               name="psto")
                for ct in range(4):
                    h, dt = ct // 2, ct % 2
                    nc.tensor.transpose(pto[:, ct, :],
                                        o_t[:, h, dt*128:(dt+1)*128],
                                        ident16)
                oT = pcs.tile([128, 4, 128], F16, tag="oT", name="oT")
                nc.vector.tensor_copy(out=oT, in_=pto)
                orow = pcs.tile([128, D], F32, tag="orow", name="orow")
                for nh in range(2):
                    pw = psC.tile([128, 512], F32, tag="pw", bufs=2, name="psw")
                    for ct in range(4):
                        nc.tensor.matmul(pw, oT[:, ct, :],
                                         wo_sb[:, ct, nh*512:(nh+1)*512],
                                         start=(ct == 0), stop=(ct == 3))
                    nc.scalar.copy(out=orow[:, nh*512:(nh+1)*512], in_=pw)
                nc.sync.dma_start(out=io["out_part"][lt*128:(lt+1)*128, :],
                                  in_=orow)


# ======================= host side =======================================

def _diag_tiles(w_own, taps, out_dtype):
    """w_own: (C, k) conv weights for this core's channels.
    Returns (4, k, 128, 128) diag tiles; tap dd uses column k-1-dd."""
    k = w_own.shape[1]
    out = np.zeros((4, k, 128, 128), dtype=out_dtype)
    for ct in range(4):
        for dd in range(k):
            np.fill_diagonal(out[ct, dd], w_own[ct*128:(ct+1)*128, k-1-dd])
    return out


def _host_inputs(inputs):
    hs = np.asarray(inputs["hidden_states"], np.float32)
    Wq = np.asarray(inputs["Wq"], np.float32)
    Wk = np.asarray(inputs["Wk"], np.float32)
    Wv = np.asarray(inputs["Wv"], np.float32)
    Wb = np.asarray(inputs["Wb"], np.float32)
    cq = np.asarray(inputs["conv_q_w"], np.float32)
    ck = np.asarray(inputs["conv_k_w"], np.float32)
    cv = np.asarray(inputs["conv_v_w"], np.float32)
    w3 = np.asarray(inputs["ms_w3"], np.float32)
    w15 = np.asarray(inputs["ms_w15"], np.float32)
    w31 = np.asarray(inputs["ms_w31"], np.float32)
    kmix = np.asarray(inputs["kernel_mix_w"], np.float32)
    cmix = np.asarray(inputs["channel_mixer_w"], np.float32)
    fw1 = np.asarray(inputs["fusion_w1"], np.float32)
    fb1 = np.asarray(inputs["fusion_b1"], np.float32)
    fw2 = np.asarray(inputs["fusion_w2"], np.float32)
    fb2 = np.asarray(inputs["fusion_b2"], np.float32)
    onw = np.asarray(inputs["o_norm_w"], np.float32)
    Wo = np.asarray(inputs["Wo"], np.float32)

    # combined kernel_mix -> channel_mixer matrix Q: (3D, D)
    Q = np.zeros((3 * D, D), np.float32)
    for h in range(H):
        Q[h*3*d:(h+1)*3*d] = kmix @ cmix[h*d:(h+1)*d]

    masks = np.zeros((5, 128, 128), np.float32)
    i_, j_ = np.mgrid[0:128, 0:128]
    blk = (i_ // 32) == (j_ // 32)
    masks[0] = -((i_ > j_) & blk).astype(np.float32)
    masks[1] = -((i_ > j_) & ~blk).astype(np.float32)
    masks[2] = -((j_ > i_) & blk).astype(np.float32)
    masks[3] = (j_ >= i_).astype(np.float32)
    masks[4] = np.eye(128, dtype=np.float32)

    Wo_s = Wo * np.tile(onw, H)[:, None]

    in_maps = []
    for c in range(8):
        b, r = divmod(c, 2)
        cs = slice(C*r, C*(r+1))
        qmix = np.concatenate(
            [Q[1024*s + C*r: 1024*s + C*r + C] for s in range(3)], 0)
        f8 = ml_dtypes.float8_e4m3
        mspair = np.zeros((4, 23, 2, 128, 128), np.float32)
        mssing = np.zeros((4, 3, 128, 128), np.float32)
        for ct in range(4):
            pi = 0
            for si, (w, ks) in enumerate(((w3, 3), (w15, 15), (w31, 31))):
                dg = _diag_tiles(w[cs], ks, np.float32)[ct] * 64.0
                for p_ in range((ks - 1) // 2):
                    mspair[ct, pi, 0] = dg[2*p_ + 1]
                    mspair[ct, pi, 1] = dg[2*p_]
                    pi += 1
                mssing[ct, si] = dg[ks - 1]
        cdiag = np.stack([_diag_tiles(w[cs], KQKV, np.float16)
                          for w in (cq, ck, cv)], 0)
        fw1b = np.zeros((16, 1024), np.float32)
        for m in range(2):
            for src in range(3):
                for h_ in range(2):
                    fw1b[m*8 + src*2 + h_] = \
                        fw1[D + src*4 + 2*m + h_, 1024*r:1024*(r+1)]
        fw2p = np.zeros((1024, 12), np.float32)
        b2o = np.zeros((6,), np.float32)
        for jm in range(2):
            for h_ in range(2):
                for br in range(3):
                    gcol = (2*jm + h_)*3 + br
                    fw2p[:, jm*6 + h_*3 + br] = fw2[1024*r:1024*(r+1), gcol]
        for h_ in range(2):
            for br in range(3):
                b2o[h_*3 + br] = fb2[(2*r + h_)*3 + br]
        m = {
            "hsT": np.ascontiguousarray(hs[b].T).astype(np.float16),
            "wq": np.ascontiguousarray(Wq[:, cs]).astype(np.float16),
            "wk": np.ascontiguousarray(Wk[:, cs]).astype(np.float16),
            "wv": np.ascontiguousarray(Wv[:, cs]).astype(np.float16),
            "wb": np.ascontiguousarray(Wb[:, 2*r:2*r+2]).astype(np.float16),
            "cdiag": cdiag,
            "mspair": mspair.astype(f8),
            "mssing": mssing.astype(f8),
            "qmix8": (qmix * 64.0).astype(f8),
            "fw1h": np.ascontiguousarray(
                fw1[:D, 1024*r:1024*(r+1)]).astype(np.float16),
            "fw1b": fw1b.astype(np.float16),
            "fb1": np.ascontiguousarray(fb1[1024*r:1024*(r+1)]),
            "fw2": fw2p.astype(np.float16),
            "b2o": np.tile(b2o, (128, 1)),
            "wo": np.ascontiguousarray(Wo_s[cs, :]).astype(np.float16),
            "masks": masks,
            "onesrow": np.ones((1, 128), np.float32),
            "onescol": np.ones((128, 1), np.float32),
            "ident16": np.eye(128, dtype=np.float16),
        }
        in_maps.append(m)
    return in_maps


_PROG = {}


def _get_program(debug=False):
    key = bool(debug)
    if key not in _PROG:
        _PROG[key] = build_program(debug=debug)
    return _PROG[key]


def run(inputs, debug=False, **kw):
    nc = _get_program(debug=debug)
    in_maps = _host_inputs(inputs)
    res = run_bass_kernel_spmd(nc, in_maps, list(range(8)), **kw)
    return res


def kernel(**inputs):
    res = run(inputs)
    out = np.zeros((B, L, D), np.float32)
    for b in range(B):
        out[b] = res.results[2*b]["out_part"] + res.results[2*b+1]["out_part"]
    return out


if __name__ == "__main__":
    nc = build_program()
    print("program built ok")
